# revision 19
# baseline (speedup 1.0000x reference)
"""Trainium2 Bass kernel for nn_ActorCritic (GNN message passing + actor/critic MLPs).

Sharding: nodes are partitioned across the 8 NeuronCores (2500 nodes each, all
8 batch elements on every core). The one-hop segment_sums (nb, nb_rev) are
computed as dense fp8 matmuls against host-built adjacency blocks:
    nbT[pay, dst] = sum_src X[src, pay] * A[src, dst]
with X = one-hot colors built on device (payload = batch*32 + color, 256 wide,
split into hi/lo 128-partition planes) and A the 0/1 adjacency (pure index
data). The MLPs run with float32r (layer 0) and bfloat16 (layers 1/2) matmuls;
log-softmax / entropy / value reductions run on DVE/ACT after a PE transpose
back to node-major layout. Outputs are gathered and re-assembled on host.
"""
import os
import sys

sys.path.insert(0, "/opt/trn_rl_repo")
sys.path.insert(0, os.path.dirname(os.path.abspath(__file__)))

import numpy as np
import ml_dtypes

import concourse.bass as bass
import concourse.mybir as mybir
import concourse.tile as tile_mod
from concourse.tile import TileContext
from concourse.bass_utils import run_bass_kernel_spmd
from concourse.masks import make_identity
from concourse.vector_clock import ScopedClock


# ---------------------------------------------------------------------------
# Walrus-compat patches: this build rejects >1 sem wait per instruction, and
# the stock TileContext tail drain carries one wait per live logical proc.
# ---------------------------------------------------------------------------

MAX_WAITS = 1


def _patched_drain_and_barrier(self, tick_clock, wait_clock):
    nc = self.nc
    probe = nc.sync.nop()
    wait_clock.add_sem_waits(probe.ins, ScopedClock({None: tick_clock.global_clock}))
    si = probe.ins.sync_info
    waits = list(si.on_wait) if si is not None else []
    if len(waits) > MAX_WAITS:
        si.on_wait = waits[:MAX_WAITS]
        rest = waits[MAX_WAITS:]
        for j in range(0, len(rest), MAX_WAITS):
            n = nc.sync.nop()
            nsi = n.ins.sync_info
            if nsi is None:
                n.ins.sync_info = mybir.SyncInfo(
                    on_update=[], on_wait=rest[j : j + MAX_WAITS]
                )
            else:
                nsi.on_wait = rest[j : j + MAX_WAITS]
    nc.sync.drain()
    nc.all_engine_barrier()
    assert self.sems is not None
    popped = nc._tile_sem_poison_stack.pop()
    assert popped is self._sem_poison
    nc.clear_and_free_semaphores(list(self.sems.allocated().values()))
    nc.all_engine_barrier()


_ws_counter = [0]


def fix_waits(nc, max_waits: int = 1):
    """Post-pass over the finished module: any instruction carrying more than
    ``max_waits`` sem waits gets the excess hoisted onto same-engine NoOps
    inserted immediately before it (this walrus build rejects multi-wait
    instructions at codegen)."""
    for f in nc.m.functions:
        for blk in f.blocks:
            insns = blk.instructions
            out = []
            changed = False
            for ins in insns:
                si = ins.sync_info
                if si is not None and len(si.on_wait) > max_waits:
                    waits = list(si.on_wait)
                    keep = waits[: max_waits]
                    rest = waits[max_waits:]
                    for j in range(0, len(rest), max_waits):
                        _ws_counter[0] += 1
                        nop = mybir.InstNoOp(
                            name=f"WSPLIT-{_ws_counter[0]}",
                            ins=[],
                            outs=[],
                            engine=ins.engine,
                            sync_info=mybir.SyncInfo(
                                on_update=[], on_wait=rest[j : j + max_waits]
                            ),
                        )
                        out.append(nop)
                    si.on_wait = keep
                    changed = True
                out.append(ins)
            if changed:
                blk.instructions = out


def install():
    tile_mod.TileContext._drain_and_barrier = _patched_drain_and_barrier


class _TP:
    fix_waits = staticmethod(fix_waits)


tile_patch = _TP()
install()

# ---- problem constants (hardcoded per spec) ----
N, B, NCOL, H, E = 20000, 8, 20, 1024, 320000
IN_D, OUT_D, MAXN = 42, 21, 20000
NCORES = 8
NPC = N // NCORES            # 2500 nodes per core
NPCP = 2560                  # padded: 20 x 128 = 5 x 512
NT = NPCP // 128             # 20 node tiles per core
NRANGE = NPCP // 512         # 5 psum ranges
KT = 157                     # src k-chunks (ceil(20000/128))
NP = KT * 128                # 20096 padded source nodes
NBLK = 2 * NRANGE * KT       # 1570 adjacency blocks per core

f32 = mybir.dt.float32
f32r = mybir.dt.float32r
bf16 = mybir.dt.bfloat16
fp8 = mybir.dt.float8e4
i32 = mybir.dt.int32

_FP8_LUT = np.arange(256, dtype=np.uint8).astype(np.float32).astype(
    ml_dtypes.float8_e4m3)

LAST_EXEC_NS = None
_CACHED = {}


def _build_program():
    nc = bass.Bass("TRN2")
    p = {}
    p["obx_g"] = nc.declare_dram_parameter("obx_g", [N, B], i32, isOutput=False)
    p["obtT"] = nc.declare_dram_parameter("obtT", [B + 1, NPCP], f32, isOutput=False)
    p["obxN"] = nc.declare_dram_parameter("obxN", [B, 128, NT], i32, isOutput=False)
    p["actN"] = nc.declare_dram_parameter("actN", [B, 128, NT], i32, isOutput=False)
    p["a_all"] = nc.declare_dram_parameter("a_all", [NBLK, 128, 512], fp8, isOutput=False)
    p["iota32"] = nc.declare_dram_parameter("iota32", [128, 32], i32, isOutput=False)
    p["iota21"] = nc.declare_dram_parameter("iota21", [128, 21], f32, isOutput=False)
    p["w0a"] = nc.declare_dram_parameter("w0a", [IN_D, H], f32, isOutput=False)
    p["w0c"] = nc.declare_dram_parameter("w0c", [IN_D, H], f32, isOutput=False)
    p["w1a"] = nc.declare_dram_parameter("w1a", [128, 64, 128], bf16, isOutput=False)
    p["w1c"] = nc.declare_dram_parameter("w1c", [128, 64, 128], bf16, isOutput=False)
    p["w2a"] = nc.declare_dram_parameter("w2a", [128, 8, OUT_D], bf16, isOutput=False)
    p["w2c"] = nc.declare_dram_parameter("w2c", [128, 8, 1], bf16, isOutput=False)
    p["b0a"] = nc.declare_dram_parameter("b0a", [128, 8], f32, isOutput=False)
    p["b0c"] = nc.declare_dram_parameter("b0c", [128, 8], f32, isOutput=False)
    p["b1a"] = nc.declare_dram_parameter("b1a", [128, 8], f32, isOutput=False)
    p["b1c"] = nc.declare_dram_parameter("b1c", [128, 8], f32, isOutput=False)
    p["b2cat"] = nc.declare_dram_parameter("b2cat", [64, 1], f32, isOutput=False)
    alp_out = nc.declare_dram_parameter("alp_out", [B, 128, NT], f32, isOutput=True)
    scal_out = nc.declare_dram_parameter("scal_out", [1, 10], f32, isOutput=True)

    with TileContext(nc) as tc:
        with (
            tc.tile_pool(name="per", bufs=1) as per,       # persistent
            tc.tile_pool(name="nbp", bufs=1) as nbp,       # nb storage
        ):
            # ---- persistent small tiles ----
            iota32_t = per.tile([128, 32], i32)
            nc.sync.dma_start(out=iota32_t[:], in_=p["iota32"][:, :])
            iota21_t = per.tile([128, 21], f32)
            nc.sync.dma_start(out=iota21_t[:], in_=p["iota21"][:, :])
            ident_f32 = per.tile([128, 128], f32)
            make_identity(nc, ident_f32[:])
            ones_col = per.tile([128, 1], f32)
            nc.vector.memset(ones_col[:], 1.0)
            acc_all = per.tile([128, 10], f32)
            nc.vector.memset(acc_all[:], 0.0)
            b2cat_t = per.tile([64, 1], f32)
            nc.sync.dma_start(out=b2cat_t[:], in_=p["b2cat"][:, :])

            # weights
            w0cat_t = per.tile([64 + IN_D, H], f32r)
            nc.gpsimd.dma_start(out=w0cat_t[0:IN_D, :], in_=p["w0a"][:, :])
            nc.gpsimd.dma_start(out=w0cat_t[64 : 64 + IN_D, :], in_=p["w0c"][:, :])
            w1a_t = per.tile([128, 64, 128], bf16)
            nc.sync.dma_start(out=w1a_t[:], in_=p["w1a"][:, :, :])
            w1c_t = per.tile([128, 64, 128], bf16)
            nc.sync.dma_start(out=w1c_t[:], in_=p["w1c"][:, :, :])
            w2a_t = per.tile([128, 8, OUT_D], bf16)
            nc.sync.dma_start(out=w2a_t[:], in_=p["w2a"][:, :, :])
            w2c_t = per.tile([128, 8, 1], bf16)
            nc.sync.dma_start(out=w2c_t[:], in_=p["w2c"][:, :, :])
            biases = {}
            for nm in ("b0a", "b0c", "b1a", "b1c"):
                t = per.tile([128, 8], f32, name=nm)
                nc.sync.dma_start(out=t[:], in_=p[nm][:, :])
                biases[nm] = t

            # nb storage: [payload(4b x 32), node] f32
            nb_hi = nbp.tile([128, NPCP], f32)
            nb_lo = nbp.tile([128, NPCP], f32)
            nbr_hi = nbp.tile([128, NPCP], f32)
            nbr_lo = nbp.tile([128, NPCP], f32)
            nbt = {(0, 0): nb_hi, (0, 1): nb_lo, (1, 0): nbr_hi, (1, 1): nbr_lo}

            # ---- phase 0: build one-hot X in SBUF ----
            with tc.tile_pool(name="xp", bufs=1) as xp:
                x_sb = xp.tile([128, KT, 256], fp8)
                nc.vector.memset(x_sb[:, :, :], 0.0)
                obx_sb = xp.tile([128, KT, 8], i32)
                nc.sync.dma_start(
                    out=obx_sb[:, : KT - 1, :],
                    in_=p["obx_g"][0 : (KT - 1) * 128, :].rearrange(
                        "(t q) b -> q t b", q=128
                    ),
                )
                # last partial tile: rows 19968..19999 (32 rows)
                nc.sync.dma_start(
                    out=obx_sb[:32, KT - 1, :],
                    in_=p["obx_g"][(KT - 1) * 128 :, :],
                )
                for t in range(KT):
                    hi = 128 if t < KT - 1 else 32
                    nc.vector.tensor_tensor(
                        out=x_sb[:hi, t, :],
                        in0=obx_sb[:hi, t, :].rearrange(
                            "p (b one) -> p b one", one=1
                        ).to_broadcast([hi, 8, 32]),
                        in1=iota32_t[:hi, :].rearrange(
                            "p (one j) -> p one j", one=1
                        ).to_broadcast([hi, 8, 32]),
                        op=mybir.AluOpType.is_equal,
                    )

                # ---- phase 1: dense scatter matmuls ----
                with (
                    tc.tile_pool(name="ab", bufs=4) as ab,
                    tc.tile_pool(name="scps", bufs=3, space="PSUM") as scps,
                ):
                    for pas in range(2):
                        for r in range(NRANGE):
                            ps_hi = scps.tile([128, 512], f32, space="PSUM", tag="hi")
                            ps_lo = scps.tile([128, 512], f32, space="PSUM", tag="lo")
                            for k0 in range(0, KT, 16):
                                kk = min(16, KT - k0)
                                blk = (pas * NRANGE + r) * KT + k0
                                a_t = ab.tile([128, 16, 512], fp8, tag="a")
                                nc.sync.dma_start(
                                    out=a_t[:, :kk, :],
                                    in_=p["a_all"][blk : blk + kk, :, :].rearrange(
                                        "k q n -> q k n"),
                                )
                                for j0 in range(0, kk, 2):
                                    k0j = k0 + j0
                                    jj = min(2, kk - j0)
                                    if jj == 2:
                                        nc.tensor.matmul(
                                            ps_hi[:],
                                            lhsT=x_sb[:, k0j : k0j + 2, 0:128],
                                            rhs=a_t[:, j0 : j0 + 2, :],
                                            start=(k0j == 0),
                                            stop=(k0j + 2 == KT),
                                            perf_mode=mybir.MatmulPerfMode.DoubleRow,
                                        )
                                        nc.tensor.matmul(
                                            ps_lo[:],
                                            lhsT=x_sb[:, k0j : k0j + 2, 128:256],
                                            rhs=a_t[:, j0 : j0 + 2, :],
                                            start=(k0j == 0),
                                            stop=(k0j + 2 == KT),
                                            perf_mode=mybir.MatmulPerfMode.DoubleRow,
                                        )
                                    else:
                                        nc.tensor.matmul(
                                            ps_hi[:],
                                            lhsT=x_sb[:, k0j, 0:128],
                                            rhs=a_t[:, j0, :],
                                            start=(k0j == 0),
                                            stop=(k0j == KT - 1),
                                        )
                                        nc.tensor.matmul(
                                            ps_lo[:],
                                            lhsT=x_sb[:, k0j, 128:256],
                                            rhs=a_t[:, j0, :],
                                            start=(k0j == 0),
                                            stop=(k0j == KT - 1),
                                        )
                            nc.vector.tensor_copy(
                                out=nbt[(pas, 0)][:, r * 512 : (r + 1) * 512],
                                in_=ps_hi[:],
                            )
                            nc.vector.tensor_copy(
                                out=nbt[(pas, 1)][:, r * 512 : (r + 1) * 512],
                                in_=ps_lo[:],
                            )

            # ---- phase 2: MLPs + post ----
            with (
                tc.tile_pool(name="mlp", bufs=1) as mlp,
                tc.tile_pool(name="post", bufs=2) as post,
                tc.tile_pool(name="l0ps", bufs=4, space="PSUM") as l0ps,
                tc.tile_pool(name="l1ps", bufs=2, space="PSUM") as l1ps,
                tc.tile_pool(name="l2ps", bufs=1, space="PSUM") as l2ps,
                tc.tile_pool(name="trps", bufs=1, space="PSUM") as trps,
            ):
                alp_sb = mlp.tile([128, B * NT], f32)
                for b in range(B):
                    bq = (b % 4) * 32
                    plane = b // 4
                    hT = mlp.tile([64 + IN_D, NPCP], f32r, tag="hT", bufs=2)
                    for base in (0, 64):
                        nc.gpsimd.dma_start(
                            out=hT[base : base + 1, :], in_=p["obtT"][b : b + 1, :])
                        nc.gpsimd.dma_start(
                            out=hT[base + 1 : base + 21, :],
                            in_=nbt[(0, plane)][bq : bq + 20, :])
                        nc.gpsimd.dma_start(
                            out=hT[base + 21 : base + 41, :],
                            in_=nbt[(1, plane)][bq : bq + 20, :])
                        nc.gpsimd.dma_start(
                            out=hT[base + 41 : base + 42, :],
                            in_=p["obtT"][B : B + 1, :])

                    maskf = post.tile([128, NT], f32, tag="maskf")
                    obxn = post.tile([128, NT], i32, tag="obxn")
                    nc.sync.dma_start(out=obxn[:], in_=p["obxN"][b, :, :])
                    nc.vector.tensor_scalar(
                        out=maskf[:], in0=obxn[:], scalar1=0, scalar2=None,
                        op0=mybir.AluOpType.is_equal,
                    )
                    actf = post.tile([128, NT], f32, tag="actf")
                    actn = post.tile([128, NT], i32, tag="actn")
                    nc.sync.dma_start(out=actn[:], in_=p["actN"][b, :, :])
                    nc.vector.tensor_copy(out=actf[:], in_=actn[:])
                    msum = post.tile([128, 1], f32, tag="msum")
                    nc.vector.tensor_reduce(
                        out=msum[:], in_=maskf[:], axis=mybir.AxisListType.X,
                        op=mybir.AluOpType.add,
                    )
                    nc.vector.tensor_tensor(
                        out=acc_all[:, 9:10], in0=acc_all[:, 9:10], in1=msum[:],
                        op=mybir.AluOpType.add,
                    )

                    for rt in range(NRANGE):
                        cs = rt * 512
                        h0a = mlp.tile([128, 8, 512], bf16, tag="h0a", bufs=2)
                        h0c = mlp.tile([128, 8, 512], bf16, tag="h0c", bufs=2)
                        for m in range(8):
                            psa = l0ps.tile([128, 512], f32, space="PSUM", tag="l0")
                            psc = l0ps.tile([128, 512], f32, space="PSUM", tag="l0")
                            nc.tensor.matmul(
                                psa[:],
                                lhsT=w0cat_t[0:IN_D, m * 128 : (m + 1) * 128],
                                rhs=hT[0:IN_D, cs : cs + 512],
                                start=True, stop=True,
                                tile_position=(0, 0),
                            )
                            nc.tensor.matmul(
                                psc[:],
                                lhsT=w0cat_t[64 : 64 + IN_D, m * 128 : (m + 1) * 128],
                                rhs=hT[64 : 64 + IN_D, cs : cs + 512],
                                start=True, stop=True,
                                tile_position=(64, 0),
                            )
                            nc.scalar.activation(
                                out=h0a[:, m, :], in_=psa[:],
                                func=mybir.ActivationFunctionType.Relu,
                                bias=biases["b0a"][:, m : m + 1],
                            )
                            nc.vector.tensor_scalar(
                                out=h0c[:, m, :], in0=psc[:],
                                scalar1=biases["b0c"][:, m : m + 1], scalar2=0.0,
                                op0=mybir.AluOpType.add, op1=mybir.AluOpType.max,
                            )
                        h1a = mlp.tile([128, 8, 512], bf16, tag="h1a", bufs=2)
                        h1c = mlp.tile([128, 8, 512], bf16, tag="h1c", bufs=2)
                        for m in range(8):
                            ps = l1ps.tile([128, 512], f32, space="PSUM", tag="l1")
                            for k in range(8):
                                nc.tensor.matmul(
                                    ps[:],
                                    lhsT=w1a_t[:, k * 8 + m, :],
                                    rhs=h0a[:, k, :],
                                    start=(k == 0), stop=(k == 7),
                                )
                            nc.scalar.activation(
                                out=h1a[:, m, :], in_=ps[:],
                                func=mybir.ActivationFunctionType.Relu,
                                bias=biases["b1a"][:, m : m + 1],
                            )
                        for m in range(8):
                            ps = l1ps.tile([128, 512], f32, space="PSUM", tag="l1")
                            for k in range(8):
                                nc.tensor.matmul(
                                    ps[:],
                                    lhsT=w1c_t[:, k * 8 + m, :],
                                    rhs=h0c[:, k, :],
                                    start=(k == 0), stop=(k == 7),
                                )
                            nc.vector.tensor_scalar(
                                out=h1c[:, m, :], in0=ps[:],
                                scalar1=biases["b1c"][:, m : m + 1], scalar2=0.0,
                                op0=mybir.AluOpType.add, op1=mybir.AluOpType.max,
                            )
                        ps2 = l2ps.tile([64, 512], f32, space="PSUM", tag="l2")
                        for k in range(8):
                            nc.tensor.matmul(
                                ps2[0:OUT_D, :], lhsT=w2a_t[:, k, :], rhs=h1a[:, k, :],
                                start=(k == 0), stop=(k == 7),
                            )
                        for k in range(8):
                            nc.tensor.matmul(
                                ps2[32:33, :], lhsT=w2c_t[:, k, :], rhs=h1c[:, k, :],
                                start=(k == 0), stop=(k == 7),
                            )
                        catT = post.tile([64, 512], f32, tag="catT")
                        nc.vector.tensor_scalar(
                            out=catT[0:33, :], in0=ps2[0:33, :],
                            scalar1=b2cat_t[0:33, :], scalar2=None,
                            op0=mybir.AluOpType.add,
                        )
                        for ntile in range(4):
                            tg = rt * 4 + ntile
                            trp = trps.tile([128, 64], f32, space="PSUM", tag="tr")
                            nc.tensor.transpose(
                                out=trp[:],
                                in_=catT[:, ntile * 128 : (ntile + 1) * 128],
                                identity=ident_f32[0:64, 0:64],
                            )
                            ln = post.tile([128, 33], f32, tag="ln")
                            nc.vector.tensor_copy(out=ln[:], in_=trp[:, 0:33])
                            lg = ln[:, 0:OUT_D]
                            mx = post.tile([128, 1], f32, tag="mx")
                            nc.vector.tensor_reduce(
                                out=mx[:], in_=lg, axis=mybir.AxisListType.X,
                                op=mybir.AluOpType.max,
                            )
                            nmx = post.tile([128, 1], f32, tag="nmx")
                            nc.vector.tensor_scalar_mul(nmx[:], mx[:], -1.0)
                            ex = post.tile([128, OUT_D], f32, tag="ex")
                            s = post.tile([128, 1], f32, tag="s")
                            nc.scalar.activation(
                                out=ex[:], in_=lg,
                                func=mybir.ActivationFunctionType.Exp,
                                bias=nmx[:], accum_out=s[:],
                            )
                            logs = post.tile([128, 1], f32, tag="logs")
                            nc.scalar.activation(
                                out=logs[:], in_=s[:],
                                func=mybir.ActivationFunctionType.Ln,
                            )
                            sel = post.tile([128, OUT_D], f32, tag="sel")
                            nc.vector.tensor_scalar(
                                out=sel[:], in0=iota21_t[:],
                                scalar1=actf[:, tg : tg + 1], scalar2=None,
                                op0=mybir.AluOpType.is_equal,
                            )
                            junk = post.tile([128, OUT_D], f32, tag="junk")
                            asel = post.tile([128, 1], f32, tag="asel")
                            nc.vector.tensor_tensor(
                                out=junk[:], in0=sel[:], in1=lg,
                                op=mybir.AluOpType.mult,
                            )
                            nc.vector.tensor_reduce(
                                out=asel[:], in_=junk[:],
                                axis=mybir.AxisListType.X, op=mybir.AluOpType.add,
                            )
                            junk2 = post.tile([128, OUT_D], f32, tag="junk2")
                            t3 = post.tile([128, 1], f32, tag="t3")
                            nc.vector.tensor_tensor(
                                out=junk2[:], in0=ex[:], in1=lg,
                                op=mybir.AluOpType.mult,
                            )
                            nc.vector.tensor_reduce(
                                out=t3[:], in_=junk2[:],
                                axis=mybir.AxisListType.X, op=mybir.AluOpType.add,
                            )
                            # alp = (asel - mx - logs) * mask
                            alp0 = post.tile([128, 1], f32, tag="alp0")
                            nc.vector.tensor_scalar(
                                out=alp0[:], in0=asel[:], scalar1=mx[:],
                                scalar2=None, op0=mybir.AluOpType.subtract,
                            )
                            nc.vector.tensor_tensor(
                                out=alp0[:], in0=alp0[:], in1=logs[:],
                                op=mybir.AluOpType.subtract,
                            )
                            nc.vector.tensor_tensor(
                                out=alp_sb[:, b * NT + tg : b * NT + tg + 1],
                                in0=alp0[:], in1=maskf[:, tg : tg + 1],
                                op=mybir.AluOpType.mult,
                            )
                            # ent = mx + logs - t3 / s
                            rs = post.tile([128, 1], f32, tag="rs")
                            nc.vector.reciprocal(rs[:], s[:])
                            ent0 = post.tile([128, 1], f32, tag="ent0")
                            nc.vector.tensor_tensor(
                                out=ent0[:], in0=t3[:], in1=rs[:],
                                op=mybir.AluOpType.mult,
                            )
                            nc.vector.tensor_scalar(
                                out=ent0[:], in0=ent0[:], scalar1=-1.0,
                                scalar2=mx[:], op0=mybir.AluOpType.mult,
                                op1=mybir.AluOpType.add,
                            )
                            nc.vector.tensor_tensor(
                                out=ent0[:], in0=ent0[:], in1=logs[:],
                                op=mybir.AluOpType.add,
                            )
                            nc.vector.tensor_tensor(
                                out=ent0[:], in0=ent0[:], in1=maskf[:, tg : tg + 1],
                                op=mybir.AluOpType.mult,
                            )
                            nc.vector.tensor_tensor(
                                out=acc_all[:, 8:9], in0=acc_all[:, 8:9],
                                in1=ent0[:], op=mybir.AluOpType.add,
                            )
                            # value
                            vm = post.tile([128, 1], f32, tag="vm")
                            nc.vector.tensor_tensor(
                                out=vm[:], in0=ln[:, 32:33],
                                in1=maskf[:, tg : tg + 1], op=mybir.AluOpType.mult,
                            )
                            nc.vector.tensor_tensor(
                                out=acc_all[:, b : b + 1], in0=acc_all[:, b : b + 1],
                                in1=vm[:], op=mybir.AluOpType.add,
                            )
                    nc.sync.dma_start(
                        out=alp_out[b, :, :], in_=alp_sb[:, b * NT : (b + 1) * NT]
                    )

                # final partition reduce via ones matmul (reuses the tr slot)
                if True:
                    red = trps.tile([1, 10], f32, space="PSUM", tag="tr")
                    nc.tensor.matmul(
                        red[:], lhsT=ones_col[:], rhs=acc_all[:],
                        start=True, stop=True,
                    )
                    scal_sb = per.tile([1, 10], f32)
                    nc.vector.tensor_copy(out=scal_sb[:], in_=red[:])
                    nc.sync.dma_start(out=scal_out[:, :], in_=scal_sb[:])

    tile_patch.fix_waits(nc)
    return nc


def _prep_inputs(ob_x, ob_t, action, src, dst, aW0, ab0, aW1, ab1, aW2, ab2,
                 cW0, cb0, cW1, cb1, cW2, cb2):
    ob_x = np.asarray(ob_x, np.int32)
    ob_t = np.asarray(ob_t, np.float32)
    action = np.asarray(action, np.int32)
    src = np.asarray(src, np.int64)
    dst = np.asarray(dst, np.int64)

    iota32 = np.tile(np.arange(1, 33, dtype=np.int32), (128, 1))
    iota21 = np.tile(np.arange(OUT_D, dtype=np.float32), (128, 1))
    w1a = np.ascontiguousarray(
        np.asarray(aW1, np.float32).reshape(8, 128, 8, 128)
        .transpose(1, 0, 2, 3).reshape(128, 64, 128)).astype(ml_dtypes.bfloat16)
    w1c = np.ascontiguousarray(
        np.asarray(cW1, np.float32).reshape(8, 128, 8, 128)
        .transpose(1, 0, 2, 3).reshape(128, 64, 128)).astype(ml_dtypes.bfloat16)
    w2a = np.ascontiguousarray(
        np.asarray(aW2, np.float32).reshape(8, 128, OUT_D).transpose(1, 0, 2)
    ).astype(ml_dtypes.bfloat16)
    w2c = np.ascontiguousarray(
        np.asarray(cW2, np.float32).reshape(8, 128, 1).transpose(1, 0, 2)
    ).astype(ml_dtypes.bfloat16)
    b0a = np.ascontiguousarray(np.asarray(ab0, np.float32).reshape(8, 128).T)
    b0c = np.ascontiguousarray(np.asarray(cb0, np.float32).reshape(8, 128).T)
    b1a = np.ascontiguousarray(np.asarray(ab1, np.float32).reshape(8, 128).T)
    b1c = np.ascontiguousarray(np.asarray(cb1, np.float32).reshape(8, 128).T)
    b2cat = np.zeros((64, 1), np.float32)
    b2cat[0:OUT_D, 0] = np.asarray(ab2, np.float32)
    b2cat[32, 0] = np.asarray(cb2, np.float32)[0]

    shared = {
        "obx_g": ob_x, "iota32": iota32, "iota21": iota21,
        "w0a": np.asarray(aW0, np.float32), "w0c": np.asarray(cW0, np.float32),
        "w1a": w1a, "w1c": w1c, "w2a": w2a, "w2c": w2c,
        "b0a": b0a, "b0c": b0c, "b1a": b1a, "b1c": b1c, "b2cat": b2cat,
    }

    in_maps = []
    for k in range(NCORES):
        lo, hi = k * NPC, (k + 1) * NPC
        obtT = np.zeros((B + 1, NPCP), np.float32)
        obtT[:B, :NPC] = ob_t[lo:hi, :].T
        obtT[B, :] = 1.0
        obxN = np.ones((B, 128, NT), np.int32)     # pad color 1 -> mask 0
        actN = np.zeros((B, 128, NT), np.int32)
        obx_loc = ob_x[lo:hi].T                    # [B, NPC]
        act_loc = action[lo:hi].T
        padded_x = np.ones((B, NPCP), np.int32)
        padded_x[:, :NPC] = obx_loc
        padded_a = np.zeros((B, NPCP), np.int32)
        padded_a[:, :NPC] = act_loc
        obxN[:] = padded_x.reshape(B, NT, 128).transpose(0, 2, 1)
        actN[:] = padded_a.reshape(B, NT, 128).transpose(0, 2, 1)

        a_all = np.zeros((NBLK, 128, 512), np.uint8)
        for pas, (g_arr, s_arr) in enumerate(((src, dst), (dst, src))):
            selm = (s_arr >= lo) & (s_arr < hi)
            gg = g_arr[selm]
            ss = s_arr[selm] - lo
            dense = np.zeros((NP, NPCP), np.uint8)
            np.add.at(dense, (gg, ss), 1)
            blocks = dense.reshape(KT, 128, NRANGE, 512).transpose(2, 0, 1, 3)
            a_all[pas * NRANGE * KT:(pas + 1) * NRANGE * KT] = blocks.reshape(
                NRANGE * KT, 128, 512)
            del dense
        a_fp8 = _FP8_LUT[a_all]
        del a_all

        m = dict(shared)
        m.update({"obtT": obtT, "obxN": obxN, "actN": actN, "a_all": a_fp8})
        in_maps.append(m)
    return in_maps


def kernel(**inputs):
    global LAST_EXEC_NS
    if "nc" not in _CACHED:
        _CACHED["nc"] = _build_program()
    nc = _CACHED["nc"]
    in_maps = _prep_inputs(**inputs)
    trace = bool(os.environ.get("KBENCH_TRACE"))
    res = run_bass_kernel_spmd(
        nc, in_maps, core_ids=list(range(NCORES)), trace=trace)
    LAST_EXEC_NS = res.exec_time_ns

    alp = np.zeros((N, B), np.float32)
    vsum = np.zeros(B, np.float64)
    esum = 0.0
    msum = 0.0
    for k in range(NCORES):
        out = res.results[k]
        a = out["alp_out"]                     # [B, 128, NT]
        loc = a.transpose(2, 1, 0).reshape(NPCP, B)[:NPC]
        alp[k * NPC:(k + 1) * NPC] = loc
        sc = out["scal_out"][0]
        vsum += sc[0:8].astype(np.float64)
        esum += float(sc[8])
        msum += float(sc[9])
    avg_entropy = np.float32(esum / max(msum, 1.0))
    value_preds = (vsum / MAXN).astype(np.float32)
    return alp, avg_entropy, value_preds


# revision 20
# speedup vs baseline: 1.0295x; 1.0295x over previous
"""Trainium2 Bass kernel for nn_ActorCritic (GNN message passing + actor/critic MLPs).

Sharding: nodes are partitioned across the 8 NeuronCores (2500 nodes each, all
8 batch elements on every core). The one-hop segment_sums (nb, nb_rev) are
computed as dense fp8 matmuls against host-built adjacency blocks:
    nbT[pay, dst] = sum_src X[src, pay] * A[src, dst]
with X = one-hot colors built on device (payload = batch*32 + color, 256 wide,
split into hi/lo 128-partition planes) and A the 0/1 adjacency (pure index
data). The MLPs run with float32r (layer 0) and bfloat16 (layers 1/2) matmuls;
log-softmax / entropy / value reductions run on DVE/ACT after a PE transpose
back to node-major layout. Outputs are gathered and re-assembled on host.
"""
import os
import sys

sys.path.insert(0, "/opt/trn_rl_repo")
sys.path.insert(0, os.path.dirname(os.path.abspath(__file__)))

import numpy as np
import ml_dtypes

import concourse.bass as bass
import concourse.mybir as mybir
import concourse.tile as tile_mod
from concourse.tile import TileContext
from concourse.bass_utils import run_bass_kernel_spmd
from concourse.masks import make_identity
from concourse.vector_clock import ScopedClock


# ---------------------------------------------------------------------------
# Walrus-compat patches: this build rejects >1 sem wait per instruction, and
# the stock TileContext tail drain carries one wait per live logical proc.
# ---------------------------------------------------------------------------

MAX_WAITS = 1


def _patched_drain_and_barrier(self, tick_clock, wait_clock):
    nc = self.nc
    probe = nc.sync.nop()
    wait_clock.add_sem_waits(probe.ins, ScopedClock({None: tick_clock.global_clock}))
    si = probe.ins.sync_info
    waits = list(si.on_wait) if si is not None else []
    if len(waits) > MAX_WAITS:
        si.on_wait = waits[:MAX_WAITS]
        rest = waits[MAX_WAITS:]
        for j in range(0, len(rest), MAX_WAITS):
            n = nc.sync.nop()
            nsi = n.ins.sync_info
            if nsi is None:
                n.ins.sync_info = mybir.SyncInfo(
                    on_update=[], on_wait=rest[j : j + MAX_WAITS]
                )
            else:
                nsi.on_wait = rest[j : j + MAX_WAITS]
    nc.sync.drain()
    nc.all_engine_barrier()
    assert self.sems is not None
    popped = nc._tile_sem_poison_stack.pop()
    assert popped is self._sem_poison
    nc.clear_and_free_semaphores(list(self.sems.allocated().values()))
    nc.all_engine_barrier()


_ws_counter = [0]


def fix_waits(nc, max_waits: int = 1):
    """Post-pass over the finished module: any instruction carrying more than
    ``max_waits`` sem waits gets the excess hoisted onto same-engine NoOps
    inserted immediately before it (this walrus build rejects multi-wait
    instructions at codegen)."""
    for f in nc.m.functions:
        for blk in f.blocks:
            insns = blk.instructions
            out = []
            changed = False
            for ins in insns:
                si = ins.sync_info
                if si is not None and len(si.on_wait) > max_waits:
                    waits = list(si.on_wait)
                    keep = waits[: max_waits]
                    rest = waits[max_waits:]
                    for j in range(0, len(rest), max_waits):
                        _ws_counter[0] += 1
                        nop = mybir.InstNoOp(
                            name=f"WSPLIT-{_ws_counter[0]}",
                            ins=[],
                            outs=[],
                            engine=ins.engine,
                            sync_info=mybir.SyncInfo(
                                on_update=[], on_wait=rest[j : j + max_waits]
                            ),
                        )
                        out.append(nop)
                    si.on_wait = keep
                    changed = True
                out.append(ins)
            if changed:
                blk.instructions = out


def install():
    tile_mod.TileContext._drain_and_barrier = _patched_drain_and_barrier


class _TP:
    fix_waits = staticmethod(fix_waits)


tile_patch = _TP()
install()

# ---- problem constants (hardcoded per spec) ----
N, B, NCOL, H, E = 20000, 8, 20, 1024, 320000
IN_D, OUT_D, MAXN = 42, 21, 20000
NCORES = 8
NPC = N // NCORES            # 2500 nodes per core
NPCP = 2560                  # padded: 20 x 128 = 5 x 512
NT = NPCP // 128             # 20 node tiles per core
NRANGE = NPCP // 512         # 5 psum ranges
KT = 157                     # src k-chunks (ceil(20000/128))
NP = KT * 128                # 20096 padded source nodes
NBLK = 2 * NRANGE * KT       # 1570 adjacency blocks per core

f32 = mybir.dt.float32
f32r = mybir.dt.float32r
bf16 = mybir.dt.bfloat16
fp8 = mybir.dt.float8e4
i32 = mybir.dt.int32

_FP8_LUT = np.arange(256, dtype=np.uint8).astype(np.float32).astype(
    ml_dtypes.float8_e4m3)

LAST_EXEC_NS = None
_CACHED = {}


def _build_program():
    nc = bass.Bass("TRN2")
    p = {}
    p["obx_g"] = nc.declare_dram_parameter("obx_g", [N, B], i32, isOutput=False)
    p["obtT"] = nc.declare_dram_parameter("obtT", [B + 1, NPCP], f32, isOutput=False)
    p["obxN"] = nc.declare_dram_parameter("obxN", [B, 128, NT], i32, isOutput=False)
    p["actN"] = nc.declare_dram_parameter("actN", [B, 128, NT], i32, isOutput=False)
    p["a_all"] = nc.declare_dram_parameter("a_all", [NBLK, 128, 512], fp8, isOutput=False)
    p["iota32"] = nc.declare_dram_parameter("iota32", [128, 32], i32, isOutput=False)
    p["iota21"] = nc.declare_dram_parameter("iota21", [128, 21], f32, isOutput=False)
    p["w0a"] = nc.declare_dram_parameter("w0a", [IN_D, H], f32, isOutput=False)
    p["w0c"] = nc.declare_dram_parameter("w0c", [IN_D, H], f32, isOutput=False)
    p["w1a"] = nc.declare_dram_parameter("w1a", [128, 64, 128], bf16, isOutput=False)
    p["w1c"] = nc.declare_dram_parameter("w1c", [128, 64, 128], bf16, isOutput=False)
    p["w2a"] = nc.declare_dram_parameter("w2a", [128, 8, OUT_D], bf16, isOutput=False)
    p["w2c"] = nc.declare_dram_parameter("w2c", [128, 8, 1], bf16, isOutput=False)
    p["b0a"] = nc.declare_dram_parameter("b0a", [128, 8], f32, isOutput=False)
    p["b0c"] = nc.declare_dram_parameter("b0c", [128, 8], f32, isOutput=False)
    p["b1a"] = nc.declare_dram_parameter("b1a", [128, 8], f32, isOutput=False)
    p["b1c"] = nc.declare_dram_parameter("b1c", [128, 8], f32, isOutput=False)
    p["b2cat"] = nc.declare_dram_parameter("b2cat", [64, 1], f32, isOutput=False)
    alp_out = nc.declare_dram_parameter("alp_out", [B, 128, NT], f32, isOutput=True)
    scal_out = nc.declare_dram_parameter("scal_out", [1, 10], f32, isOutput=True)

    with TileContext(nc) as tc:
        with (
            tc.tile_pool(name="per", bufs=1) as per,       # persistent
            tc.tile_pool(name="nbp", bufs=1) as nbp,       # nb storage
        ):
            # ---- persistent small tiles ----
            iota32_t = per.tile([128, 32], i32)
            nc.sync.dma_start(out=iota32_t[:], in_=p["iota32"][:, :])
            iota21_t = per.tile([128, 21], f32)
            nc.sync.dma_start(out=iota21_t[:], in_=p["iota21"][:, :])
            ident_f32 = per.tile([128, 128], f32)
            make_identity(nc, ident_f32[:])
            ones_col = per.tile([128, 1], f32)
            nc.vector.memset(ones_col[:], 1.0)
            acc_all = per.tile([128, 10], f32)
            nc.vector.memset(acc_all[:], 0.0)
            b2cat_t = per.tile([64, 1], f32)
            nc.sync.dma_start(out=b2cat_t[:], in_=p["b2cat"][:, :])

            # weights
            w0a_t = per.tile([IN_D, H], f32r)
            nc.gpsimd.dma_start(out=w0a_t[:], in_=p["w0a"][:, :])
            w0c_t = per.tile([IN_D, H], f32r)
            nc.gpsimd.dma_start(out=w0c_t[:], in_=p["w0c"][:, :])
            w1a_t = per.tile([128, 64, 128], bf16)
            nc.sync.dma_start(out=w1a_t[:], in_=p["w1a"][:, :, :])
            w1c_t = per.tile([128, 64, 128], bf16)
            nc.sync.dma_start(out=w1c_t[:], in_=p["w1c"][:, :, :])
            w2a_t = per.tile([128, 8, OUT_D], bf16)
            nc.sync.dma_start(out=w2a_t[:], in_=p["w2a"][:, :, :])
            w2c_t = per.tile([128, 8, 1], bf16)
            nc.sync.dma_start(out=w2c_t[:], in_=p["w2c"][:, :, :])
            biases = {}
            for nm in ("b0a", "b0c", "b1a", "b1c"):
                t = per.tile([128, 8], f32, name=nm)
                nc.sync.dma_start(out=t[:], in_=p[nm][:, :])
                biases[nm] = t

            # nb storage: [payload(4b x 32), node] f32
            nb_hi = nbp.tile([128, NPCP], f32)
            nb_lo = nbp.tile([128, NPCP], f32)
            nbr_hi = nbp.tile([128, NPCP], f32)
            nbr_lo = nbp.tile([128, NPCP], f32)
            nbt = {(0, 0): nb_hi, (0, 1): nb_lo, (1, 0): nbr_hi, (1, 1): nbr_lo}

            # ---- phase 0: build one-hot X in SBUF ----
            with tc.tile_pool(name="xp", bufs=1) as xp:
                x_sb = xp.tile([128, KT, 256], fp8)
                nc.vector.memset(x_sb[:, :, :], 0.0)
                obx_sb = xp.tile([128, KT, 8], i32)
                nc.sync.dma_start(
                    out=obx_sb[:, : KT - 1, :],
                    in_=p["obx_g"][0 : (KT - 1) * 128, :].rearrange(
                        "(t q) b -> q t b", q=128
                    ),
                )
                # last partial tile: rows 19968..19999 (32 rows)
                nc.sync.dma_start(
                    out=obx_sb[:32, KT - 1, :],
                    in_=p["obx_g"][(KT - 1) * 128 :, :],
                )
                for t in range(KT):
                    hi = 128 if t < KT - 1 else 32
                    nc.vector.tensor_tensor(
                        out=x_sb[:hi, t, :],
                        in0=obx_sb[:hi, t, :].rearrange(
                            "p (b one) -> p b one", one=1
                        ).to_broadcast([hi, 8, 32]),
                        in1=iota32_t[:hi, :].rearrange(
                            "p (one j) -> p one j", one=1
                        ).to_broadcast([hi, 8, 32]),
                        op=mybir.AluOpType.is_equal,
                    )

                # ---- phase 1: dense scatter matmuls ----
                with (
                    tc.tile_pool(name="ab", bufs=4) as ab,
                    tc.tile_pool(name="scps", bufs=3, space="PSUM") as scps,
                ):
                    for pas in range(2):
                        for r in range(NRANGE):
                            ps_hi = scps.tile([128, 512], f32, space="PSUM", tag="hi")
                            ps_lo = scps.tile([128, 512], f32, space="PSUM", tag="lo")
                            for k0 in range(0, KT, 16):
                                kk = min(16, KT - k0)
                                blk = (pas * NRANGE + r) * KT + k0
                                a_t = ab.tile([128, 16, 512], fp8, tag="a")
                                nc.sync.dma_start(
                                    out=a_t[:, :kk, :],
                                    in_=p["a_all"][blk : blk + kk, :, :].rearrange(
                                        "k q n -> q k n"),
                                )
                                for j0 in range(0, kk, 2):
                                    k0j = k0 + j0
                                    jj = min(2, kk - j0)
                                    if jj == 2:
                                        nc.tensor.matmul(
                                            ps_hi[:],
                                            lhsT=x_sb[:, k0j : k0j + 2, 0:128],
                                            rhs=a_t[:, j0 : j0 + 2, :],
                                            start=(k0j == 0),
                                            stop=(k0j + 2 == KT),
                                            perf_mode=mybir.MatmulPerfMode.DoubleRow,
                                        )
                                        nc.tensor.matmul(
                                            ps_lo[:],
                                            lhsT=x_sb[:, k0j : k0j + 2, 128:256],
                                            rhs=a_t[:, j0 : j0 + 2, :],
                                            start=(k0j == 0),
                                            stop=(k0j + 2 == KT),
                                            perf_mode=mybir.MatmulPerfMode.DoubleRow,
                                        )
                                    else:
                                        nc.tensor.matmul(
                                            ps_hi[:],
                                            lhsT=x_sb[:, k0j, 0:128],
                                            rhs=a_t[:, j0, :],
                                            start=(k0j == 0),
                                            stop=(k0j == KT - 1),
                                        )
                                        nc.tensor.matmul(
                                            ps_lo[:],
                                            lhsT=x_sb[:, k0j, 128:256],
                                            rhs=a_t[:, j0, :],
                                            start=(k0j == 0),
                                            stop=(k0j == KT - 1),
                                        )
                            nc.vector.tensor_copy(
                                out=nbt[(pas, 0)][:, r * 512 : (r + 1) * 512],
                                in_=ps_hi[:],
                            )
                            nc.vector.tensor_copy(
                                out=nbt[(pas, 1)][:, r * 512 : (r + 1) * 512],
                                in_=ps_lo[:],
                            )

            # ---- phase 2: MLPs + post ----
            with (
                tc.tile_pool(name="mlp", bufs=1) as mlp,
                tc.tile_pool(name="post", bufs=2) as post,
                tc.tile_pool(name="l0ps", bufs=2, space="PSUM") as l0ps,
                tc.tile_pool(name="l1ps", bufs=2, space="PSUM") as l1ps,
                tc.tile_pool(name="l2ps", bufs=1, space="PSUM") as l2ps,
                tc.tile_pool(name="trps", bufs=2, space="PSUM") as trps,
            ):
                alp_sb = mlp.tile([128, B * NT], f32)
                for b in range(B):
                    bq = (b % 4) * 32
                    plane = b // 4
                    hT = mlp.tile([IN_D, NPCP], f32r, tag="hT", bufs=2)
                    nc.gpsimd.dma_start(out=hT[0:1, :], in_=p["obtT"][b : b + 1, :])
                    nc.gpsimd.dma_start(
                        out=hT[1:21, :], in_=nbt[(0, plane)][bq : bq + 20, :]
                    )
                    nc.gpsimd.dma_start(
                        out=hT[21:41, :], in_=nbt[(1, plane)][bq : bq + 20, :]
                    )
                    nc.gpsimd.dma_start(out=hT[41:42, :], in_=p["obtT"][B : B + 1, :])

                    maskf = post.tile([128, NT], f32, tag="maskf")
                    obxn = post.tile([128, NT], i32, tag="obxn")
                    nc.sync.dma_start(out=obxn[:], in_=p["obxN"][b, :, :])
                    nc.vector.tensor_scalar(
                        out=maskf[:], in0=obxn[:], scalar1=0, scalar2=None,
                        op0=mybir.AluOpType.is_equal,
                    )
                    actf = post.tile([128, NT], f32, tag="actf")
                    actn = post.tile([128, NT], i32, tag="actn")
                    nc.sync.dma_start(out=actn[:], in_=p["actN"][b, :, :])
                    nc.vector.tensor_copy(out=actf[:], in_=actn[:])
                    msum = post.tile([128, 1], f32, tag="msum")
                    nc.vector.tensor_reduce(
                        out=msum[:], in_=maskf[:], axis=mybir.AxisListType.X,
                        op=mybir.AluOpType.add,
                    )
                    nc.vector.tensor_tensor(
                        out=acc_all[:, 9:10], in0=acc_all[:, 9:10], in1=msum[:],
                        op=mybir.AluOpType.add,
                    )

                    for rt in range(NRANGE):
                        cs = rt * 512
                        h0a = mlp.tile([128, 8, 512], bf16, tag="h0a", bufs=2)
                        h0c = mlp.tile([128, 8, 512], bf16, tag="h0c", bufs=2)
                        for net, w0t, h0t, b0 in (
                            ("a", w0a_t, h0a, biases["b0a"]),
                            ("c", w0c_t, h0c, biases["b0c"]),
                        ):
                            for m in range(8):
                                ps = l0ps.tile([128, 512], f32, space="PSUM", tag="l0")
                                nc.tensor.matmul(
                                    ps[:],
                                    lhsT=w0t[:, m * 128 : (m + 1) * 128],
                                    rhs=hT[:, cs : cs + 512],
                                    start=True, stop=True,
                                )
                                nc.scalar.activation(
                                    out=h0t[:, m, :], in_=ps[:],
                                    func=mybir.ActivationFunctionType.Relu,
                                    bias=b0[:, m : m + 1],
                                )
                        h1a = mlp.tile([128, 8, 512], bf16, tag="h1a", bufs=2)
                        h1c = mlp.tile([128, 8, 512], bf16, tag="h1c", bufs=2)
                        for m in range(8):
                            ps = l1ps.tile([128, 512], f32, space="PSUM", tag="l1")
                            for k in range(8):
                                nc.tensor.matmul(
                                    ps[:],
                                    lhsT=w1a_t[:, k * 8 + m, :],
                                    rhs=h0a[:, k, :],
                                    start=(k == 0), stop=(k == 7),
                                )
                            nc.scalar.activation(
                                out=h1a[:, m, :], in_=ps[:],
                                func=mybir.ActivationFunctionType.Relu,
                                bias=biases["b1a"][:, m : m + 1],
                            )
                        for m in range(8):
                            ps = l1ps.tile([128, 512], f32, space="PSUM", tag="l1")
                            for k in range(8):
                                nc.tensor.matmul(
                                    ps[:],
                                    lhsT=w1c_t[:, k * 8 + m, :],
                                    rhs=h0c[:, k, :],
                                    start=(k == 0), stop=(k == 7),
                                )
                            nc.scalar.activation(
                                out=h1c[:, m, :], in_=ps[:],
                                func=mybir.ActivationFunctionType.Relu,
                                bias=biases["b1c"][:, m : m + 1],
                            )
                        ps2 = l2ps.tile([64, 512], f32, space="PSUM", tag="l2")
                        for k in range(8):
                            nc.tensor.matmul(
                                ps2[0:OUT_D, :], lhsT=w2a_t[:, k, :], rhs=h1a[:, k, :],
                                start=(k == 0), stop=(k == 7),
                            )
                        for k in range(8):
                            nc.tensor.matmul(
                                ps2[32:33, :], lhsT=w2c_t[:, k, :], rhs=h1c[:, k, :],
                                start=(k == 0), stop=(k == 7),
                            )
                        catT = post.tile([64, 512], f32, tag="catT")
                        nc.vector.tensor_scalar(
                            out=catT[0:33, :], in0=ps2[0:33, :],
                            scalar1=b2cat_t[0:33, :], scalar2=None,
                            op0=mybir.AluOpType.add,
                        )
                        for ntile in range(4):
                            tg = rt * 4 + ntile
                            trp = trps.tile([128, 64], f32, space="PSUM", tag="tr")
                            nc.tensor.transpose(
                                out=trp[:],
                                in_=catT[:, ntile * 128 : (ntile + 1) * 128],
                                identity=ident_f32[0:64, 0:64],
                            )
                            ln = post.tile([128, 33], f32, tag="ln")
                            nc.vector.tensor_copy(out=ln[:], in_=trp[:, 0:33])
                            lg = ln[:, 0:OUT_D]
                            mx = post.tile([128, 1], f32, tag="mx")
                            nc.vector.tensor_reduce(
                                out=mx[:], in_=lg, axis=mybir.AxisListType.X,
                                op=mybir.AluOpType.max,
                            )
                            nmx = post.tile([128, 1], f32, tag="nmx")
                            nc.vector.tensor_scalar_mul(nmx[:], mx[:], -1.0)
                            ex = post.tile([128, OUT_D], f32, tag="ex")
                            s = post.tile([128, 1], f32, tag="s")
                            nc.scalar.activation(
                                out=ex[:], in_=lg,
                                func=mybir.ActivationFunctionType.Exp,
                                bias=nmx[:], accum_out=s[:],
                            )
                            logs = post.tile([128, 1], f32, tag="logs")
                            nc.scalar.activation(
                                out=logs[:], in_=s[:],
                                func=mybir.ActivationFunctionType.Ln,
                            )
                            sel = post.tile([128, OUT_D], f32, tag="sel")
                            nc.vector.tensor_scalar(
                                out=sel[:], in0=iota21_t[:],
                                scalar1=actf[:, tg : tg + 1], scalar2=None,
                                op0=mybir.AluOpType.is_equal,
                            )
                            junk = post.tile([128, OUT_D], f32, tag="junk")
                            asel = post.tile([128, 1], f32, tag="asel")
                            nc.vector.tensor_tensor(
                                out=junk[:], in0=sel[:], in1=lg,
                                op=mybir.AluOpType.mult,
                            )
                            nc.vector.tensor_reduce(
                                out=asel[:], in_=junk[:],
                                axis=mybir.AxisListType.X, op=mybir.AluOpType.add,
                            )
                            junk2 = post.tile([128, OUT_D], f32, tag="junk2")
                            t3 = post.tile([128, 1], f32, tag="t3")
                            nc.vector.tensor_tensor(
                                out=junk2[:], in0=ex[:], in1=lg,
                                op=mybir.AluOpType.mult,
                            )
                            nc.vector.tensor_reduce(
                                out=t3[:], in_=junk2[:],
                                axis=mybir.AxisListType.X, op=mybir.AluOpType.add,
                            )
                            # alp = (asel - mx - logs) * mask
                            alp0 = post.tile([128, 1], f32, tag="alp0")
                            nc.vector.tensor_scalar(
                                out=alp0[:], in0=asel[:], scalar1=mx[:],
                                scalar2=None, op0=mybir.AluOpType.subtract,
                            )
                            nc.vector.tensor_tensor(
                                out=alp0[:], in0=alp0[:], in1=logs[:],
                                op=mybir.AluOpType.subtract,
                            )
                            nc.vector.tensor_tensor(
                                out=alp_sb[:, b * NT + tg : b * NT + tg + 1],
                                in0=alp0[:], in1=maskf[:, tg : tg + 1],
                                op=mybir.AluOpType.mult,
                            )
                            # ent = mx + logs - t3 / s
                            rs = post.tile([128, 1], f32, tag="rs")
                            nc.vector.reciprocal(rs[:], s[:])
                            ent0 = post.tile([128, 1], f32, tag="ent0")
                            nc.vector.tensor_tensor(
                                out=ent0[:], in0=t3[:], in1=rs[:],
                                op=mybir.AluOpType.mult,
                            )
                            nc.vector.tensor_scalar(
                                out=ent0[:], in0=ent0[:], scalar1=-1.0,
                                scalar2=mx[:], op0=mybir.AluOpType.mult,
                                op1=mybir.AluOpType.add,
                            )
                            nc.vector.tensor_tensor(
                                out=ent0[:], in0=ent0[:], in1=logs[:],
                                op=mybir.AluOpType.add,
                            )
                            nc.vector.tensor_tensor(
                                out=ent0[:], in0=ent0[:], in1=maskf[:, tg : tg + 1],
                                op=mybir.AluOpType.mult,
                            )
                            nc.vector.tensor_tensor(
                                out=acc_all[:, 8:9], in0=acc_all[:, 8:9],
                                in1=ent0[:], op=mybir.AluOpType.add,
                            )
                            # value
                            vm = post.tile([128, 1], f32, tag="vm")
                            nc.vector.tensor_tensor(
                                out=vm[:], in0=ln[:, 32:33],
                                in1=maskf[:, tg : tg + 1], op=mybir.AluOpType.mult,
                            )
                            nc.vector.tensor_tensor(
                                out=acc_all[:, b : b + 1], in0=acc_all[:, b : b + 1],
                                in1=vm[:], op=mybir.AluOpType.add,
                            )
                    nc.sync.dma_start(
                        out=alp_out[b, :, :], in_=alp_sb[:, b * NT : (b + 1) * NT]
                    )

                # final partition reduce via ones matmul
                with tc.tile_pool(name="rdps", bufs=1, space="PSUM") as rdps:
                    red = rdps.tile([1, 10], f32, space="PSUM")
                    nc.tensor.matmul(
                        red[:], lhsT=ones_col[:], rhs=acc_all[:],
                        start=True, stop=True,
                    )
                    scal_sb = per.tile([1, 10], f32)
                    nc.vector.tensor_copy(out=scal_sb[:], in_=red[:])
                    nc.sync.dma_start(out=scal_out[:, :], in_=scal_sb[:])

    tile_patch.fix_waits(nc)
    return nc


def _prep_inputs(ob_x, ob_t, action, src, dst, aW0, ab0, aW1, ab1, aW2, ab2,
                 cW0, cb0, cW1, cb1, cW2, cb2):
    ob_x = np.asarray(ob_x, np.int32)
    ob_t = np.asarray(ob_t, np.float32)
    action = np.asarray(action, np.int32)
    src = np.asarray(src, np.int64)
    dst = np.asarray(dst, np.int64)

    iota32 = np.tile(np.arange(1, 33, dtype=np.int32), (128, 1))
    iota21 = np.tile(np.arange(OUT_D, dtype=np.float32), (128, 1))
    w1a = np.ascontiguousarray(
        np.asarray(aW1, np.float32).reshape(8, 128, 8, 128)
        .transpose(1, 0, 2, 3).reshape(128, 64, 128)).astype(ml_dtypes.bfloat16)
    w1c = np.ascontiguousarray(
        np.asarray(cW1, np.float32).reshape(8, 128, 8, 128)
        .transpose(1, 0, 2, 3).reshape(128, 64, 128)).astype(ml_dtypes.bfloat16)
    w2a = np.ascontiguousarray(
        np.asarray(aW2, np.float32).reshape(8, 128, OUT_D).transpose(1, 0, 2)
    ).astype(ml_dtypes.bfloat16)
    w2c = np.ascontiguousarray(
        np.asarray(cW2, np.float32).reshape(8, 128, 1).transpose(1, 0, 2)
    ).astype(ml_dtypes.bfloat16)
    b0a = np.ascontiguousarray(np.asarray(ab0, np.float32).reshape(8, 128).T)
    b0c = np.ascontiguousarray(np.asarray(cb0, np.float32).reshape(8, 128).T)
    b1a = np.ascontiguousarray(np.asarray(ab1, np.float32).reshape(8, 128).T)
    b1c = np.ascontiguousarray(np.asarray(cb1, np.float32).reshape(8, 128).T)
    b2cat = np.zeros((64, 1), np.float32)
    b2cat[0:OUT_D, 0] = np.asarray(ab2, np.float32)
    b2cat[32, 0] = np.asarray(cb2, np.float32)[0]

    shared = {
        "obx_g": ob_x, "iota32": iota32, "iota21": iota21,
        "w0a": np.asarray(aW0, np.float32), "w0c": np.asarray(cW0, np.float32),
        "w1a": w1a, "w1c": w1c, "w2a": w2a, "w2c": w2c,
        "b0a": b0a, "b0c": b0c, "b1a": b1a, "b1c": b1c, "b2cat": b2cat,
    }

    in_maps = []
    for k in range(NCORES):
        lo, hi = k * NPC, (k + 1) * NPC
        obtT = np.zeros((B + 1, NPCP), np.float32)
        obtT[:B, :NPC] = ob_t[lo:hi, :].T
        obtT[B, :] = 1.0
        obxN = np.ones((B, 128, NT), np.int32)     # pad color 1 -> mask 0
        actN = np.zeros((B, 128, NT), np.int32)
        obx_loc = ob_x[lo:hi].T                    # [B, NPC]
        act_loc = action[lo:hi].T
        padded_x = np.ones((B, NPCP), np.int32)
        padded_x[:, :NPC] = obx_loc
        padded_a = np.zeros((B, NPCP), np.int32)
        padded_a[:, :NPC] = act_loc
        obxN[:] = padded_x.reshape(B, NT, 128).transpose(0, 2, 1)
        actN[:] = padded_a.reshape(B, NT, 128).transpose(0, 2, 1)

        a_all = np.zeros((NBLK, 128, 512), np.uint8)
        for pas, (g_arr, s_arr) in enumerate(((src, dst), (dst, src))):
            selm = (s_arr >= lo) & (s_arr < hi)
            gg = g_arr[selm]
            ss = s_arr[selm] - lo
            dense = np.zeros((NP, NPCP), np.uint8)
            np.add.at(dense, (gg, ss), 1)
            blocks = dense.reshape(KT, 128, NRANGE, 512).transpose(2, 0, 1, 3)
            a_all[pas * NRANGE * KT:(pas + 1) * NRANGE * KT] = blocks.reshape(
                NRANGE * KT, 128, 512)
            del dense
        a_fp8 = _FP8_LUT[a_all]
        del a_all

        m = dict(shared)
        m.update({"obtT": obtT, "obxN": obxN, "actN": actN, "a_all": a_fp8})
        in_maps.append(m)
    return in_maps


def kernel(**inputs):
    global LAST_EXEC_NS
    if "nc" not in _CACHED:
        _CACHED["nc"] = _build_program()
    nc = _CACHED["nc"]
    in_maps = _prep_inputs(**inputs)
    trace = bool(os.environ.get("KBENCH_TRACE"))
    res = run_bass_kernel_spmd(
        nc, in_maps, core_ids=list(range(NCORES)), trace=trace)
    LAST_EXEC_NS = res.exec_time_ns

    alp = np.zeros((N, B), np.float32)
    vsum = np.zeros(B, np.float64)
    esum = 0.0
    msum = 0.0
    for k in range(NCORES):
        out = res.results[k]
        a = out["alp_out"]                     # [B, 128, NT]
        loc = a.transpose(2, 1, 0).reshape(NPCP, B)[:NPC]
        alp[k * NPC:(k + 1) * NPC] = loc
        sc = out["scal_out"][0]
        vsum += sc[0:8].astype(np.float64)
        esum += float(sc[8])
        msum += float(sc[9])
    avg_entropy = np.float32(esum / max(msum, 1.0))
    value_preds = (vsum / MAXN).astype(np.float32)
    return alp, avg_entropy, value_preds


# revision 21
# speedup vs baseline: 1.0489x; 1.0188x over previous
"""Trainium2 Bass kernel for nn_ActorCritic (GNN message passing + actor/critic MLPs).

Sharding: nodes are partitioned across the 8 NeuronCores (2500 nodes each, all
8 batch elements on every core). The one-hop segment_sums (nb, nb_rev) are
computed as dense fp8 matmuls against host-built adjacency blocks:
    nbT[pay, dst] = sum_src X[src, pay] * A[src, dst]
with X = one-hot colors built on device (payload = batch*32 + color, 256 wide,
split into hi/lo 128-partition planes) and A the 0/1 adjacency (pure index
data). The MLPs run with float32r (layer 0) and bfloat16 (layers 1/2) matmuls;
log-softmax / entropy / value reductions run on DVE/ACT after a PE transpose
back to node-major layout. Outputs are gathered and re-assembled on host.
"""
import os
import sys

sys.path.insert(0, "/opt/trn_rl_repo")
sys.path.insert(0, os.path.dirname(os.path.abspath(__file__)))

import numpy as np
import ml_dtypes

import concourse.bass as bass
import concourse.mybir as mybir
import concourse.tile as tile_mod
from concourse.tile import TileContext
from concourse.bass_utils import run_bass_kernel_spmd
from concourse.masks import make_identity
from concourse.vector_clock import ScopedClock


# ---------------------------------------------------------------------------
# Walrus-compat patches: this build rejects >1 sem wait per instruction, and
# the stock TileContext tail drain carries one wait per live logical proc.
# ---------------------------------------------------------------------------

MAX_WAITS = 1


def _patched_drain_and_barrier(self, tick_clock, wait_clock):
    nc = self.nc
    probe = nc.sync.nop()
    wait_clock.add_sem_waits(probe.ins, ScopedClock({None: tick_clock.global_clock}))
    si = probe.ins.sync_info
    waits = list(si.on_wait) if si is not None else []
    if len(waits) > MAX_WAITS:
        si.on_wait = waits[:MAX_WAITS]
        rest = waits[MAX_WAITS:]
        for j in range(0, len(rest), MAX_WAITS):
            n = nc.sync.nop()
            nsi = n.ins.sync_info
            if nsi is None:
                n.ins.sync_info = mybir.SyncInfo(
                    on_update=[], on_wait=rest[j : j + MAX_WAITS]
                )
            else:
                nsi.on_wait = rest[j : j + MAX_WAITS]
    nc.sync.drain()
    nc.all_engine_barrier()
    assert self.sems is not None
    popped = nc._tile_sem_poison_stack.pop()
    assert popped is self._sem_poison
    nc.clear_and_free_semaphores(list(self.sems.allocated().values()))
    nc.all_engine_barrier()


_ws_counter = [0]


def fix_waits(nc, max_waits: int = 1):
    """Post-pass over the finished module: any instruction carrying more than
    ``max_waits`` sem waits gets the excess hoisted onto same-engine NoOps
    inserted immediately before it (this walrus build rejects multi-wait
    instructions at codegen)."""
    for f in nc.m.functions:
        for blk in f.blocks:
            insns = blk.instructions
            out = []
            changed = False
            for ins in insns:
                si = ins.sync_info
                if si is not None and len(si.on_wait) > max_waits:
                    waits = list(si.on_wait)
                    keep = waits[: max_waits]
                    rest = waits[max_waits:]
                    for j in range(0, len(rest), max_waits):
                        _ws_counter[0] += 1
                        nop = mybir.InstNoOp(
                            name=f"WSPLIT-{_ws_counter[0]}",
                            ins=[],
                            outs=[],
                            engine=ins.engine,
                            sync_info=mybir.SyncInfo(
                                on_update=[], on_wait=rest[j : j + max_waits]
                            ),
                        )
                        out.append(nop)
                    si.on_wait = keep
                    changed = True
                out.append(ins)
            if changed:
                blk.instructions = out


def install():
    tile_mod.TileContext._drain_and_barrier = _patched_drain_and_barrier


class _TP:
    fix_waits = staticmethod(fix_waits)


tile_patch = _TP()
install()

# ---- problem constants (hardcoded per spec) ----
N, B, NCOL, H, E = 20000, 8, 20, 1024, 320000
IN_D, OUT_D, MAXN = 42, 21, 20000
NCORES = 8
NPC = N // NCORES            # 2500 nodes per core
NPCP = 2560                  # padded: 20 x 128 = 5 x 512
NT = NPCP // 128             # 20 node tiles per core
NRANGE = NPCP // 512         # 5 psum ranges
KT = 157                     # src k-chunks (ceil(20000/128))
NP = KT * 128                # 20096 padded source nodes
NBLK = 2 * NRANGE * KT       # 1570 adjacency blocks per core

f32 = mybir.dt.float32
f32r = mybir.dt.float32r
bf16 = mybir.dt.bfloat16
fp8 = mybir.dt.float8e4
i32 = mybir.dt.int32

_FP8_LUT = np.arange(256, dtype=np.uint8).astype(np.float32).astype(
    ml_dtypes.float8_e4m3)

LAST_EXEC_NS = None
_CACHED = {}


def _build_program():
    nc = bass.Bass("TRN2")
    p = {}
    p["obx_g"] = nc.declare_dram_parameter("obx_g", [N, B], i32, isOutput=False)
    p["obtT"] = nc.declare_dram_parameter("obtT", [B + 1, NPCP], f32, isOutput=False)
    p["obxN"] = nc.declare_dram_parameter("obxN", [B, 128, NT], i32, isOutput=False)
    p["actN"] = nc.declare_dram_parameter("actN", [B, 128, NT], i32, isOutput=False)
    p["a_all"] = nc.declare_dram_parameter("a_all", [NBLK, 128, 512], fp8, isOutput=False)
    p["iota32"] = nc.declare_dram_parameter("iota32", [128, 32], i32, isOutput=False)
    p["iota21"] = nc.declare_dram_parameter("iota21", [128, 21], f32, isOutput=False)
    p["w0a"] = nc.declare_dram_parameter("w0a", [IN_D, H], f32, isOutput=False)
    p["w0c"] = nc.declare_dram_parameter("w0c", [IN_D, H], f32, isOutput=False)
    p["w1a"] = nc.declare_dram_parameter("w1a", [128, 64, 128], bf16, isOutput=False)
    p["w1c"] = nc.declare_dram_parameter("w1c", [128, 64, 128], bf16, isOutput=False)
    p["w2a"] = nc.declare_dram_parameter("w2a", [128, 8, OUT_D], bf16, isOutput=False)
    p["w2c"] = nc.declare_dram_parameter("w2c", [128, 8, 1], bf16, isOutput=False)
    p["b0a"] = nc.declare_dram_parameter("b0a", [128, 8], f32, isOutput=False)
    p["b0c"] = nc.declare_dram_parameter("b0c", [128, 8], f32, isOutput=False)
    p["b1a"] = nc.declare_dram_parameter("b1a", [128, 8], f32, isOutput=False)
    p["b1c"] = nc.declare_dram_parameter("b1c", [128, 8], f32, isOutput=False)
    p["b2cat"] = nc.declare_dram_parameter("b2cat", [64, 1], f32, isOutput=False)
    alp_out = nc.declare_dram_parameter("alp_out", [B, 128, NT], f32, isOutput=True)
    scal_out = nc.declare_dram_parameter("scal_out", [1, 10], f32, isOutput=True)

    with TileContext(nc) as tc:
        with (
            tc.tile_pool(name="per", bufs=1) as per,       # persistent
            tc.tile_pool(name="nbp", bufs=1) as nbp,       # nb storage
        ):
            # ---- persistent small tiles ----
            iota32_t = per.tile([128, 32], i32)
            nc.sync.dma_start(out=iota32_t[:], in_=p["iota32"][:, :])
            iota21_t = per.tile([128, 21], f32)
            nc.sync.dma_start(out=iota21_t[:], in_=p["iota21"][:, :])
            ident_f32 = per.tile([128, 128], f32)
            make_identity(nc, ident_f32[:])
            ones_col = per.tile([128, 1], f32)
            nc.vector.memset(ones_col[:], 1.0)
            acc_all = per.tile([128, 10], f32)
            nc.vector.memset(acc_all[:], 0.0)
            b2cat_t = per.tile([64, 1], f32)
            nc.sync.dma_start(out=b2cat_t[:], in_=p["b2cat"][:, :])

            # weights
            w0a_t = per.tile([IN_D, H], f32r)
            nc.gpsimd.dma_start(out=w0a_t[:], in_=p["w0a"][:, :])
            w0c_t = per.tile([IN_D, H], f32r)
            nc.gpsimd.dma_start(out=w0c_t[:], in_=p["w0c"][:, :])
            w1a_t = per.tile([128, 64, 128], bf16)
            nc.sync.dma_start(out=w1a_t[:], in_=p["w1a"][:, :, :])
            w1c_t = per.tile([128, 64, 128], bf16)
            nc.sync.dma_start(out=w1c_t[:], in_=p["w1c"][:, :, :])
            w2a_t = per.tile([128, 8, OUT_D], bf16)
            nc.sync.dma_start(out=w2a_t[:], in_=p["w2a"][:, :, :])
            w2c_t = per.tile([128, 8, 1], bf16)
            nc.sync.dma_start(out=w2c_t[:], in_=p["w2c"][:, :, :])
            biases = {}
            for nm in ("b0a", "b0c", "b1a", "b1c"):
                t = per.tile([128, 8], f32, name=nm)
                nc.sync.dma_start(out=t[:], in_=p[nm][:, :])
                biases[nm] = t

            # nb storage: [payload(4b x 32), node] f32
            nb_hi = nbp.tile([128, NPCP], f32)
            nb_lo = nbp.tile([128, NPCP], f32)
            nbr_hi = nbp.tile([128, NPCP], f32)
            nbr_lo = nbp.tile([128, NPCP], f32)
            nbt = {(0, 0): nb_hi, (0, 1): nb_lo, (1, 0): nbr_hi, (1, 1): nbr_lo}

            # ---- phase 0: build one-hot X in SBUF ----
            with tc.tile_pool(name="xp", bufs=1) as xp:
                KS = 80
                x_lo = xp.tile([128, KS, 256], fp8)
                x_hi = xp.tile([128, KT - KS, 256], fp8)
                nc.vector.memset(x_lo[:, :, :], 0.0)
                nc.vector.memset(x_hi[:, :, :], 0.0)

                def x_sb_at(k):
                    return (x_lo, k) if k < KS else (x_hi, k - KS)
                obx_sb = xp.tile([128, KT, 8], i32)
                nc.sync.dma_start(
                    out=obx_sb[:, : KT - 1, :],
                    in_=p["obx_g"][0 : (KT - 1) * 128, :].rearrange(
                        "(t q) b -> q t b", q=128
                    ),
                )
                # last partial tile: rows 19968..19999 (32 rows)
                nc.sync.dma_start(
                    out=obx_sb[:32, KT - 1, :],
                    in_=p["obx_g"][(KT - 1) * 128 :, :],
                )
                for t in range(KT):
                    hi = 128 if t < KT - 1 else 32
                    xt, tt = x_sb_at(t)
                    nc.vector.tensor_tensor(
                        out=xt[:hi, tt, :],
                        in0=obx_sb[:hi, t, :].rearrange(
                            "p (b one) -> p b one", one=1
                        ).to_broadcast([hi, 8, 32]),
                        in1=iota32_t[:hi, :].rearrange(
                            "p (one j) -> p one j", one=1
                        ).to_broadcast([hi, 8, 32]),
                        op=mybir.AluOpType.is_equal,
                    )

                # ---- phase 1: dense scatter matmuls ----
                with (
                    tc.tile_pool(name="ab", bufs=6) as ab,
                    tc.tile_pool(name="scps", bufs=3, space="PSUM") as scps,
                ):
                    for pas in range(2):
                        for r in range(NRANGE):
                            ps_hi = scps.tile([128, 512], f32, space="PSUM", tag="hi")
                            ps_lo = scps.tile([128, 512], f32, space="PSUM", tag="lo")
                            for k0 in range(0, KT, 16):
                                kk = min(16, KT - k0)
                                blk = (pas * NRANGE + r) * KT + k0
                                a_t = ab.tile([128, 16, 512], fp8, tag="a")
                                nc.sync.dma_start(
                                    out=a_t[:, :kk, :],
                                    in_=p["a_all"][blk : blk + kk, :, :].rearrange(
                                        "k q n -> q k n"),
                                )
                                for j0 in range(0, kk, 2):
                                    k0j = k0 + j0
                                    jj = min(2, kk - j0)
                                    if jj == 2:
                                        nc.tensor.matmul(
                                            ps_hi[:],
                                            lhsT=x_sb_at(k0j)[0][:, x_sb_at(k0j)[1] : x_sb_at(k0j)[1] + 2, 0:128],
                                            rhs=a_t[:, j0 : j0 + 2, :],
                                            start=(k0j == 0),
                                            stop=(k0j + 2 == KT),
                                            perf_mode=mybir.MatmulPerfMode.DoubleRow,
                                        )
                                        nc.tensor.matmul(
                                            ps_lo[:],
                                            lhsT=x_sb_at(k0j)[0][:, x_sb_at(k0j)[1] : x_sb_at(k0j)[1] + 2, 128:256],
                                            rhs=a_t[:, j0 : j0 + 2, :],
                                            start=(k0j == 0),
                                            stop=(k0j + 2 == KT),
                                            perf_mode=mybir.MatmulPerfMode.DoubleRow,
                                        )
                                    else:
                                        nc.tensor.matmul(
                                            ps_hi[:],
                                            lhsT=x_sb_at(k0j)[0][:, x_sb_at(k0j)[1], 0:128],
                                            rhs=a_t[:, j0, :],
                                            start=(k0j == 0),
                                            stop=(k0j == KT - 1),
                                        )
                                        nc.tensor.matmul(
                                            ps_lo[:],
                                            lhsT=x_sb_at(k0j)[0][:, x_sb_at(k0j)[1], 128:256],
                                            rhs=a_t[:, j0, :],
                                            start=(k0j == 0),
                                            stop=(k0j == KT - 1),
                                        )
                            nc.vector.tensor_copy(
                                out=nbt[(pas, 0)][:, r * 512 : (r + 1) * 512],
                                in_=ps_hi[:],
                            )
                            nc.vector.tensor_copy(
                                out=nbt[(pas, 1)][:, r * 512 : (r + 1) * 512],
                                in_=ps_lo[:],
                            )

            # ---- phase 2: MLPs + post ----
            with (
                tc.tile_pool(name="mlp", bufs=1) as mlp,
                tc.tile_pool(name="post", bufs=2) as post,
                tc.tile_pool(name="l0ps", bufs=2, space="PSUM") as l0ps,
                tc.tile_pool(name="l1ps", bufs=2, space="PSUM") as l1ps,
                tc.tile_pool(name="l2ps", bufs=1, space="PSUM") as l2ps,
                tc.tile_pool(name="trps", bufs=2, space="PSUM") as trps,
            ):
                alp_sb = mlp.tile([128, B * NT], f32)
                for b in range(B):
                    bq = (b % 4) * 32
                    plane = b // 4
                    hT = mlp.tile([IN_D, NPCP], f32r, tag="hT", bufs=2)
                    nc.gpsimd.dma_start(out=hT[0:1, :], in_=p["obtT"][b : b + 1, :])
                    nc.gpsimd.dma_start(
                        out=hT[1:21, :], in_=nbt[(0, plane)][bq : bq + 20, :]
                    )
                    nc.gpsimd.dma_start(
                        out=hT[21:41, :], in_=nbt[(1, plane)][bq : bq + 20, :]
                    )
                    nc.gpsimd.dma_start(out=hT[41:42, :], in_=p["obtT"][B : B + 1, :])

                    maskf = post.tile([128, NT], f32, tag="maskf")
                    obxn = post.tile([128, NT], i32, tag="obxn")
                    nc.sync.dma_start(out=obxn[:], in_=p["obxN"][b, :, :])
                    nc.vector.tensor_scalar(
                        out=maskf[:], in0=obxn[:], scalar1=0, scalar2=None,
                        op0=mybir.AluOpType.is_equal,
                    )
                    actf = post.tile([128, NT], f32, tag="actf")
                    actn = post.tile([128, NT], i32, tag="actn")
                    nc.sync.dma_start(out=actn[:], in_=p["actN"][b, :, :])
                    nc.vector.tensor_copy(out=actf[:], in_=actn[:])
                    msum = post.tile([128, 1], f32, tag="msum")
                    nc.vector.tensor_reduce(
                        out=msum[:], in_=maskf[:], axis=mybir.AxisListType.X,
                        op=mybir.AluOpType.add,
                    )
                    nc.vector.tensor_tensor(
                        out=acc_all[:, 9:10], in0=acc_all[:, 9:10], in1=msum[:],
                        op=mybir.AluOpType.add,
                    )

                    for rt in range(NRANGE):
                        cs = rt * 512
                        h0a = mlp.tile([128, 8, 512], bf16, tag="h0a", bufs=2)
                        h0c = mlp.tile([128, 8, 512], bf16, tag="h0c", bufs=2)
                        for net, w0t, h0t, b0 in (
                            ("a", w0a_t, h0a, biases["b0a"]),
                            ("c", w0c_t, h0c, biases["b0c"]),
                        ):
                            for m in range(8):
                                ps = l0ps.tile([128, 512], f32, space="PSUM", tag="l0")
                                nc.tensor.matmul(
                                    ps[:],
                                    lhsT=w0t[:, m * 128 : (m + 1) * 128],
                                    rhs=hT[:, cs : cs + 512],
                                    start=True, stop=True,
                                )
                                nc.scalar.activation(
                                    out=h0t[:, m, :], in_=ps[:],
                                    func=mybir.ActivationFunctionType.Relu,
                                    bias=b0[:, m : m + 1],
                                )
                        h1a = mlp.tile([128, 8, 512], bf16, tag="h1a", bufs=2)
                        h1c = mlp.tile([128, 8, 512], bf16, tag="h1c", bufs=2)
                        for m in range(8):
                            ps = l1ps.tile([128, 512], f32, space="PSUM", tag="l1")
                            for k in range(8):
                                nc.tensor.matmul(
                                    ps[:],
                                    lhsT=w1a_t[:, k * 8 + m, :],
                                    rhs=h0a[:, k, :],
                                    start=(k == 0), stop=(k == 7),
                                )
                            nc.scalar.activation(
                                out=h1a[:, m, :], in_=ps[:],
                                func=mybir.ActivationFunctionType.Relu,
                                bias=biases["b1a"][:, m : m + 1],
                            )
                        for m in range(8):
                            ps = l1ps.tile([128, 512], f32, space="PSUM", tag="l1")
                            for k in range(8):
                                nc.tensor.matmul(
                                    ps[:],
                                    lhsT=w1c_t[:, k * 8 + m, :],
                                    rhs=h0c[:, k, :],
                                    start=(k == 0), stop=(k == 7),
                                )
                            nc.scalar.activation(
                                out=h1c[:, m, :], in_=ps[:],
                                func=mybir.ActivationFunctionType.Relu,
                                bias=biases["b1c"][:, m : m + 1],
                            )
                        ps2 = l2ps.tile([64, 512], f32, space="PSUM", tag="l2")
                        for k in range(8):
                            nc.tensor.matmul(
                                ps2[0:OUT_D, :], lhsT=w2a_t[:, k, :], rhs=h1a[:, k, :],
                                start=(k == 0), stop=(k == 7),
                            )
                        for k in range(8):
                            nc.tensor.matmul(
                                ps2[32:33, :], lhsT=w2c_t[:, k, :], rhs=h1c[:, k, :],
                                start=(k == 0), stop=(k == 7),
                            )
                        catT = post.tile([64, 512], f32, tag="catT")
                        nc.vector.tensor_scalar(
                            out=catT[0:33, :], in0=ps2[0:33, :],
                            scalar1=b2cat_t[0:33, :], scalar2=None,
                            op0=mybir.AluOpType.add,
                        )
                        for ntile in range(4):
                            tg = rt * 4 + ntile
                            trp = trps.tile([128, 64], f32, space="PSUM", tag="tr")
                            nc.tensor.transpose(
                                out=trp[:],
                                in_=catT[:, ntile * 128 : (ntile + 1) * 128],
                                identity=ident_f32[0:64, 0:64],
                            )
                            ln = post.tile([128, 33], f32, tag="ln")
                            nc.vector.tensor_copy(out=ln[:], in_=trp[:, 0:33])
                            lg = ln[:, 0:OUT_D]
                            mx = post.tile([128, 1], f32, tag="mx")
                            nc.vector.tensor_reduce(
                                out=mx[:], in_=lg, axis=mybir.AxisListType.X,
                                op=mybir.AluOpType.max,
                            )
                            nmx = post.tile([128, 1], f32, tag="nmx")
                            nc.vector.tensor_scalar_mul(nmx[:], mx[:], -1.0)
                            ex = post.tile([128, OUT_D], f32, tag="ex")
                            s = post.tile([128, 1], f32, tag="s")
                            nc.scalar.activation(
                                out=ex[:], in_=lg,
                                func=mybir.ActivationFunctionType.Exp,
                                bias=nmx[:], accum_out=s[:],
                            )
                            logs = post.tile([128, 1], f32, tag="logs")
                            nc.scalar.activation(
                                out=logs[:], in_=s[:],
                                func=mybir.ActivationFunctionType.Ln,
                            )
                            sel = post.tile([128, OUT_D], f32, tag="sel")
                            nc.vector.tensor_scalar(
                                out=sel[:], in0=iota21_t[:],
                                scalar1=actf[:, tg : tg + 1], scalar2=None,
                                op0=mybir.AluOpType.is_equal,
                            )
                            junk = post.tile([128, OUT_D], f32, tag="junk")
                            asel = post.tile([128, 1], f32, tag="asel")
                            nc.vector.tensor_tensor(
                                out=junk[:], in0=sel[:], in1=lg,
                                op=mybir.AluOpType.mult,
                            )
                            nc.vector.tensor_reduce(
                                out=asel[:], in_=junk[:],
                                axis=mybir.AxisListType.X, op=mybir.AluOpType.add,
                            )
                            junk2 = post.tile([128, OUT_D], f32, tag="junk2")
                            t3 = post.tile([128, 1], f32, tag="t3")
                            nc.vector.tensor_tensor(
                                out=junk2[:], in0=ex[:], in1=lg,
                                op=mybir.AluOpType.mult,
                            )
                            nc.vector.tensor_reduce(
                                out=t3[:], in_=junk2[:],
                                axis=mybir.AxisListType.X, op=mybir.AluOpType.add,
                            )
                            # alp = (asel - mx - logs) * mask
                            alp0 = post.tile([128, 1], f32, tag="alp0")
                            nc.vector.tensor_scalar(
                                out=alp0[:], in0=asel[:], scalar1=mx[:],
                                scalar2=None, op0=mybir.AluOpType.subtract,
                            )
                            nc.vector.tensor_tensor(
                                out=alp0[:], in0=alp0[:], in1=logs[:],
                                op=mybir.AluOpType.subtract,
                            )
                            nc.vector.tensor_tensor(
                                out=alp_sb[:, b * NT + tg : b * NT + tg + 1],
                                in0=alp0[:], in1=maskf[:, tg : tg + 1],
                                op=mybir.AluOpType.mult,
                            )
                            # ent = mx + logs - t3 / s
                            rs = post.tile([128, 1], f32, tag="rs")
                            nc.vector.reciprocal(rs[:], s[:])
                            ent0 = post.tile([128, 1], f32, tag="ent0")
                            nc.vector.tensor_tensor(
                                out=ent0[:], in0=t3[:], in1=rs[:],
                                op=mybir.AluOpType.mult,
                            )
                            nc.vector.tensor_scalar(
                                out=ent0[:], in0=ent0[:], scalar1=-1.0,
                                scalar2=mx[:], op0=mybir.AluOpType.mult,
                                op1=mybir.AluOpType.add,
                            )
                            nc.vector.tensor_tensor(
                                out=ent0[:], in0=ent0[:], in1=logs[:],
                                op=mybir.AluOpType.add,
                            )
                            nc.vector.tensor_tensor(
                                out=ent0[:], in0=ent0[:], in1=maskf[:, tg : tg + 1],
                                op=mybir.AluOpType.mult,
                            )
                            nc.vector.tensor_tensor(
                                out=acc_all[:, 8:9], in0=acc_all[:, 8:9],
                                in1=ent0[:], op=mybir.AluOpType.add,
                            )
                            # value
                            vm = post.tile([128, 1], f32, tag="vm")
                            nc.vector.tensor_tensor(
                                out=vm[:], in0=ln[:, 32:33],
                                in1=maskf[:, tg : tg + 1], op=mybir.AluOpType.mult,
                            )
                            nc.vector.tensor_tensor(
                                out=acc_all[:, b : b + 1], in0=acc_all[:, b : b + 1],
                                in1=vm[:], op=mybir.AluOpType.add,
                            )
                    nc.sync.dma_start(
                        out=alp_out[b, :, :], in_=alp_sb[:, b * NT : (b + 1) * NT]
                    )

                # final partition reduce via ones matmul
                with tc.tile_pool(name="rdps", bufs=1, space="PSUM") as rdps:
                    red = rdps.tile([1, 10], f32, space="PSUM")
                    nc.tensor.matmul(
                        red[:], lhsT=ones_col[:], rhs=acc_all[:],
                        start=True, stop=True,
                    )
                    scal_sb = per.tile([1, 10], f32)
                    nc.vector.tensor_copy(out=scal_sb[:], in_=red[:])
                    nc.sync.dma_start(out=scal_out[:, :], in_=scal_sb[:])

    tile_patch.fix_waits(nc)
    return nc


def _prep_inputs(ob_x, ob_t, action, src, dst, aW0, ab0, aW1, ab1, aW2, ab2,
                 cW0, cb0, cW1, cb1, cW2, cb2):
    ob_x = np.asarray(ob_x, np.int32)
    ob_t = np.asarray(ob_t, np.float32)
    action = np.asarray(action, np.int32)
    src = np.asarray(src, np.int64)
    dst = np.asarray(dst, np.int64)

    iota32 = np.tile(np.arange(1, 33, dtype=np.int32), (128, 1))
    iota21 = np.tile(np.arange(OUT_D, dtype=np.float32), (128, 1))
    w1a = np.ascontiguousarray(
        np.asarray(aW1, np.float32).reshape(8, 128, 8, 128)
        .transpose(1, 0, 2, 3).reshape(128, 64, 128)).astype(ml_dtypes.bfloat16)
    w1c = np.ascontiguousarray(
        np.asarray(cW1, np.float32).reshape(8, 128, 8, 128)
        .transpose(1, 0, 2, 3).reshape(128, 64, 128)).astype(ml_dtypes.bfloat16)
    w2a = np.ascontiguousarray(
        np.asarray(aW2, np.float32).reshape(8, 128, OUT_D).transpose(1, 0, 2)
    ).astype(ml_dtypes.bfloat16)
    w2c = np.ascontiguousarray(
        np.asarray(cW2, np.float32).reshape(8, 128, 1).transpose(1, 0, 2)
    ).astype(ml_dtypes.bfloat16)
    b0a = np.ascontiguousarray(np.asarray(ab0, np.float32).reshape(8, 128).T)
    b0c = np.ascontiguousarray(np.asarray(cb0, np.float32).reshape(8, 128).T)
    b1a = np.ascontiguousarray(np.asarray(ab1, np.float32).reshape(8, 128).T)
    b1c = np.ascontiguousarray(np.asarray(cb1, np.float32).reshape(8, 128).T)
    b2cat = np.zeros((64, 1), np.float32)
    b2cat[0:OUT_D, 0] = np.asarray(ab2, np.float32)
    b2cat[32, 0] = np.asarray(cb2, np.float32)[0]

    shared = {
        "obx_g": ob_x, "iota32": iota32, "iota21": iota21,
        "w0a": np.asarray(aW0, np.float32), "w0c": np.asarray(cW0, np.float32),
        "w1a": w1a, "w1c": w1c, "w2a": w2a, "w2c": w2c,
        "b0a": b0a, "b0c": b0c, "b1a": b1a, "b1c": b1c, "b2cat": b2cat,
    }

    in_maps = []
    for k in range(NCORES):
        lo, hi = k * NPC, (k + 1) * NPC
        obtT = np.zeros((B + 1, NPCP), np.float32)
        obtT[:B, :NPC] = ob_t[lo:hi, :].T
        obtT[B, :] = 1.0
        obxN = np.ones((B, 128, NT), np.int32)     # pad color 1 -> mask 0
        actN = np.zeros((B, 128, NT), np.int32)
        obx_loc = ob_x[lo:hi].T                    # [B, NPC]
        act_loc = action[lo:hi].T
        padded_x = np.ones((B, NPCP), np.int32)
        padded_x[:, :NPC] = obx_loc
        padded_a = np.zeros((B, NPCP), np.int32)
        padded_a[:, :NPC] = act_loc
        obxN[:] = padded_x.reshape(B, NT, 128).transpose(0, 2, 1)
        actN[:] = padded_a.reshape(B, NT, 128).transpose(0, 2, 1)

        a_all = np.zeros((NBLK, 128, 512), np.uint8)
        for pas, (g_arr, s_arr) in enumerate(((src, dst), (dst, src))):
            selm = (s_arr >= lo) & (s_arr < hi)
            gg = g_arr[selm]
            ss = s_arr[selm] - lo
            dense = np.zeros((NP, NPCP), np.uint8)
            np.add.at(dense, (gg, ss), 1)
            blocks = dense.reshape(KT, 128, NRANGE, 512).transpose(2, 0, 1, 3)
            a_all[pas * NRANGE * KT:(pas + 1) * NRANGE * KT] = blocks.reshape(
                NRANGE * KT, 128, 512)
            del dense
        a_fp8 = _FP8_LUT[a_all]
        del a_all

        m = dict(shared)
        m.update({"obtT": obtT, "obxN": obxN, "actN": actN, "a_all": a_fp8})
        in_maps.append(m)
    return in_maps


def kernel(**inputs):
    global LAST_EXEC_NS
    if "nc" not in _CACHED:
        _CACHED["nc"] = _build_program()
    nc = _CACHED["nc"]
    in_maps = _prep_inputs(**inputs)
    trace = bool(os.environ.get("KBENCH_TRACE"))
    res = run_bass_kernel_spmd(
        nc, in_maps, core_ids=list(range(NCORES)), trace=trace)
    LAST_EXEC_NS = res.exec_time_ns

    alp = np.zeros((N, B), np.float32)
    vsum = np.zeros(B, np.float64)
    esum = 0.0
    msum = 0.0
    for k in range(NCORES):
        out = res.results[k]
        a = out["alp_out"]                     # [B, 128, NT]
        loc = a.transpose(2, 1, 0).reshape(NPCP, B)[:NPC]
        alp[k * NPC:(k + 1) * NPC] = loc
        sc = out["scal_out"][0]
        vsum += sc[0:8].astype(np.float64)
        esum += float(sc[8])
        msum += float(sc[9])
    avg_entropy = np.float32(esum / max(msum, 1.0))
    value_preds = (vsum / MAXN).astype(np.float32)
    return alp, avg_entropy, value_preds


# revision 22
# speedup vs baseline: 1.0921x; 1.0413x over previous
"""Trainium2 Bass kernel for nn_ActorCritic (GNN message passing + actor/critic MLPs).

Sharding: nodes are partitioned across the 8 NeuronCores (2500 nodes each, all
8 batch elements on every core). The one-hop segment_sums (nb, nb_rev) are
computed as dense fp8 matmuls against host-built adjacency blocks:
    nbT[pay, dst] = sum_src X[src, pay] * A[src, dst]
with X = one-hot colors built on device (payload = batch*32 + color, 256 wide,
split into hi/lo 128-partition planes) and A the 0/1 adjacency (pure index
data). The MLPs run with float32r (layer 0) and bfloat16 (layers 1/2) matmuls;
log-softmax / entropy / value reductions run on DVE/ACT after a PE transpose
back to node-major layout. Outputs are gathered and re-assembled on host.
"""
import os
import sys

sys.path.insert(0, "/opt/trn_rl_repo")
sys.path.insert(0, os.path.dirname(os.path.abspath(__file__)))

import numpy as np
import ml_dtypes

import concourse.bass as bass
import concourse.mybir as mybir
import concourse.tile as tile_mod
from concourse.tile import TileContext
from concourse.bass_utils import run_bass_kernel_spmd
from concourse.masks import make_identity
from concourse.vector_clock import ScopedClock


# ---------------------------------------------------------------------------
# Walrus-compat patches: this build rejects >1 sem wait per instruction, and
# the stock TileContext tail drain carries one wait per live logical proc.
# ---------------------------------------------------------------------------

MAX_WAITS = 1


def _patched_drain_and_barrier(self, tick_clock, wait_clock):
    nc = self.nc
    probe = nc.sync.nop()
    wait_clock.add_sem_waits(probe.ins, ScopedClock({None: tick_clock.global_clock}))
    si = probe.ins.sync_info
    waits = list(si.on_wait) if si is not None else []
    if len(waits) > MAX_WAITS:
        si.on_wait = waits[:MAX_WAITS]
        rest = waits[MAX_WAITS:]
        for j in range(0, len(rest), MAX_WAITS):
            n = nc.sync.nop()
            nsi = n.ins.sync_info
            if nsi is None:
                n.ins.sync_info = mybir.SyncInfo(
                    on_update=[], on_wait=rest[j : j + MAX_WAITS]
                )
            else:
                nsi.on_wait = rest[j : j + MAX_WAITS]
    nc.sync.drain()
    nc.all_engine_barrier()
    assert self.sems is not None
    popped = nc._tile_sem_poison_stack.pop()
    assert popped is self._sem_poison
    nc.clear_and_free_semaphores(list(self.sems.allocated().values()))
    nc.all_engine_barrier()


_ws_counter = [0]


def fix_waits(nc, max_waits: int = 1):
    """Post-pass over the finished module: any instruction carrying more than
    ``max_waits`` sem waits gets the excess hoisted onto same-engine NoOps
    inserted immediately before it (this walrus build rejects multi-wait
    instructions at codegen)."""
    for f in nc.m.functions:
        for blk in f.blocks:
            insns = blk.instructions
            out = []
            changed = False
            for ins in insns:
                si = ins.sync_info
                if si is not None and len(si.on_wait) > max_waits:
                    waits = list(si.on_wait)
                    keep = waits[: max_waits]
                    rest = waits[max_waits:]
                    for j in range(0, len(rest), max_waits):
                        _ws_counter[0] += 1
                        nop = mybir.InstNoOp(
                            name=f"WSPLIT-{_ws_counter[0]}",
                            ins=[],
                            outs=[],
                            engine=ins.engine,
                            sync_info=mybir.SyncInfo(
                                on_update=[], on_wait=rest[j : j + max_waits]
                            ),
                        )
                        out.append(nop)
                    si.on_wait = keep
                    changed = True
                out.append(ins)
            if changed:
                blk.instructions = out


def install():
    tile_mod.TileContext._drain_and_barrier = _patched_drain_and_barrier


class _TP:
    fix_waits = staticmethod(fix_waits)


tile_patch = _TP()
install()

# ---- problem constants (hardcoded per spec) ----
N, B, NCOL, H, E = 20000, 8, 20, 1024, 320000
IN_D, OUT_D, MAXN = 42, 21, 20000
NCORES = 8
NPC = N // NCORES            # 2500 nodes per core
NPCP = 2560                  # padded: 20 x 128 = 5 x 512
NT = NPCP // 128             # 20 node tiles per core
NRANGE = NPCP // 512         # 5 psum ranges
KT = 157                     # src k-chunks (ceil(20000/128))
NP = KT * 128                # 20096 padded source nodes
NBLK = 2 * NRANGE * KT       # 1570 adjacency blocks per core

f32 = mybir.dt.float32
f32r = mybir.dt.float32r
bf16 = mybir.dt.bfloat16
fp8 = mybir.dt.float8e4
i32 = mybir.dt.int32

_FP8_LUT = np.arange(256, dtype=np.uint8).astype(np.float32).astype(
    ml_dtypes.float8_e4m3)

LAST_EXEC_NS = None
_CACHED = {}


def _build_program():
    nc = bass.Bass("TRN2")
    p = {}
    p["obx_g"] = nc.declare_dram_parameter("obx_g", [N, B], i32, isOutput=False)
    p["obtT"] = nc.declare_dram_parameter("obtT", [B + 1, NPCP], f32, isOutput=False)
    p["obxN"] = nc.declare_dram_parameter("obxN", [B, 128, NT], i32, isOutput=False)
    p["actN"] = nc.declare_dram_parameter("actN", [B, 128, NT], i32, isOutput=False)
    p["a_all"] = nc.declare_dram_parameter("a_all", [NBLK, 128, 512], fp8, isOutput=False)
    p["iota32"] = nc.declare_dram_parameter("iota32", [128, 32], i32, isOutput=False)
    p["iota21"] = nc.declare_dram_parameter("iota21", [128, 21], f32, isOutput=False)
    p["w0a"] = nc.declare_dram_parameter("w0a", [IN_D, H], f32, isOutput=False)
    p["w0c"] = nc.declare_dram_parameter("w0c", [IN_D, H], f32, isOutput=False)
    p["w1a"] = nc.declare_dram_parameter("w1a", [128, 64, 128], bf16, isOutput=False)
    p["w1c"] = nc.declare_dram_parameter("w1c", [128, 64, 128], bf16, isOutput=False)
    p["w2a"] = nc.declare_dram_parameter("w2a", [128, 8, OUT_D], bf16, isOutput=False)
    p["w2c"] = nc.declare_dram_parameter("w2c", [128, 8, 1], bf16, isOutput=False)
    p["b0a"] = nc.declare_dram_parameter("b0a", [128, 8], f32, isOutput=False)
    p["b0c"] = nc.declare_dram_parameter("b0c", [128, 8], f32, isOutput=False)
    p["b1a"] = nc.declare_dram_parameter("b1a", [128, 8], f32, isOutput=False)
    p["b1c"] = nc.declare_dram_parameter("b1c", [128, 8], f32, isOutput=False)
    p["b2cat"] = nc.declare_dram_parameter("b2cat", [64, 1], f32, isOutput=False)
    alp_out = nc.declare_dram_parameter("alp_out", [B, 128, NT], f32, isOutput=True)
    scal_out = nc.declare_dram_parameter("scal_out", [1, 10], f32, isOutput=True)

    with TileContext(nc) as tc:
        with (
            tc.tile_pool(name="per", bufs=1) as per,       # persistent
            tc.tile_pool(name="nbp", bufs=1) as nbp,       # nb storage
        ):
            # ---- persistent small tiles ----
            iota32_t = per.tile([128, 32], i32)
            nc.sync.dma_start(out=iota32_t[:], in_=p["iota32"][:, :])
            iota21_t = per.tile([128, 21], f32)
            nc.sync.dma_start(out=iota21_t[:], in_=p["iota21"][:, :])
            ident_f32 = per.tile([128, 128], f32)
            make_identity(nc, ident_f32[:])
            ones_col = per.tile([128, 1], f32)
            nc.vector.memset(ones_col[:], 1.0)
            acc_all = per.tile([128, 10], f32)
            nc.vector.memset(acc_all[:], 0.0)
            b2cat_t = per.tile([64, 1], f32)
            nc.sync.dma_start(out=b2cat_t[:], in_=p["b2cat"][:, :])

            # weights
            w0a_t = per.tile([IN_D, H], f32r)
            nc.gpsimd.dma_start(out=w0a_t[:], in_=p["w0a"][:, :])
            w0c_t = per.tile([IN_D, H], f32r)
            nc.gpsimd.dma_start(out=w0c_t[:], in_=p["w0c"][:, :])
            w1a_t = per.tile([128, 64, 128], bf16)
            nc.sync.dma_start(out=w1a_t[:], in_=p["w1a"][:, :, :])
            w1c_t = per.tile([128, 64, 128], bf16)
            nc.sync.dma_start(out=w1c_t[:], in_=p["w1c"][:, :, :])
            w2a_t = per.tile([128, 8, OUT_D], bf16)
            nc.sync.dma_start(out=w2a_t[:], in_=p["w2a"][:, :, :])
            w2c_t = per.tile([128, 8, 1], bf16)
            nc.sync.dma_start(out=w2c_t[:], in_=p["w2c"][:, :, :])
            biases = {}
            for nm in ("b0a", "b0c", "b1a", "b1c"):
                t = per.tile([128, 8], f32, name=nm)
                nc.sync.dma_start(out=t[:], in_=p[nm][:, :])
                biases[nm] = t

            # nb storage: [payload(4b x 32), node] f32
            nb_hi = nbp.tile([128, NPCP], f32)
            nb_lo = nbp.tile([128, NPCP], f32)
            nbr_hi = nbp.tile([128, NPCP], f32)
            nbr_lo = nbp.tile([128, NPCP], f32)
            nbt = {(0, 0): nb_hi, (0, 1): nb_lo, (1, 0): nbr_hi, (1, 1): nbr_lo}

            # ---- phase 0: build one-hot X in SBUF ----
            with tc.tile_pool(name="xp", bufs=1) as xp:
                KS = 80
                x_lo = xp.tile([128, KS, 256], fp8)
                x_hi = xp.tile([128, KT - KS, 256], fp8)
                nc.vector.memset(x_lo[:, :, :], 0.0)
                nc.vector.memset(x_hi[:, :, :], 0.0)

                def x_sb_at(k):
                    return (x_lo, k) if k < KS else (x_hi, k - KS)
                obx_sb = xp.tile([128, KT, 8], i32)
                nc.sync.dma_start(
                    out=obx_sb[:, : KT - 1, :],
                    in_=p["obx_g"][0 : (KT - 1) * 128, :].rearrange(
                        "(t q) b -> q t b", q=128
                    ),
                )
                # last partial tile: rows 19968..19999 (32 rows)
                nc.sync.dma_start(
                    out=obx_sb[:32, KT - 1, :],
                    in_=p["obx_g"][(KT - 1) * 128 :, :],
                )
                for t in range(KT):
                    hi = 128 if t < KT - 1 else 32
                    xt, tt = x_sb_at(t)
                    nc.vector.tensor_tensor(
                        out=xt[:hi, tt, :],
                        in0=obx_sb[:hi, t, :].rearrange(
                            "p (b one) -> p b one", one=1
                        ).to_broadcast([hi, 8, 32]),
                        in1=iota32_t[:hi, :].rearrange(
                            "p (one j) -> p one j", one=1
                        ).to_broadcast([hi, 8, 32]),
                        op=mybir.AluOpType.is_equal,
                    )

                # ---- phase 1: dense scatter matmuls ----
                with (
                    tc.tile_pool(name="ab", bufs=6) as ab,
                    tc.tile_pool(name="scps", bufs=3, space="PSUM") as scps,
                ):
                    for pas in range(2):
                        for r in range(NRANGE):
                            ps_hi = scps.tile([128, 512], f32, space="PSUM", tag="hi")
                            ps_lo = scps.tile([128, 512], f32, space="PSUM", tag="lo")
                            for k0 in range(0, KT, 16):
                                kk = min(16, KT - k0)
                                blk = (pas * NRANGE + r) * KT + k0
                                a_t = ab.tile([128, 16, 512], fp8, tag="a")
                                nc.sync.dma_start(
                                    out=a_t[:, :kk, :],
                                    in_=p["a_all"][blk : blk + kk, :, :].rearrange(
                                        "k q n -> q k n"),
                                )
                                for j0 in range(0, kk, 2):
                                    k0j = k0 + j0
                                    jj = min(2, kk - j0)
                                    if jj == 2:
                                        nc.tensor.matmul(
                                            ps_hi[:],
                                            lhsT=x_sb_at(k0j)[0][:, x_sb_at(k0j)[1] : x_sb_at(k0j)[1] + 2, 0:128],
                                            rhs=a_t[:, j0 : j0 + 2, :],
                                            start=(k0j == 0),
                                            stop=(k0j + 2 == KT),
                                            perf_mode=mybir.MatmulPerfMode.DoubleRow,
                                        )
                                        nc.tensor.matmul(
                                            ps_lo[:],
                                            lhsT=x_sb_at(k0j)[0][:, x_sb_at(k0j)[1] : x_sb_at(k0j)[1] + 2, 128:256],
                                            rhs=a_t[:, j0 : j0 + 2, :],
                                            start=(k0j == 0),
                                            stop=(k0j + 2 == KT),
                                            perf_mode=mybir.MatmulPerfMode.DoubleRow,
                                        )
                                    else:
                                        nc.tensor.matmul(
                                            ps_hi[:],
                                            lhsT=x_sb_at(k0j)[0][:, x_sb_at(k0j)[1], 0:128],
                                            rhs=a_t[:, j0, :],
                                            start=(k0j == 0),
                                            stop=(k0j == KT - 1),
                                        )
                                        nc.tensor.matmul(
                                            ps_lo[:],
                                            lhsT=x_sb_at(k0j)[0][:, x_sb_at(k0j)[1], 128:256],
                                            rhs=a_t[:, j0, :],
                                            start=(k0j == 0),
                                            stop=(k0j == KT - 1),
                                        )
                            nc.vector.tensor_copy(
                                out=nbt[(pas, 0)][:, r * 512 : (r + 1) * 512],
                                in_=ps_hi[:],
                            )
                            nc.vector.tensor_copy(
                                out=nbt[(pas, 1)][:, r * 512 : (r + 1) * 512],
                                in_=ps_lo[:],
                            )

            # ---- phase 2: MLPs + post ----
            with (
                tc.tile_pool(name="mlp", bufs=1) as mlp,
                tc.tile_pool(name="post", bufs=2) as post,
                tc.tile_pool(name="l0ps", bufs=2, space="PSUM") as l0ps,
                tc.tile_pool(name="l1ps", bufs=3, space="PSUM") as l1ps,
                tc.tile_pool(name="l2ps", bufs=1, space="PSUM") as l2ps,
                tc.tile_pool(name="trps", bufs=2, space="PSUM") as trps,
            ):
                alp_sb = mlp.tile([128, B * NT], f32)
                for b in range(B):
                    bq = (b % 4) * 32
                    plane = b // 4
                    hT = mlp.tile([IN_D, NPCP], f32r, tag="hT", bufs=2)
                    nc.gpsimd.dma_start(out=hT[0:1, :], in_=p["obtT"][b : b + 1, :])
                    nc.gpsimd.dma_start(
                        out=hT[1:21, :], in_=nbt[(0, plane)][bq : bq + 20, :]
                    )
                    nc.gpsimd.dma_start(
                        out=hT[21:41, :], in_=nbt[(1, plane)][bq : bq + 20, :]
                    )
                    nc.gpsimd.dma_start(out=hT[41:42, :], in_=p["obtT"][B : B + 1, :])

                    maskf = post.tile([128, NT], f32, tag="maskf")
                    obxn = post.tile([128, NT], i32, tag="obxn")
                    nc.sync.dma_start(out=obxn[:], in_=p["obxN"][b, :, :])
                    nc.vector.tensor_scalar(
                        out=maskf[:], in0=obxn[:], scalar1=0, scalar2=None,
                        op0=mybir.AluOpType.is_equal,
                    )
                    actf = post.tile([128, NT], f32, tag="actf")
                    actn = post.tile([128, NT], i32, tag="actn")
                    nc.sync.dma_start(out=actn[:], in_=p["actN"][b, :, :])
                    nc.vector.tensor_copy(out=actf[:], in_=actn[:])
                    msum = post.tile([128, 1], f32, tag="msum")
                    nc.vector.tensor_reduce(
                        out=msum[:], in_=maskf[:], axis=mybir.AxisListType.X,
                        op=mybir.AluOpType.add,
                    )
                    nc.vector.tensor_tensor(
                        out=acc_all[:, 9:10], in0=acc_all[:, 9:10], in1=msum[:],
                        op=mybir.AluOpType.add,
                    )

                    for rt in range(NRANGE):
                        cs = rt * 512
                        h0a = mlp.tile([128, 8, 512], bf16, tag="h0a", bufs=2)
                        h0c = mlp.tile([128, 8, 512], bf16, tag="h0c", bufs=2)
                        for net, w0t, h0t, b0 in (
                            ("a", w0a_t, h0a, biases["b0a"]),
                            ("c", w0c_t, h0c, biases["b0c"]),
                        ):
                            for m in range(8):
                                ps = l0ps.tile([128, 512], f32, space="PSUM", tag="l0")
                                nc.tensor.matmul(
                                    ps[:],
                                    lhsT=w0t[:, m * 128 : (m + 1) * 128],
                                    rhs=hT[:, cs : cs + 512],
                                    start=True, stop=True,
                                )
                                nc.scalar.activation(
                                    out=h0t[:, m, :], in_=ps[:],
                                    func=mybir.ActivationFunctionType.Relu,
                                    bias=b0[:, m : m + 1],
                                )
                        h1a = mlp.tile([128, 8, 512], bf16, tag="h1a", bufs=2)
                        h1c = mlp.tile([128, 8, 512], bf16, tag="h1c", bufs=2)
                        for m in range(8):
                            ps = l1ps.tile([128, 512], f32, space="PSUM", tag="l1")
                            for k in range(8):
                                nc.tensor.matmul(
                                    ps[:],
                                    lhsT=w1a_t[:, k * 8 + m, :],
                                    rhs=h0a[:, k, :],
                                    start=(k == 0), stop=(k == 7),
                                )
                            nc.scalar.activation(
                                out=h1a[:, m, :], in_=ps[:],
                                func=mybir.ActivationFunctionType.Relu,
                                bias=biases["b1a"][:, m : m + 1],
                            )
                        for m in range(8):
                            ps = l1ps.tile([128, 512], f32, space="PSUM", tag="l1")
                            for k in range(8):
                                nc.tensor.matmul(
                                    ps[:],
                                    lhsT=w1c_t[:, k * 8 + m, :],
                                    rhs=h0c[:, k, :],
                                    start=(k == 0), stop=(k == 7),
                                )
                            nc.scalar.activation(
                                out=h1c[:, m, :], in_=ps[:],
                                func=mybir.ActivationFunctionType.Relu,
                                bias=biases["b1c"][:, m : m + 1],
                            )
                        ps2 = l2ps.tile([64, 512], f32, space="PSUM", tag="l2")
                        for k in range(8):
                            nc.tensor.matmul(
                                ps2[0:OUT_D, :], lhsT=w2a_t[:, k, :], rhs=h1a[:, k, :],
                                start=(k == 0), stop=(k == 7),
                            )
                        for k in range(8):
                            nc.tensor.matmul(
                                ps2[32:33, :], lhsT=w2c_t[:, k, :], rhs=h1c[:, k, :],
                                start=(k == 0), stop=(k == 7),
                            )
                        catT = post.tile([64, 512], f32, tag="catT")
                        nc.vector.tensor_scalar(
                            out=catT[0:33, :], in0=ps2[0:33, :],
                            scalar1=b2cat_t[0:33, :], scalar2=None,
                            op0=mybir.AluOpType.add,
                        )
                        for ntile in range(4):
                            tg = rt * 4 + ntile
                            trp = trps.tile([128, 64], f32, space="PSUM", tag="tr")
                            nc.tensor.transpose(
                                out=trp[:],
                                in_=catT[:, ntile * 128 : (ntile + 1) * 128],
                                identity=ident_f32[0:64, 0:64],
                            )
                            ln = post.tile([128, 33], f32, tag="ln")
                            nc.vector.tensor_copy(out=ln[:], in_=trp[:, 0:33])
                            lg = ln[:, 0:OUT_D]
                            mx = post.tile([128, 1], f32, tag="mx")
                            nc.vector.tensor_reduce(
                                out=mx[:], in_=lg, axis=mybir.AxisListType.X,
                                op=mybir.AluOpType.max,
                            )
                            nmx = post.tile([128, 1], f32, tag="nmx")
                            nc.vector.tensor_scalar_mul(nmx[:], mx[:], -1.0)
                            ex = post.tile([128, OUT_D], f32, tag="ex")
                            s = post.tile([128, 1], f32, tag="s")
                            nc.scalar.activation(
                                out=ex[:], in_=lg,
                                func=mybir.ActivationFunctionType.Exp,
                                bias=nmx[:], accum_out=s[:],
                            )
                            logs = post.tile([128, 1], f32, tag="logs")
                            nc.scalar.activation(
                                out=logs[:], in_=s[:],
                                func=mybir.ActivationFunctionType.Ln,
                            )
                            sel = post.tile([128, OUT_D], f32, tag="sel")
                            nc.vector.tensor_scalar(
                                out=sel[:], in0=iota21_t[:],
                                scalar1=actf[:, tg : tg + 1], scalar2=None,
                                op0=mybir.AluOpType.is_equal,
                            )
                            junk = post.tile([128, OUT_D], f32, tag="junk")
                            asel = post.tile([128, 1], f32, tag="asel")
                            nc.vector.tensor_tensor(
                                out=junk[:], in0=sel[:], in1=lg,
                                op=mybir.AluOpType.mult,
                            )
                            nc.vector.tensor_reduce(
                                out=asel[:], in_=junk[:],
                                axis=mybir.AxisListType.X, op=mybir.AluOpType.add,
                            )
                            junk2 = post.tile([128, OUT_D], f32, tag="junk2")
                            t3 = post.tile([128, 1], f32, tag="t3")
                            nc.vector.tensor_tensor(
                                out=junk2[:], in0=ex[:], in1=lg,
                                op=mybir.AluOpType.mult,
                            )
                            nc.vector.tensor_reduce(
                                out=t3[:], in_=junk2[:],
                                axis=mybir.AxisListType.X, op=mybir.AluOpType.add,
                            )
                            # alp = (asel - mx - logs) * mask
                            alp0 = post.tile([128, 1], f32, tag="alp0")
                            nc.vector.tensor_scalar(
                                out=alp0[:], in0=asel[:], scalar1=mx[:],
                                scalar2=None, op0=mybir.AluOpType.subtract,
                            )
                            nc.vector.tensor_tensor(
                                out=alp0[:], in0=alp0[:], in1=logs[:],
                                op=mybir.AluOpType.subtract,
                            )
                            nc.vector.tensor_tensor(
                                out=alp_sb[:, b * NT + tg : b * NT + tg + 1],
                                in0=alp0[:], in1=maskf[:, tg : tg + 1],
                                op=mybir.AluOpType.mult,
                            )
                            # ent = mx + logs - t3 / s
                            rs = post.tile([128, 1], f32, tag="rs")
                            nc.vector.reciprocal(rs[:], s[:])
                            ent0 = post.tile([128, 1], f32, tag="ent0")
                            nc.vector.tensor_tensor(
                                out=ent0[:], in0=t3[:], in1=rs[:],
                                op=mybir.AluOpType.mult,
                            )
                            nc.vector.tensor_scalar(
                                out=ent0[:], in0=ent0[:], scalar1=-1.0,
                                scalar2=mx[:], op0=mybir.AluOpType.mult,
                                op1=mybir.AluOpType.add,
                            )
                            nc.vector.tensor_tensor(
                                out=ent0[:], in0=ent0[:], in1=logs[:],
                                op=mybir.AluOpType.add,
                            )
                            nc.vector.tensor_tensor(
                                out=ent0[:], in0=ent0[:], in1=maskf[:, tg : tg + 1],
                                op=mybir.AluOpType.mult,
                            )
                            nc.vector.tensor_tensor(
                                out=acc_all[:, 8:9], in0=acc_all[:, 8:9],
                                in1=ent0[:], op=mybir.AluOpType.add,
                            )
                            # value
                            vm = post.tile([128, 1], f32, tag="vm")
                            nc.vector.tensor_tensor(
                                out=vm[:], in0=ln[:, 32:33],
                                in1=maskf[:, tg : tg + 1], op=mybir.AluOpType.mult,
                            )
                            nc.vector.tensor_tensor(
                                out=acc_all[:, b : b + 1], in0=acc_all[:, b : b + 1],
                                in1=vm[:], op=mybir.AluOpType.add,
                            )
                    nc.sync.dma_start(
                        out=alp_out[b, :, :], in_=alp_sb[:, b * NT : (b + 1) * NT]
                    )

                # final partition reduce via ones matmul (reuses a trps slot)
                if True:
                    red = trps.tile([128, 10], f32, space="PSUM", tag="tr")
                    nc.tensor.matmul(
                        red[0:1, :], lhsT=ones_col[:], rhs=acc_all[:],
                        start=True, stop=True,
                    )
                    scal_sb = per.tile([1, 10], f32)
                    nc.vector.tensor_copy(out=scal_sb[:], in_=red[0:1, :])
                    nc.sync.dma_start(out=scal_out[:, :], in_=scal_sb[:])

    tile_patch.fix_waits(nc)
    return nc


def _prep_inputs(ob_x, ob_t, action, src, dst, aW0, ab0, aW1, ab1, aW2, ab2,
                 cW0, cb0, cW1, cb1, cW2, cb2):
    ob_x = np.asarray(ob_x, np.int32)
    ob_t = np.asarray(ob_t, np.float32)
    action = np.asarray(action, np.int32)
    src = np.asarray(src, np.int64)
    dst = np.asarray(dst, np.int64)

    iota32 = np.tile(np.arange(1, 33, dtype=np.int32), (128, 1))
    iota21 = np.tile(np.arange(OUT_D, dtype=np.float32), (128, 1))
    w1a = np.ascontiguousarray(
        np.asarray(aW1, np.float32).reshape(8, 128, 8, 128)
        .transpose(1, 0, 2, 3).reshape(128, 64, 128)).astype(ml_dtypes.bfloat16)
    w1c = np.ascontiguousarray(
        np.asarray(cW1, np.float32).reshape(8, 128, 8, 128)
        .transpose(1, 0, 2, 3).reshape(128, 64, 128)).astype(ml_dtypes.bfloat16)
    w2a = np.ascontiguousarray(
        np.asarray(aW2, np.float32).reshape(8, 128, OUT_D).transpose(1, 0, 2)
    ).astype(ml_dtypes.bfloat16)
    w2c = np.ascontiguousarray(
        np.asarray(cW2, np.float32).reshape(8, 128, 1).transpose(1, 0, 2)
    ).astype(ml_dtypes.bfloat16)
    b0a = np.ascontiguousarray(np.asarray(ab0, np.float32).reshape(8, 128).T)
    b0c = np.ascontiguousarray(np.asarray(cb0, np.float32).reshape(8, 128).T)
    b1a = np.ascontiguousarray(np.asarray(ab1, np.float32).reshape(8, 128).T)
    b1c = np.ascontiguousarray(np.asarray(cb1, np.float32).reshape(8, 128).T)
    b2cat = np.zeros((64, 1), np.float32)
    b2cat[0:OUT_D, 0] = np.asarray(ab2, np.float32)
    b2cat[32, 0] = np.asarray(cb2, np.float32)[0]

    shared = {
        "obx_g": ob_x, "iota32": iota32, "iota21": iota21,
        "w0a": np.asarray(aW0, np.float32), "w0c": np.asarray(cW0, np.float32),
        "w1a": w1a, "w1c": w1c, "w2a": w2a, "w2c": w2c,
        "b0a": b0a, "b0c": b0c, "b1a": b1a, "b1c": b1c, "b2cat": b2cat,
    }

    in_maps = []
    for k in range(NCORES):
        lo, hi = k * NPC, (k + 1) * NPC
        obtT = np.zeros((B + 1, NPCP), np.float32)
        obtT[:B, :NPC] = ob_t[lo:hi, :].T
        obtT[B, :] = 1.0
        obxN = np.ones((B, 128, NT), np.int32)     # pad color 1 -> mask 0
        actN = np.zeros((B, 128, NT), np.int32)
        obx_loc = ob_x[lo:hi].T                    # [B, NPC]
        act_loc = action[lo:hi].T
        padded_x = np.ones((B, NPCP), np.int32)
        padded_x[:, :NPC] = obx_loc
        padded_a = np.zeros((B, NPCP), np.int32)
        padded_a[:, :NPC] = act_loc
        obxN[:] = padded_x.reshape(B, NT, 128).transpose(0, 2, 1)
        actN[:] = padded_a.reshape(B, NT, 128).transpose(0, 2, 1)

        a_all = np.zeros((NBLK, 128, 512), np.uint8)
        for pas, (g_arr, s_arr) in enumerate(((src, dst), (dst, src))):
            selm = (s_arr >= lo) & (s_arr < hi)
            gg = g_arr[selm]
            ss = s_arr[selm] - lo
            dense = np.zeros((NP, NPCP), np.uint8)
            np.add.at(dense, (gg, ss), 1)
            blocks = dense.reshape(KT, 128, NRANGE, 512).transpose(2, 0, 1, 3)
            a_all[pas * NRANGE * KT:(pas + 1) * NRANGE * KT] = blocks.reshape(
                NRANGE * KT, 128, 512)
            del dense
        a_fp8 = _FP8_LUT[a_all]
        del a_all

        m = dict(shared)
        m.update({"obtT": obtT, "obxN": obxN, "actN": actN, "a_all": a_fp8})
        in_maps.append(m)
    return in_maps


def kernel(**inputs):
    global LAST_EXEC_NS
    if "nc" not in _CACHED:
        _CACHED["nc"] = _build_program()
    nc = _CACHED["nc"]
    in_maps = _prep_inputs(**inputs)
    trace = bool(os.environ.get("KBENCH_TRACE"))
    res = run_bass_kernel_spmd(
        nc, in_maps, core_ids=list(range(NCORES)), trace=trace)
    LAST_EXEC_NS = res.exec_time_ns

    alp = np.zeros((N, B), np.float32)
    vsum = np.zeros(B, np.float64)
    esum = 0.0
    msum = 0.0
    for k in range(NCORES):
        out = res.results[k]
        a = out["alp_out"]                     # [B, 128, NT]
        loc = a.transpose(2, 1, 0).reshape(NPCP, B)[:NPC]
        alp[k * NPC:(k + 1) * NPC] = loc
        sc = out["scal_out"][0]
        vsum += sc[0:8].astype(np.float64)
        esum += float(sc[8])
        msum += float(sc[9])
    avg_entropy = np.float32(esum / max(msum, 1.0))
    value_preds = (vsum / MAXN).astype(np.float32)
    return alp, avg_entropy, value_preds


# revision 23
# speedup vs baseline: 1.0955x; 1.0031x over previous
"""Trainium2 Bass kernel for nn_ActorCritic (GNN message passing + actor/critic MLPs).

Sharding: nodes are partitioned across the 8 NeuronCores (2500 nodes each, all
8 batch elements on every core). The one-hop segment_sums (nb, nb_rev) are
computed as dense fp8 matmuls against host-built adjacency blocks:
    nbT[pay, dst] = sum_src X[src, pay] * A[src, dst]
with X = one-hot colors built on device (payload = batch*32 + color, 256 wide,
split into hi/lo 128-partition planes) and A the 0/1 adjacency (pure index
data). The MLPs run with float32r (layer 0) and bfloat16 (layers 1/2) matmuls;
log-softmax / entropy / value reductions run on DVE/ACT after a PE transpose
back to node-major layout. Outputs are gathered and re-assembled on host.
"""
import os
import sys

sys.path.insert(0, "/opt/trn_rl_repo")
sys.path.insert(0, os.path.dirname(os.path.abspath(__file__)))

import numpy as np
import ml_dtypes

import concourse.bass as bass
import concourse.mybir as mybir
import concourse.tile as tile_mod
from concourse.tile import TileContext
from concourse.bass_utils import run_bass_kernel_spmd
from concourse.masks import make_identity
from concourse.vector_clock import ScopedClock


# ---------------------------------------------------------------------------
# Walrus-compat patches: this build rejects >1 sem wait per instruction, and
# the stock TileContext tail drain carries one wait per live logical proc.
# ---------------------------------------------------------------------------

MAX_WAITS = 1


def _patched_drain_and_barrier(self, tick_clock, wait_clock):
    nc = self.nc
    probe = nc.sync.nop()
    wait_clock.add_sem_waits(probe.ins, ScopedClock({None: tick_clock.global_clock}))
    si = probe.ins.sync_info
    waits = list(si.on_wait) if si is not None else []
    if len(waits) > MAX_WAITS:
        si.on_wait = waits[:MAX_WAITS]
        rest = waits[MAX_WAITS:]
        for j in range(0, len(rest), MAX_WAITS):
            n = nc.sync.nop()
            nsi = n.ins.sync_info
            if nsi is None:
                n.ins.sync_info = mybir.SyncInfo(
                    on_update=[], on_wait=rest[j : j + MAX_WAITS]
                )
            else:
                nsi.on_wait = rest[j : j + MAX_WAITS]
    nc.sync.drain()
    nc.all_engine_barrier()
    assert self.sems is not None
    popped = nc._tile_sem_poison_stack.pop()
    assert popped is self._sem_poison
    nc.clear_and_free_semaphores(list(self.sems.allocated().values()))
    nc.all_engine_barrier()


_ws_counter = [0]


def fix_waits(nc, max_waits: int = 1):
    """Post-pass over the finished module: any instruction carrying more than
    ``max_waits`` sem waits gets the excess hoisted onto same-engine NoOps
    inserted immediately before it (this walrus build rejects multi-wait
    instructions at codegen)."""
    for f in nc.m.functions:
        for blk in f.blocks:
            insns = blk.instructions
            out = []
            changed = False
            for ins in insns:
                si = ins.sync_info
                if si is not None and len(si.on_wait) > max_waits:
                    waits = list(si.on_wait)
                    keep = waits[: max_waits]
                    rest = waits[max_waits:]
                    for j in range(0, len(rest), max_waits):
                        _ws_counter[0] += 1
                        nop = mybir.InstNoOp(
                            name=f"WSPLIT-{_ws_counter[0]}",
                            ins=[],
                            outs=[],
                            engine=ins.engine,
                            sync_info=mybir.SyncInfo(
                                on_update=[], on_wait=rest[j : j + max_waits]
                            ),
                        )
                        out.append(nop)
                    si.on_wait = keep
                    changed = True
                out.append(ins)
            if changed:
                blk.instructions = out


def install():
    tile_mod.TileContext._drain_and_barrier = _patched_drain_and_barrier


class _TP:
    fix_waits = staticmethod(fix_waits)


tile_patch = _TP()
install()

# ---- problem constants (hardcoded per spec) ----
N, B, NCOL, H, E = 20000, 8, 20, 1024, 320000
IN_D, OUT_D, MAXN = 42, 21, 20000
NCORES = 8
NPC = N // NCORES            # 2500 nodes per core
NPCP = 2560                  # padded: 20 x 128 = 5 x 512
NT = NPCP // 128             # 20 node tiles per core
NRANGE = NPCP // 512         # 5 psum ranges
KT = 157                     # src k-chunks (ceil(20000/128))
NP = KT * 128                # 20096 padded source nodes
NBLK = 2 * NRANGE * KT       # 1570 adjacency blocks per core

f32 = mybir.dt.float32
f32r = mybir.dt.float32r
bf16 = mybir.dt.bfloat16
fp8 = mybir.dt.float8e4
i32 = mybir.dt.int32

_FP8_LUT = np.arange(256, dtype=np.uint8).astype(np.float32).astype(
    ml_dtypes.float8_e4m3)

LAST_EXEC_NS = None
_CACHED = {}


def _build_program():
    nc = bass.Bass("TRN2")
    p = {}
    p["obx_g"] = nc.declare_dram_parameter("obx_g", [N, B], i32, isOutput=False)
    p["obtT"] = nc.declare_dram_parameter("obtT", [B + 1, NPCP], f32, isOutput=False)
    p["obxN"] = nc.declare_dram_parameter("obxN", [B, 128, NT], i32, isOutput=False)
    p["actN"] = nc.declare_dram_parameter("actN", [B, 128, NT], i32, isOutput=False)
    p["a_all"] = nc.declare_dram_parameter("a_all", [NBLK, 128, 512], fp8, isOutput=False)
    p["iota32"] = nc.declare_dram_parameter("iota32", [128, 32], i32, isOutput=False)
    p["iota21"] = nc.declare_dram_parameter("iota21", [128, 21], f32, isOutput=False)
    p["w0a"] = nc.declare_dram_parameter("w0a", [IN_D, H], f32, isOutput=False)
    p["w0c"] = nc.declare_dram_parameter("w0c", [IN_D, H], f32, isOutput=False)
    p["w1a"] = nc.declare_dram_parameter("w1a", [128, 64, 128], bf16, isOutput=False)
    p["w1c"] = nc.declare_dram_parameter("w1c", [128, 64, 128], bf16, isOutput=False)
    p["w2a"] = nc.declare_dram_parameter("w2a", [128, 8, OUT_D], bf16, isOutput=False)
    p["w2c"] = nc.declare_dram_parameter("w2c", [128, 8, 1], bf16, isOutput=False)
    p["b0a"] = nc.declare_dram_parameter("b0a", [128, 8], f32, isOutput=False)
    p["b0c"] = nc.declare_dram_parameter("b0c", [128, 8], f32, isOutput=False)
    p["b1a"] = nc.declare_dram_parameter("b1a", [128, 8], f32, isOutput=False)
    p["b1c"] = nc.declare_dram_parameter("b1c", [128, 8], f32, isOutput=False)
    p["b2cat"] = nc.declare_dram_parameter("b2cat", [64, 1], f32, isOutput=False)
    alp_out = nc.declare_dram_parameter("alp_out", [B, 128, NT], f32, isOutput=True)
    scal_out = nc.declare_dram_parameter("scal_out", [1, 10], f32, isOutput=True)

    with TileContext(nc) as tc:
        with (
            tc.tile_pool(name="per", bufs=1) as per,       # persistent
            tc.tile_pool(name="nbp", bufs=1) as nbp,       # nb storage
        ):
            # ---- persistent small tiles ----
            iota32_t = per.tile([128, 32], i32)
            nc.sync.dma_start(out=iota32_t[:], in_=p["iota32"][:, :])
            iota21_t = per.tile([128, 21], f32)
            nc.sync.dma_start(out=iota21_t[:], in_=p["iota21"][:, :])
            ident_f32 = per.tile([128, 128], f32)
            make_identity(nc, ident_f32[:])
            ones_col = per.tile([128, 1], f32)
            nc.vector.memset(ones_col[:], 1.0)
            acc_all = per.tile([128, 10], f32)
            nc.vector.memset(acc_all[:], 0.0)
            b2cat_t = per.tile([64, 1], f32)
            nc.sync.dma_start(out=b2cat_t[:], in_=p["b2cat"][:, :])

            # weights
            w0a_t = per.tile([IN_D, H], f32r)
            nc.gpsimd.dma_start(out=w0a_t[:], in_=p["w0a"][:, :])
            w0c_t = per.tile([IN_D, H], f32r)
            nc.gpsimd.dma_start(out=w0c_t[:], in_=p["w0c"][:, :])
            w1a_t = per.tile([128, 64, 128], bf16)
            nc.sync.dma_start(out=w1a_t[:], in_=p["w1a"][:, :, :])
            w1c_t = per.tile([128, 64, 128], bf16)
            nc.sync.dma_start(out=w1c_t[:], in_=p["w1c"][:, :, :])
            w2a_t = per.tile([128, 8, OUT_D], bf16)
            nc.sync.dma_start(out=w2a_t[:], in_=p["w2a"][:, :, :])
            w2c_t = per.tile([128, 8, 1], bf16)
            nc.sync.dma_start(out=w2c_t[:], in_=p["w2c"][:, :, :])
            biases = {}
            for nm in ("b0a", "b0c", "b1a", "b1c"):
                t = per.tile([128, 8], f32, name=nm)
                nc.sync.dma_start(out=t[:], in_=p[nm][:, :])
                biases[nm] = t

            # nb storage: [payload(4b x 32), node] f32
            nb_hi = nbp.tile([128, NPCP], f32)
            nb_lo = nbp.tile([128, NPCP], f32)
            nbr_hi = nbp.tile([128, NPCP], f32)
            nbr_lo = nbp.tile([128, NPCP], f32)
            nbt = {(0, 0): nb_hi, (0, 1): nb_lo, (1, 0): nbr_hi, (1, 1): nbr_lo}

            # ---- phase 0: build one-hot X in SBUF ----
            with tc.tile_pool(name="xp", bufs=1) as xp:
                KS = 80
                x_lo = xp.tile([128, KS, 256], fp8)
                x_hi = xp.tile([128, KT - KS, 256], fp8)
                nc.vector.memset(x_lo[:, :, :], 0.0)
                nc.vector.memset(x_hi[:, :, :], 0.0)

                def x_sb_at(k):
                    return (x_lo, k) if k < KS else (x_hi, k - KS)
                obx_sb = xp.tile([128, KT, 8], i32)
                nc.sync.dma_start(
                    out=obx_sb[:, : KT - 1, :],
                    in_=p["obx_g"][0 : (KT - 1) * 128, :].rearrange(
                        "(t q) b -> q t b", q=128
                    ),
                )
                # last partial tile: rows 19968..19999 (32 rows)
                nc.sync.dma_start(
                    out=obx_sb[:32, KT - 1, :],
                    in_=p["obx_g"][(KT - 1) * 128 :, :],
                )
                for t in range(KT):
                    hi = 128 if t < KT - 1 else 32
                    xt, tt = x_sb_at(t)
                    nc.vector.tensor_tensor(
                        out=xt[:hi, tt, :],
                        in0=obx_sb[:hi, t, :].rearrange(
                            "p (b one) -> p b one", one=1
                        ).to_broadcast([hi, 8, 32]),
                        in1=iota32_t[:hi, :].rearrange(
                            "p (one j) -> p one j", one=1
                        ).to_broadcast([hi, 8, 32]),
                        op=mybir.AluOpType.is_equal,
                    )

                # ---- phase 1: dense scatter matmuls ----
                with (
                    tc.tile_pool(name="ab", bufs=8) as ab,
                    tc.tile_pool(name="scps", bufs=3, space="PSUM") as scps,
                ):
                    for pas in range(2):
                        for r in range(NRANGE):
                            ps_hi = scps.tile([128, 512], f32, space="PSUM", tag="hi")
                            ps_lo = scps.tile([128, 512], f32, space="PSUM", tag="lo")
                            for k0 in range(0, KT, 16):
                                kk = min(16, KT - k0)
                                blk = (pas * NRANGE + r) * KT + k0
                                a_t = ab.tile([128, 16, 512], fp8, tag="a")
                                nc.sync.dma_start(
                                    out=a_t[:, :kk, :],
                                    in_=p["a_all"][blk : blk + kk, :, :].rearrange(
                                        "k q n -> q k n"),
                                )
                                for j0 in range(0, kk, 2):
                                    k0j = k0 + j0
                                    jj = min(2, kk - j0)
                                    if jj == 2:
                                        nc.tensor.matmul(
                                            ps_hi[:],
                                            lhsT=x_sb_at(k0j)[0][:, x_sb_at(k0j)[1] : x_sb_at(k0j)[1] + 2, 0:128],
                                            rhs=a_t[:, j0 : j0 + 2, :],
                                            start=(k0j == 0),
                                            stop=(k0j + 2 == KT),
                                            perf_mode=mybir.MatmulPerfMode.DoubleRow,
                                        )
                                        nc.tensor.matmul(
                                            ps_lo[:],
                                            lhsT=x_sb_at(k0j)[0][:, x_sb_at(k0j)[1] : x_sb_at(k0j)[1] + 2, 128:256],
                                            rhs=a_t[:, j0 : j0 + 2, :],
                                            start=(k0j == 0),
                                            stop=(k0j + 2 == KT),
                                            perf_mode=mybir.MatmulPerfMode.DoubleRow,
                                        )
                                    else:
                                        nc.tensor.matmul(
                                            ps_hi[:],
                                            lhsT=x_sb_at(k0j)[0][:, x_sb_at(k0j)[1], 0:128],
                                            rhs=a_t[:, j0, :],
                                            start=(k0j == 0),
                                            stop=(k0j == KT - 1),
                                        )
                                        nc.tensor.matmul(
                                            ps_lo[:],
                                            lhsT=x_sb_at(k0j)[0][:, x_sb_at(k0j)[1], 128:256],
                                            rhs=a_t[:, j0, :],
                                            start=(k0j == 0),
                                            stop=(k0j == KT - 1),
                                        )
                            nc.vector.tensor_copy(
                                out=nbt[(pas, 0)][:, r * 512 : (r + 1) * 512],
                                in_=ps_hi[:],
                            )
                            nc.vector.tensor_copy(
                                out=nbt[(pas, 1)][:, r * 512 : (r + 1) * 512],
                                in_=ps_lo[:],
                            )

            # ---- phase 2: MLPs + post ----
            with (
                tc.tile_pool(name="mlp", bufs=1) as mlp,
                tc.tile_pool(name="post", bufs=2) as post,
                tc.tile_pool(name="l0ps", bufs=2, space="PSUM") as l0ps,
                tc.tile_pool(name="l1ps", bufs=3, space="PSUM") as l1ps,
                tc.tile_pool(name="l2ps", bufs=1, space="PSUM") as l2ps,
                tc.tile_pool(name="trps", bufs=2, space="PSUM") as trps,
            ):
                alp_sb = mlp.tile([128, B * NT], f32)
                for b in range(B):
                    bq = (b % 4) * 32
                    plane = b // 4
                    hT = mlp.tile([IN_D, NPCP], f32r, tag="hT", bufs=2)
                    nc.gpsimd.dma_start(out=hT[0:1, :], in_=p["obtT"][b : b + 1, :])
                    nc.gpsimd.dma_start(
                        out=hT[1:21, :], in_=nbt[(0, plane)][bq : bq + 20, :]
                    )
                    nc.gpsimd.dma_start(
                        out=hT[21:41, :], in_=nbt[(1, plane)][bq : bq + 20, :]
                    )
                    nc.gpsimd.dma_start(out=hT[41:42, :], in_=p["obtT"][B : B + 1, :])

                    maskf = post.tile([128, NT], f32, tag="maskf")
                    obxn = post.tile([128, NT], i32, tag="obxn")
                    nc.sync.dma_start(out=obxn[:], in_=p["obxN"][b, :, :])
                    nc.vector.tensor_scalar(
                        out=maskf[:], in0=obxn[:], scalar1=0, scalar2=None,
                        op0=mybir.AluOpType.is_equal,
                    )
                    actf = post.tile([128, NT], f32, tag="actf")
                    actn = post.tile([128, NT], i32, tag="actn")
                    nc.sync.dma_start(out=actn[:], in_=p["actN"][b, :, :])
                    nc.vector.tensor_copy(out=actf[:], in_=actn[:])
                    msum = post.tile([128, 1], f32, tag="msum")
                    nc.vector.tensor_reduce(
                        out=msum[:], in_=maskf[:], axis=mybir.AxisListType.X,
                        op=mybir.AluOpType.add,
                    )
                    nc.vector.tensor_tensor(
                        out=acc_all[:, 9:10], in0=acc_all[:, 9:10], in1=msum[:],
                        op=mybir.AluOpType.add,
                    )

                    for rt in range(NRANGE):
                        cs = rt * 512
                        h0a = mlp.tile([128, 8, 512], bf16, tag="h0a", bufs=2)
                        h0c = mlp.tile([128, 8, 512], bf16, tag="h0c", bufs=2)
                        for net, w0t, h0t, b0 in (
                            ("a", w0a_t, h0a, biases["b0a"]),
                            ("c", w0c_t, h0c, biases["b0c"]),
                        ):
                            for m in range(8):
                                ps = l0ps.tile([128, 512], f32, space="PSUM", tag="l0")
                                nc.tensor.matmul(
                                    ps[:],
                                    lhsT=w0t[:, m * 128 : (m + 1) * 128],
                                    rhs=hT[:, cs : cs + 512],
                                    start=True, stop=True,
                                )
                                nc.scalar.activation(
                                    out=h0t[:, m, :], in_=ps[:],
                                    func=mybir.ActivationFunctionType.Relu,
                                    bias=b0[:, m : m + 1],
                                )
                        h1a = mlp.tile([128, 8, 512], bf16, tag="h1a", bufs=2)
                        h1c = mlp.tile([128, 8, 512], bf16, tag="h1c", bufs=2)
                        for m in range(8):
                            ps = l1ps.tile([128, 512], f32, space="PSUM", tag="l1")
                            for k in range(8):
                                nc.tensor.matmul(
                                    ps[:],
                                    lhsT=w1a_t[:, k * 8 + m, :],
                                    rhs=h0a[:, k, :],
                                    start=(k == 0), stop=(k == 7),
                                )
                            nc.scalar.activation(
                                out=h1a[:, m, :], in_=ps[:],
                                func=mybir.ActivationFunctionType.Relu,
                                bias=biases["b1a"][:, m : m + 1],
                            )
                        for m in range(8):
                            ps = l1ps.tile([128, 512], f32, space="PSUM", tag="l1")
                            for k in range(8):
                                nc.tensor.matmul(
                                    ps[:],
                                    lhsT=w1c_t[:, k * 8 + m, :],
                                    rhs=h0c[:, k, :],
                                    start=(k == 0), stop=(k == 7),
                                )
                            nc.scalar.activation(
                                out=h1c[:, m, :], in_=ps[:],
                                func=mybir.ActivationFunctionType.Relu,
                                bias=biases["b1c"][:, m : m + 1],
                            )
                        ps2 = l2ps.tile([64, 512], f32, space="PSUM", tag="l2")
                        for k in range(8):
                            nc.tensor.matmul(
                                ps2[0:OUT_D, :], lhsT=w2a_t[:, k, :], rhs=h1a[:, k, :],
                                start=(k == 0), stop=(k == 7),
                            )
                        for k in range(8):
                            nc.tensor.matmul(
                                ps2[32:33, :], lhsT=w2c_t[:, k, :], rhs=h1c[:, k, :],
                                start=(k == 0), stop=(k == 7),
                            )
                        catT = post.tile([64, 512], f32, tag="catT")
                        nc.vector.tensor_scalar(
                            out=catT[0:33, :], in0=ps2[0:33, :],
                            scalar1=b2cat_t[0:33, :], scalar2=None,
                            op0=mybir.AluOpType.add,
                        )
                        for ntile in range(4):
                            tg = rt * 4 + ntile
                            trp = trps.tile([128, 64], f32, space="PSUM", tag="tr")
                            nc.tensor.transpose(
                                out=trp[:],
                                in_=catT[:, ntile * 128 : (ntile + 1) * 128],
                                identity=ident_f32[0:64, 0:64],
                            )
                            ln = post.tile([128, 33], f32, tag="ln")
                            nc.vector.tensor_copy(out=ln[:], in_=trp[:, 0:33])
                            lg = ln[:, 0:OUT_D]
                            mx = post.tile([128, 1], f32, tag="mx")
                            nc.vector.tensor_reduce(
                                out=mx[:], in_=lg, axis=mybir.AxisListType.X,
                                op=mybir.AluOpType.max,
                            )
                            nmx = post.tile([128, 1], f32, tag="nmx")
                            nc.vector.tensor_scalar_mul(nmx[:], mx[:], -1.0)
                            ex = post.tile([128, OUT_D], f32, tag="ex")
                            s = post.tile([128, 1], f32, tag="s")
                            nc.scalar.activation(
                                out=ex[:], in_=lg,
                                func=mybir.ActivationFunctionType.Exp,
                                bias=nmx[:], accum_out=s[:],
                            )
                            logs = post.tile([128, 1], f32, tag="logs")
                            nc.scalar.activation(
                                out=logs[:], in_=s[:],
                                func=mybir.ActivationFunctionType.Ln,
                            )
                            sel = post.tile([128, OUT_D], f32, tag="sel")
                            nc.vector.tensor_scalar(
                                out=sel[:], in0=iota21_t[:],
                                scalar1=actf[:, tg : tg + 1], scalar2=None,
                                op0=mybir.AluOpType.is_equal,
                            )
                            junk = post.tile([128, OUT_D], f32, tag="junk")
                            asel = post.tile([128, 1], f32, tag="asel")
                            nc.vector.tensor_tensor(
                                out=junk[:], in0=sel[:], in1=lg,
                                op=mybir.AluOpType.mult,
                            )
                            nc.vector.tensor_reduce(
                                out=asel[:], in_=junk[:],
                                axis=mybir.AxisListType.X, op=mybir.AluOpType.add,
                            )
                            junk2 = post.tile([128, OUT_D], f32, tag="junk2")
                            t3 = post.tile([128, 1], f32, tag="t3")
                            nc.vector.tensor_tensor(
                                out=junk2[:], in0=ex[:], in1=lg,
                                op=mybir.AluOpType.mult,
                            )
                            nc.vector.tensor_reduce(
                                out=t3[:], in_=junk2[:],
                                axis=mybir.AxisListType.X, op=mybir.AluOpType.add,
                            )
                            # alp = (asel - mx - logs) * mask
                            alp0 = post.tile([128, 1], f32, tag="alp0")
                            nc.vector.tensor_scalar(
                                out=alp0[:], in0=asel[:], scalar1=mx[:],
                                scalar2=None, op0=mybir.AluOpType.subtract,
                            )
                            nc.vector.tensor_tensor(
                                out=alp0[:], in0=alp0[:], in1=logs[:],
                                op=mybir.AluOpType.subtract,
                            )
                            nc.vector.tensor_tensor(
                                out=alp_sb[:, b * NT + tg : b * NT + tg + 1],
                                in0=alp0[:], in1=maskf[:, tg : tg + 1],
                                op=mybir.AluOpType.mult,
                            )
                            # ent = mx + logs - t3 / s
                            rs = post.tile([128, 1], f32, tag="rs")
                            nc.vector.reciprocal(rs[:], s[:])
                            ent0 = post.tile([128, 1], f32, tag="ent0")
                            nc.vector.tensor_tensor(
                                out=ent0[:], in0=t3[:], in1=rs[:],
                                op=mybir.AluOpType.mult,
                            )
                            nc.vector.tensor_scalar(
                                out=ent0[:], in0=ent0[:], scalar1=-1.0,
                                scalar2=mx[:], op0=mybir.AluOpType.mult,
                                op1=mybir.AluOpType.add,
                            )
                            nc.vector.tensor_tensor(
                                out=ent0[:], in0=ent0[:], in1=logs[:],
                                op=mybir.AluOpType.add,
                            )
                            nc.vector.tensor_tensor(
                                out=ent0[:], in0=ent0[:], in1=maskf[:, tg : tg + 1],
                                op=mybir.AluOpType.mult,
                            )
                            nc.vector.tensor_tensor(
                                out=acc_all[:, 8:9], in0=acc_all[:, 8:9],
                                in1=ent0[:], op=mybir.AluOpType.add,
                            )
                            # value
                            vm = post.tile([128, 1], f32, tag="vm")
                            nc.vector.tensor_tensor(
                                out=vm[:], in0=ln[:, 32:33],
                                in1=maskf[:, tg : tg + 1], op=mybir.AluOpType.mult,
                            )
                            nc.vector.tensor_tensor(
                                out=acc_all[:, b : b + 1], in0=acc_all[:, b : b + 1],
                                in1=vm[:], op=mybir.AluOpType.add,
                            )
                    nc.sync.dma_start(
                        out=alp_out[b, :, :], in_=alp_sb[:, b * NT : (b + 1) * NT]
                    )

                # final partition reduce via ones matmul (reuses a trps slot)
                if True:
                    red = trps.tile([128, 10], f32, space="PSUM", tag="tr")
                    nc.tensor.matmul(
                        red[0:1, :], lhsT=ones_col[:], rhs=acc_all[:],
                        start=True, stop=True,
                    )
                    scal_sb = per.tile([1, 10], f32)
                    nc.vector.tensor_copy(out=scal_sb[:], in_=red[0:1, :])
                    nc.sync.dma_start(out=scal_out[:, :], in_=scal_sb[:])

    tile_patch.fix_waits(nc)
    return nc


def _prep_inputs(ob_x, ob_t, action, src, dst, aW0, ab0, aW1, ab1, aW2, ab2,
                 cW0, cb0, cW1, cb1, cW2, cb2):
    ob_x = np.asarray(ob_x, np.int32)
    ob_t = np.asarray(ob_t, np.float32)
    action = np.asarray(action, np.int32)
    src = np.asarray(src, np.int64)
    dst = np.asarray(dst, np.int64)

    iota32 = np.tile(np.arange(1, 33, dtype=np.int32), (128, 1))
    iota21 = np.tile(np.arange(OUT_D, dtype=np.float32), (128, 1))
    w1a = np.ascontiguousarray(
        np.asarray(aW1, np.float32).reshape(8, 128, 8, 128)
        .transpose(1, 0, 2, 3).reshape(128, 64, 128)).astype(ml_dtypes.bfloat16)
    w1c = np.ascontiguousarray(
        np.asarray(cW1, np.float32).reshape(8, 128, 8, 128)
        .transpose(1, 0, 2, 3).reshape(128, 64, 128)).astype(ml_dtypes.bfloat16)
    w2a = np.ascontiguousarray(
        np.asarray(aW2, np.float32).reshape(8, 128, OUT_D).transpose(1, 0, 2)
    ).astype(ml_dtypes.bfloat16)
    w2c = np.ascontiguousarray(
        np.asarray(cW2, np.float32).reshape(8, 128, 1).transpose(1, 0, 2)
    ).astype(ml_dtypes.bfloat16)
    b0a = np.ascontiguousarray(np.asarray(ab0, np.float32).reshape(8, 128).T)
    b0c = np.ascontiguousarray(np.asarray(cb0, np.float32).reshape(8, 128).T)
    b1a = np.ascontiguousarray(np.asarray(ab1, np.float32).reshape(8, 128).T)
    b1c = np.ascontiguousarray(np.asarray(cb1, np.float32).reshape(8, 128).T)
    b2cat = np.zeros((64, 1), np.float32)
    b2cat[0:OUT_D, 0] = np.asarray(ab2, np.float32)
    b2cat[32, 0] = np.asarray(cb2, np.float32)[0]

    shared = {
        "obx_g": ob_x, "iota32": iota32, "iota21": iota21,
        "w0a": np.asarray(aW0, np.float32), "w0c": np.asarray(cW0, np.float32),
        "w1a": w1a, "w1c": w1c, "w2a": w2a, "w2c": w2c,
        "b0a": b0a, "b0c": b0c, "b1a": b1a, "b1c": b1c, "b2cat": b2cat,
    }

    in_maps = []
    for k in range(NCORES):
        lo, hi = k * NPC, (k + 1) * NPC
        obtT = np.zeros((B + 1, NPCP), np.float32)
        obtT[:B, :NPC] = ob_t[lo:hi, :].T
        obtT[B, :] = 1.0
        obxN = np.ones((B, 128, NT), np.int32)     # pad color 1 -> mask 0
        actN = np.zeros((B, 128, NT), np.int32)
        obx_loc = ob_x[lo:hi].T                    # [B, NPC]
        act_loc = action[lo:hi].T
        padded_x = np.ones((B, NPCP), np.int32)
        padded_x[:, :NPC] = obx_loc
        padded_a = np.zeros((B, NPCP), np.int32)
        padded_a[:, :NPC] = act_loc
        obxN[:] = padded_x.reshape(B, NT, 128).transpose(0, 2, 1)
        actN[:] = padded_a.reshape(B, NT, 128).transpose(0, 2, 1)

        a_all = np.zeros((NBLK, 128, 512), np.uint8)
        for pas, (g_arr, s_arr) in enumerate(((src, dst), (dst, src))):
            selm = (s_arr >= lo) & (s_arr < hi)
            gg = g_arr[selm]
            ss = s_arr[selm] - lo
            dense = np.zeros((NP, NPCP), np.uint8)
            np.add.at(dense, (gg, ss), 1)
            blocks = dense.reshape(KT, 128, NRANGE, 512).transpose(2, 0, 1, 3)
            a_all[pas * NRANGE * KT:(pas + 1) * NRANGE * KT] = blocks.reshape(
                NRANGE * KT, 128, 512)
            del dense
        a_fp8 = _FP8_LUT[a_all]
        del a_all

        m = dict(shared)
        m.update({"obtT": obtT, "obxN": obxN, "actN": actN, "a_all": a_fp8})
        in_maps.append(m)
    return in_maps


def kernel(**inputs):
    global LAST_EXEC_NS
    if "nc" not in _CACHED:
        _CACHED["nc"] = _build_program()
    nc = _CACHED["nc"]
    in_maps = _prep_inputs(**inputs)
    trace = bool(os.environ.get("KBENCH_TRACE"))
    res = run_bass_kernel_spmd(
        nc, in_maps, core_ids=list(range(NCORES)), trace=trace)
    LAST_EXEC_NS = res.exec_time_ns

    alp = np.zeros((N, B), np.float32)
    vsum = np.zeros(B, np.float64)
    esum = 0.0
    msum = 0.0
    for k in range(NCORES):
        out = res.results[k]
        a = out["alp_out"]                     # [B, 128, NT]
        loc = a.transpose(2, 1, 0).reshape(NPCP, B)[:NPC]
        alp[k * NPC:(k + 1) * NPC] = loc
        sc = out["scal_out"][0]
        vsum += sc[0:8].astype(np.float64)
        esum += float(sc[8])
        msum += float(sc[9])
    avg_entropy = np.float32(esum / max(msum, 1.0))
    value_preds = (vsum / MAXN).astype(np.float32)
    return alp, avg_entropy, value_preds


# revision 24
# speedup vs baseline: 1.1026x; 1.0064x over previous
"""Trainium2 Bass kernel for nn_ActorCritic (GNN message passing + actor/critic MLPs).

Sharding: nodes are partitioned across the 8 NeuronCores (2500 nodes each, all
8 batch elements on every core). The one-hop segment_sums (nb, nb_rev) are
computed as dense fp8 matmuls against host-built adjacency blocks:
    nbT[pay, dst] = sum_src X[src, pay] * A[src, dst]
with X = one-hot colors built on device (payload = batch*32 + color, 256 wide,
split into hi/lo 128-partition planes) and A the 0/1 adjacency (pure index
data). The MLPs run with float32r (layer 0) and bfloat16 (layers 1/2) matmuls;
log-softmax / entropy / value reductions run on DVE/ACT after a PE transpose
back to node-major layout. Outputs are gathered and re-assembled on host.
"""
import os
import sys

sys.path.insert(0, "/opt/trn_rl_repo")
sys.path.insert(0, os.path.dirname(os.path.abspath(__file__)))

import numpy as np
import ml_dtypes

import concourse.bass as bass
import concourse.mybir as mybir
import concourse.tile as tile_mod
from concourse.tile import TileContext
from concourse.bass_utils import run_bass_kernel_spmd
from concourse.masks import make_identity
from concourse.vector_clock import ScopedClock


# ---------------------------------------------------------------------------
# Walrus-compat patches: this build rejects >1 sem wait per instruction, and
# the stock TileContext tail drain carries one wait per live logical proc.
# ---------------------------------------------------------------------------

MAX_WAITS = 1


def _patched_drain_and_barrier(self, tick_clock, wait_clock):
    nc = self.nc
    probe = nc.sync.nop()
    wait_clock.add_sem_waits(probe.ins, ScopedClock({None: tick_clock.global_clock}))
    si = probe.ins.sync_info
    waits = list(si.on_wait) if si is not None else []
    if len(waits) > MAX_WAITS:
        si.on_wait = waits[:MAX_WAITS]
        rest = waits[MAX_WAITS:]
        for j in range(0, len(rest), MAX_WAITS):
            n = nc.sync.nop()
            nsi = n.ins.sync_info
            if nsi is None:
                n.ins.sync_info = mybir.SyncInfo(
                    on_update=[], on_wait=rest[j : j + MAX_WAITS]
                )
            else:
                nsi.on_wait = rest[j : j + MAX_WAITS]
    nc.sync.drain()
    nc.all_engine_barrier()
    assert self.sems is not None
    popped = nc._tile_sem_poison_stack.pop()
    assert popped is self._sem_poison
    nc.clear_and_free_semaphores(list(self.sems.allocated().values()))
    nc.all_engine_barrier()


_ws_counter = [0]


def fix_waits(nc, max_waits: int = 1):
    """Post-pass over the finished module: any instruction carrying more than
    ``max_waits`` sem waits gets the excess hoisted onto same-engine NoOps
    inserted immediately before it (this walrus build rejects multi-wait
    instructions at codegen)."""
    for f in nc.m.functions:
        for blk in f.blocks:
            insns = blk.instructions
            out = []
            changed = False
            for ins in insns:
                si = ins.sync_info
                if si is not None and len(si.on_wait) > max_waits:
                    waits = list(si.on_wait)
                    keep = waits[: max_waits]
                    rest = waits[max_waits:]
                    for j in range(0, len(rest), max_waits):
                        _ws_counter[0] += 1
                        nop = mybir.InstNoOp(
                            name=f"WSPLIT-{_ws_counter[0]}",
                            ins=[],
                            outs=[],
                            engine=ins.engine,
                            sync_info=mybir.SyncInfo(
                                on_update=[], on_wait=rest[j : j + max_waits]
                            ),
                        )
                        out.append(nop)
                    si.on_wait = keep
                    changed = True
                out.append(ins)
            if changed:
                blk.instructions = out


def install():
    tile_mod.TileContext._drain_and_barrier = _patched_drain_and_barrier


class _TP:
    fix_waits = staticmethod(fix_waits)


tile_patch = _TP()
install()

# ---- problem constants (hardcoded per spec) ----
N, B, NCOL, H, E = 20000, 8, 20, 1024, 320000
IN_D, OUT_D, MAXN = 42, 21, 20000
NCORES = 8
NPC = N // NCORES            # 2500 nodes per core
NPCP = 2560                  # padded: 20 x 128 = 5 x 512
NT = NPCP // 128             # 20 node tiles per core
NRANGE = NPCP // 512         # 5 psum ranges
KT = 157                     # src k-chunks (ceil(20000/128))
NP = KT * 128                # 20096 padded source nodes
NBLK = 2 * NRANGE * KT       # 1570 adjacency blocks per core

f32 = mybir.dt.float32
f32r = mybir.dt.float32r
bf16 = mybir.dt.bfloat16
fp8 = mybir.dt.float8e4
i32 = mybir.dt.int32

_FP8_LUT = np.arange(256, dtype=np.uint8).astype(np.float32).astype(
    ml_dtypes.float8_e4m3)

LAST_EXEC_NS = None
_CACHED = {}


def _build_program():
    nc = bass.Bass("TRN2")
    p = {}
    p["obx_g"] = nc.declare_dram_parameter("obx_g", [128, KT, B], i32, isOutput=False)
    p["obtT"] = nc.declare_dram_parameter("obtT", [B + 1, NPCP], f32, isOutput=False)
    p["obxN"] = nc.declare_dram_parameter("obxN", [B, 128, NT], i32, isOutput=False)
    p["actN"] = nc.declare_dram_parameter("actN", [B, 128, NT], i32, isOutput=False)
    p["a_all"] = nc.declare_dram_parameter("a_all", [NBLK, 128, 512], fp8, isOutput=False)
    p["iota32"] = nc.declare_dram_parameter("iota32", [128, 32], i32, isOutput=False)
    p["iota21"] = nc.declare_dram_parameter("iota21", [128, 21], f32, isOutput=False)
    p["w0a"] = nc.declare_dram_parameter("w0a", [IN_D, H], f32, isOutput=False)
    p["w0c"] = nc.declare_dram_parameter("w0c", [IN_D, H], f32, isOutput=False)
    p["w1a"] = nc.declare_dram_parameter("w1a", [128, 64, 128], bf16, isOutput=False)
    p["w1c"] = nc.declare_dram_parameter("w1c", [128, 64, 128], bf16, isOutput=False)
    p["w2a"] = nc.declare_dram_parameter("w2a", [128, 8, OUT_D], bf16, isOutput=False)
    p["w2c"] = nc.declare_dram_parameter("w2c", [128, 8, 1], bf16, isOutput=False)
    p["b0a"] = nc.declare_dram_parameter("b0a", [128, 8], f32, isOutput=False)
    p["b0c"] = nc.declare_dram_parameter("b0c", [128, 8], f32, isOutput=False)
    p["b1a"] = nc.declare_dram_parameter("b1a", [128, 8], f32, isOutput=False)
    p["b1c"] = nc.declare_dram_parameter("b1c", [128, 8], f32, isOutput=False)
    p["b2cat"] = nc.declare_dram_parameter("b2cat", [64, 1], f32, isOutput=False)
    alp_out = nc.declare_dram_parameter("alp_out", [B, 128, NT], f32, isOutput=True)
    scal_out = nc.declare_dram_parameter("scal_out", [1, 10], f32, isOutput=True)

    with TileContext(nc) as tc:
        with (
            tc.tile_pool(name="per", bufs=1) as per,       # persistent
            tc.tile_pool(name="nbp", bufs=1) as nbp,       # nb storage
        ):
            # ---- persistent small tiles ----
            iota32_t = per.tile([128, 32], i32)
            nc.sync.dma_start(out=iota32_t[:], in_=p["iota32"][:, :])
            iota21_t = per.tile([128, 21], f32)
            nc.sync.dma_start(out=iota21_t[:], in_=p["iota21"][:, :])
            ident_f32 = per.tile([128, 128], f32)
            make_identity(nc, ident_f32[:])
            ones_col = per.tile([128, 1], f32)
            nc.vector.memset(ones_col[:], 1.0)
            acc_all = per.tile([128, 10], f32)
            nc.vector.memset(acc_all[:], 0.0)
            b2cat_t = per.tile([64, 1], f32)
            nc.sync.dma_start(out=b2cat_t[:], in_=p["b2cat"][:, :])

            # weights
            w0a_t = per.tile([IN_D, H], f32r)
            nc.gpsimd.dma_start(out=w0a_t[:], in_=p["w0a"][:, :])
            w0c_t = per.tile([IN_D, H], f32r)
            nc.gpsimd.dma_start(out=w0c_t[:], in_=p["w0c"][:, :])
            w1a_t = per.tile([128, 64, 128], bf16)
            nc.sync.dma_start(out=w1a_t[:], in_=p["w1a"][:, :, :])
            w1c_t = per.tile([128, 64, 128], bf16)
            nc.sync.dma_start(out=w1c_t[:], in_=p["w1c"][:, :, :])
            w2a_t = per.tile([128, 8, OUT_D], bf16)
            nc.sync.dma_start(out=w2a_t[:], in_=p["w2a"][:, :, :])
            w2c_t = per.tile([128, 8, 1], bf16)
            nc.sync.dma_start(out=w2c_t[:], in_=p["w2c"][:, :, :])
            biases = {}
            for nm in ("b0a", "b0c", "b1a", "b1c"):
                t = per.tile([128, 8], f32, name=nm)
                nc.sync.dma_start(out=t[:], in_=p[nm][:, :])
                biases[nm] = t

            # nb storage: [payload(4b x 32), node] f32
            nb_hi = nbp.tile([128, NPCP], f32)
            nb_lo = nbp.tile([128, NPCP], f32)
            nbr_hi = nbp.tile([128, NPCP], f32)
            nbr_lo = nbp.tile([128, NPCP], f32)
            nbt = {(0, 0): nb_hi, (0, 1): nb_lo, (1, 0): nbr_hi, (1, 1): nbr_lo}

            # ---- phase 0: build one-hot X in SBUF ----
            with tc.tile_pool(name="xp", bufs=1) as xp:
                KS = 80
                x_lo = xp.tile([128, KS, 256], fp8)
                x_hi = xp.tile([128, KT - KS, 256], fp8)

                def x_sb_at(k):
                    return (x_lo, k) if k < KS else (x_hi, k - KS)
                obx_sb = xp.tile([128, KT, 8], i32)
                nc.sync.dma_start(
                    out=obx_sb[:, :KS, :], in_=p["obx_g"][:, :KS, :])
                nc.sync.dma_start(
                    out=obx_sb[:, KS:, :], in_=p["obx_g"][:, KS:, :])
                for t in range(KT):
                    xt, tt = x_sb_at(t)
                    nc.vector.tensor_tensor(
                        out=xt[:, tt, :],
                        in0=obx_sb[:, t, :].rearrange(
                            "p (b one) -> p b one", one=1
                        ).to_broadcast([128, 8, 32]),
                        in1=iota32_t[:, :].rearrange(
                            "p (one j) -> p one j", one=1
                        ).to_broadcast([128, 8, 32]),
                        op=mybir.AluOpType.is_equal,
                    )

                # ---- phase 1: dense scatter matmuls ----
                with (
                    tc.tile_pool(name="ab", bufs=8) as ab,
                    tc.tile_pool(name="scps", bufs=3, space="PSUM") as scps,
                ):
                    for pas in range(2):
                        for r in range(NRANGE):
                            ps_hi = scps.tile([128, 512], f32, space="PSUM", tag="hi")
                            ps_lo = scps.tile([128, 512], f32, space="PSUM", tag="lo")
                            for k0 in range(0, KT, 16):
                                kk = min(16, KT - k0)
                                blk = (pas * NRANGE + r) * KT + k0
                                a_t = ab.tile([128, 16, 512], fp8, tag="a")
                                nc.sync.dma_start(
                                    out=a_t[:, :kk, :],
                                    in_=p["a_all"][blk : blk + kk, :, :].rearrange(
                                        "k q n -> q k n"),
                                )
                                for j0 in range(0, kk, 2):
                                    k0j = k0 + j0
                                    jj = min(2, kk - j0)
                                    if jj == 2:
                                        nc.tensor.matmul(
                                            ps_hi[:],
                                            lhsT=x_sb_at(k0j)[0][:, x_sb_at(k0j)[1] : x_sb_at(k0j)[1] + 2, 0:128],
                                            rhs=a_t[:, j0 : j0 + 2, :],
                                            start=(k0j == 0),
                                            stop=(k0j + 2 == KT),
                                            perf_mode=mybir.MatmulPerfMode.DoubleRow,
                                        )
                                        nc.tensor.matmul(
                                            ps_lo[:],
                                            lhsT=x_sb_at(k0j)[0][:, x_sb_at(k0j)[1] : x_sb_at(k0j)[1] + 2, 128:256],
                                            rhs=a_t[:, j0 : j0 + 2, :],
                                            start=(k0j == 0),
                                            stop=(k0j + 2 == KT),
                                            perf_mode=mybir.MatmulPerfMode.DoubleRow,
                                        )
                                    else:
                                        nc.tensor.matmul(
                                            ps_hi[:],
                                            lhsT=x_sb_at(k0j)[0][:, x_sb_at(k0j)[1], 0:128],
                                            rhs=a_t[:, j0, :],
                                            start=(k0j == 0),
                                            stop=(k0j == KT - 1),
                                        )
                                        nc.tensor.matmul(
                                            ps_lo[:],
                                            lhsT=x_sb_at(k0j)[0][:, x_sb_at(k0j)[1], 128:256],
                                            rhs=a_t[:, j0, :],
                                            start=(k0j == 0),
                                            stop=(k0j == KT - 1),
                                        )
                            nc.vector.tensor_copy(
                                out=nbt[(pas, 0)][:, r * 512 : (r + 1) * 512],
                                in_=ps_hi[:],
                            )
                            nc.vector.tensor_copy(
                                out=nbt[(pas, 1)][:, r * 512 : (r + 1) * 512],
                                in_=ps_lo[:],
                            )

            # ---- phase 2: MLPs + post ----
            with (
                tc.tile_pool(name="mlp", bufs=1) as mlp,
                tc.tile_pool(name="post", bufs=2) as post,
                tc.tile_pool(name="l0ps", bufs=2, space="PSUM") as l0ps,
                tc.tile_pool(name="l1ps", bufs=3, space="PSUM") as l1ps,
                tc.tile_pool(name="l2ps", bufs=1, space="PSUM") as l2ps,
                tc.tile_pool(name="trps", bufs=2, space="PSUM") as trps,
            ):
                alp_sb = mlp.tile([128, B * NT], f32)
                for b in range(B):
                    bq = (b % 4) * 32
                    plane = b // 4
                    hT = mlp.tile([IN_D, NPCP], f32r, tag="hT", bufs=2)
                    nc.gpsimd.dma_start(out=hT[0:1, :], in_=p["obtT"][b : b + 1, :])
                    nc.gpsimd.dma_start(
                        out=hT[1:21, :], in_=nbt[(0, plane)][bq : bq + 20, :]
                    )
                    nc.gpsimd.dma_start(
                        out=hT[21:41, :], in_=nbt[(1, plane)][bq : bq + 20, :]
                    )
                    nc.gpsimd.dma_start(out=hT[41:42, :], in_=p["obtT"][B : B + 1, :])

                    maskf = post.tile([128, NT], f32, tag="maskf")
                    obxn = post.tile([128, NT], i32, tag="obxn")
                    nc.sync.dma_start(out=obxn[:], in_=p["obxN"][b, :, :])
                    nc.vector.tensor_scalar(
                        out=maskf[:], in0=obxn[:], scalar1=0, scalar2=None,
                        op0=mybir.AluOpType.is_equal,
                    )
                    actf = post.tile([128, NT], f32, tag="actf")
                    actn = post.tile([128, NT], i32, tag="actn")
                    nc.sync.dma_start(out=actn[:], in_=p["actN"][b, :, :])
                    nc.vector.tensor_copy(out=actf[:], in_=actn[:])
                    msum = post.tile([128, 1], f32, tag="msum")
                    nc.vector.tensor_reduce(
                        out=msum[:], in_=maskf[:], axis=mybir.AxisListType.X,
                        op=mybir.AluOpType.add,
                    )
                    nc.vector.tensor_tensor(
                        out=acc_all[:, 9:10], in0=acc_all[:, 9:10], in1=msum[:],
                        op=mybir.AluOpType.add,
                    )

                    for rt in range(NRANGE):
                        cs = rt * 512
                        h0a = mlp.tile([128, 8, 512], bf16, tag="h0a", bufs=2)
                        h0c = mlp.tile([128, 8, 512], bf16, tag="h0c", bufs=2)
                        for net, w0t, h0t, b0 in (
                            ("a", w0a_t, h0a, biases["b0a"]),
                            ("c", w0c_t, h0c, biases["b0c"]),
                        ):
                            for m in range(8):
                                ps = l0ps.tile([128, 512], f32, space="PSUM", tag="l0")
                                nc.tensor.matmul(
                                    ps[:],
                                    lhsT=w0t[:, m * 128 : (m + 1) * 128],
                                    rhs=hT[:, cs : cs + 512],
                                    start=True, stop=True,
                                )
                                nc.scalar.activation(
                                    out=h0t[:, m, :], in_=ps[:],
                                    func=mybir.ActivationFunctionType.Relu,
                                    bias=b0[:, m : m + 1],
                                )
                        h1a = mlp.tile([128, 8, 512], bf16, tag="h1a", bufs=2)
                        h1c = mlp.tile([128, 8, 512], bf16, tag="h1c", bufs=2)
                        for m in range(8):
                            ps = l1ps.tile([128, 512], f32, space="PSUM", tag="l1")
                            for k in range(8):
                                nc.tensor.matmul(
                                    ps[:],
                                    lhsT=w1a_t[:, k * 8 + m, :],
                                    rhs=h0a[:, k, :],
                                    start=(k == 0), stop=(k == 7),
                                )
                            nc.scalar.activation(
                                out=h1a[:, m, :], in_=ps[:],
                                func=mybir.ActivationFunctionType.Relu,
                                bias=biases["b1a"][:, m : m + 1],
                            )
                        for m in range(8):
                            ps = l1ps.tile([128, 512], f32, space="PSUM", tag="l1")
                            for k in range(8):
                                nc.tensor.matmul(
                                    ps[:],
                                    lhsT=w1c_t[:, k * 8 + m, :],
                                    rhs=h0c[:, k, :],
                                    start=(k == 0), stop=(k == 7),
                                )
                            nc.scalar.activation(
                                out=h1c[:, m, :], in_=ps[:],
                                func=mybir.ActivationFunctionType.Relu,
                                bias=biases["b1c"][:, m : m + 1],
                            )
                        ps2 = l2ps.tile([64, 512], f32, space="PSUM", tag="l2")
                        for k in range(8):
                            nc.tensor.matmul(
                                ps2[0:OUT_D, :], lhsT=w2a_t[:, k, :], rhs=h1a[:, k, :],
                                start=(k == 0), stop=(k == 7),
                            )
                        for k in range(8):
                            nc.tensor.matmul(
                                ps2[32:33, :], lhsT=w2c_t[:, k, :], rhs=h1c[:, k, :],
                                start=(k == 0), stop=(k == 7),
                            )
                        catT = post.tile([64, 512], f32, tag="catT")
                        nc.vector.tensor_scalar(
                            out=catT[0:33, :], in0=ps2[0:33, :],
                            scalar1=b2cat_t[0:33, :], scalar2=None,
                            op0=mybir.AluOpType.add,
                        )
                        for ntile in range(4):
                            tg = rt * 4 + ntile
                            trp = trps.tile([128, 64], f32, space="PSUM", tag="tr")
                            nc.tensor.transpose(
                                out=trp[:],
                                in_=catT[:, ntile * 128 : (ntile + 1) * 128],
                                identity=ident_f32[0:64, 0:64],
                            )
                            ln = post.tile([128, 33], f32, tag="ln")
                            nc.vector.tensor_copy(out=ln[:], in_=trp[:, 0:33])
                            lg = ln[:, 0:OUT_D]
                            mx = post.tile([128, 1], f32, tag="mx")
                            nc.vector.tensor_reduce(
                                out=mx[:], in_=lg, axis=mybir.AxisListType.X,
                                op=mybir.AluOpType.max,
                            )
                            nmx = post.tile([128, 1], f32, tag="nmx")
                            nc.vector.tensor_scalar_mul(nmx[:], mx[:], -1.0)
                            ex = post.tile([128, OUT_D], f32, tag="ex")
                            s = post.tile([128, 1], f32, tag="s")
                            nc.scalar.activation(
                                out=ex[:], in_=lg,
                                func=mybir.ActivationFunctionType.Exp,
                                bias=nmx[:], accum_out=s[:],
                            )
                            logs = post.tile([128, 1], f32, tag="logs")
                            nc.scalar.activation(
                                out=logs[:], in_=s[:],
                                func=mybir.ActivationFunctionType.Ln,
                            )
                            sel = post.tile([128, OUT_D], f32, tag="sel")
                            nc.vector.tensor_scalar(
                                out=sel[:], in0=iota21_t[:],
                                scalar1=actf[:, tg : tg + 1], scalar2=None,
                                op0=mybir.AluOpType.is_equal,
                            )
                            junk = post.tile([128, OUT_D], f32, tag="junk")
                            asel = post.tile([128, 1], f32, tag="asel")
                            nc.vector.tensor_tensor(
                                out=junk[:], in0=sel[:], in1=lg,
                                op=mybir.AluOpType.mult,
                            )
                            nc.vector.tensor_reduce(
                                out=asel[:], in_=junk[:],
                                axis=mybir.AxisListType.X, op=mybir.AluOpType.add,
                            )
                            junk2 = post.tile([128, OUT_D], f32, tag="junk2")
                            t3 = post.tile([128, 1], f32, tag="t3")
                            nc.vector.tensor_tensor(
                                out=junk2[:], in0=ex[:], in1=lg,
                                op=mybir.AluOpType.mult,
                            )
                            nc.vector.tensor_reduce(
                                out=t3[:], in_=junk2[:],
                                axis=mybir.AxisListType.X, op=mybir.AluOpType.add,
                            )
                            # alp = (asel - mx - logs) * mask
                            alp0 = post.tile([128, 1], f32, tag="alp0")
                            nc.vector.tensor_scalar(
                                out=alp0[:], in0=asel[:], scalar1=mx[:],
                                scalar2=None, op0=mybir.AluOpType.subtract,
                            )
                            nc.vector.tensor_tensor(
                                out=alp0[:], in0=alp0[:], in1=logs[:],
                                op=mybir.AluOpType.subtract,
                            )
                            nc.vector.tensor_tensor(
                                out=alp_sb[:, b * NT + tg : b * NT + tg + 1],
                                in0=alp0[:], in1=maskf[:, tg : tg + 1],
                                op=mybir.AluOpType.mult,
                            )
                            # ent = mx + logs - t3 / s
                            rs = post.tile([128, 1], f32, tag="rs")
                            nc.vector.reciprocal(rs[:], s[:])
                            ent0 = post.tile([128, 1], f32, tag="ent0")
                            nc.vector.tensor_tensor(
                                out=ent0[:], in0=t3[:], in1=rs[:],
                                op=mybir.AluOpType.mult,
                            )
                            nc.vector.tensor_scalar(
                                out=ent0[:], in0=ent0[:], scalar1=-1.0,
                                scalar2=mx[:], op0=mybir.AluOpType.mult,
                                op1=mybir.AluOpType.add,
                            )
                            nc.vector.tensor_tensor(
                                out=ent0[:], in0=ent0[:], in1=logs[:],
                                op=mybir.AluOpType.add,
                            )
                            nc.vector.tensor_tensor(
                                out=ent0[:], in0=ent0[:], in1=maskf[:, tg : tg + 1],
                                op=mybir.AluOpType.mult,
                            )
                            nc.vector.tensor_tensor(
                                out=acc_all[:, 8:9], in0=acc_all[:, 8:9],
                                in1=ent0[:], op=mybir.AluOpType.add,
                            )
                            # value
                            vm = post.tile([128, 1], f32, tag="vm")
                            nc.vector.tensor_tensor(
                                out=vm[:], in0=ln[:, 32:33],
                                in1=maskf[:, tg : tg + 1], op=mybir.AluOpType.mult,
                            )
                            nc.vector.tensor_tensor(
                                out=acc_all[:, b : b + 1], in0=acc_all[:, b : b + 1],
                                in1=vm[:], op=mybir.AluOpType.add,
                            )
                    nc.sync.dma_start(
                        out=alp_out[b, :, :], in_=alp_sb[:, b * NT : (b + 1) * NT]
                    )

                # final partition reduce via ones matmul (reuses a trps slot)
                if True:
                    red = trps.tile([128, 10], f32, space="PSUM", tag="tr")
                    nc.tensor.matmul(
                        red[0:1, :], lhsT=ones_col[:], rhs=acc_all[:],
                        start=True, stop=True,
                    )
                    scal_sb = per.tile([1, 10], f32)
                    nc.vector.tensor_copy(out=scal_sb[:], in_=red[0:1, :])
                    nc.sync.dma_start(out=scal_out[:, :], in_=scal_sb[:])

    tile_patch.fix_waits(nc)
    return nc


def _prep_inputs(ob_x, ob_t, action, src, dst, aW0, ab0, aW1, ab1, aW2, ab2,
                 cW0, cb0, cW1, cb1, cW2, cb2):
    ob_x = np.asarray(ob_x, np.int32)
    ob_t = np.asarray(ob_t, np.float32)
    action = np.asarray(action, np.int32)
    src = np.asarray(src, np.int64)
    dst = np.asarray(dst, np.int64)

    iota32 = np.tile(np.arange(1, 33, dtype=np.int32), (128, 1))
    iota21 = np.tile(np.arange(OUT_D, dtype=np.float32), (128, 1))
    w1a = np.ascontiguousarray(
        np.asarray(aW1, np.float32).reshape(8, 128, 8, 128)
        .transpose(1, 0, 2, 3).reshape(128, 64, 128)).astype(ml_dtypes.bfloat16)
    w1c = np.ascontiguousarray(
        np.asarray(cW1, np.float32).reshape(8, 128, 8, 128)
        .transpose(1, 0, 2, 3).reshape(128, 64, 128)).astype(ml_dtypes.bfloat16)
    w2a = np.ascontiguousarray(
        np.asarray(aW2, np.float32).reshape(8, 128, OUT_D).transpose(1, 0, 2)
    ).astype(ml_dtypes.bfloat16)
    w2c = np.ascontiguousarray(
        np.asarray(cW2, np.float32).reshape(8, 128, 1).transpose(1, 0, 2)
    ).astype(ml_dtypes.bfloat16)
    b0a = np.ascontiguousarray(np.asarray(ab0, np.float32).reshape(8, 128).T)
    b0c = np.ascontiguousarray(np.asarray(cb0, np.float32).reshape(8, 128).T)
    b1a = np.ascontiguousarray(np.asarray(ab1, np.float32).reshape(8, 128).T)
    b1c = np.ascontiguousarray(np.asarray(cb1, np.float32).reshape(8, 128).T)
    b2cat = np.zeros((64, 1), np.float32)
    b2cat[0:OUT_D, 0] = np.asarray(ab2, np.float32)
    b2cat[32, 0] = np.asarray(cb2, np.float32)[0]

    obx_pad = np.zeros((NP, B), np.int32)
    obx_pad[:N] = ob_x
    obx_g = np.ascontiguousarray(
        obx_pad.reshape(KT, 128, B).transpose(1, 0, 2))
    shared = {
        "obx_g": obx_g, "iota32": iota32, "iota21": iota21,
        "w0a": np.asarray(aW0, np.float32), "w0c": np.asarray(cW0, np.float32),
        "w1a": w1a, "w1c": w1c, "w2a": w2a, "w2c": w2c,
        "b0a": b0a, "b0c": b0c, "b1a": b1a, "b1c": b1c, "b2cat": b2cat,
    }

    in_maps = []
    for k in range(NCORES):
        lo, hi = k * NPC, (k + 1) * NPC
        obtT = np.zeros((B + 1, NPCP), np.float32)
        obtT[:B, :NPC] = ob_t[lo:hi, :].T
        obtT[B, :] = 1.0
        obxN = np.ones((B, 128, NT), np.int32)     # pad color 1 -> mask 0
        actN = np.zeros((B, 128, NT), np.int32)
        obx_loc = ob_x[lo:hi].T                    # [B, NPC]
        act_loc = action[lo:hi].T
        padded_x = np.ones((B, NPCP), np.int32)
        padded_x[:, :NPC] = obx_loc
        padded_a = np.zeros((B, NPCP), np.int32)
        padded_a[:, :NPC] = act_loc
        obxN[:] = padded_x.reshape(B, NT, 128).transpose(0, 2, 1)
        actN[:] = padded_a.reshape(B, NT, 128).transpose(0, 2, 1)

        a_all = np.zeros((NBLK, 128, 512), np.uint8)
        for pas, (g_arr, s_arr) in enumerate(((src, dst), (dst, src))):
            selm = (s_arr >= lo) & (s_arr < hi)
            gg = g_arr[selm]
            ss = s_arr[selm] - lo
            dense = np.zeros((NP, NPCP), np.uint8)
            np.add.at(dense, (gg, ss), 1)
            blocks = dense.reshape(KT, 128, NRANGE, 512).transpose(2, 0, 1, 3)
            a_all[pas * NRANGE * KT:(pas + 1) * NRANGE * KT] = blocks.reshape(
                NRANGE * KT, 128, 512)
            del dense
        a_fp8 = _FP8_LUT[a_all]
        del a_all

        m = dict(shared)
        m.update({"obtT": obtT, "obxN": obxN, "actN": actN, "a_all": a_fp8})
        in_maps.append(m)
    return in_maps


def kernel(**inputs):
    global LAST_EXEC_NS
    if "nc" not in _CACHED:
        _CACHED["nc"] = _build_program()
    nc = _CACHED["nc"]
    in_maps = _prep_inputs(**inputs)
    trace = bool(os.environ.get("KBENCH_TRACE"))
    res = run_bass_kernel_spmd(
        nc, in_maps, core_ids=list(range(NCORES)), trace=trace)
    LAST_EXEC_NS = res.exec_time_ns

    alp = np.zeros((N, B), np.float32)
    vsum = np.zeros(B, np.float64)
    esum = 0.0
    msum = 0.0
    for k in range(NCORES):
        out = res.results[k]
        a = out["alp_out"]                     # [B, 128, NT]
        loc = a.transpose(2, 1, 0).reshape(NPCP, B)[:NPC]
        alp[k * NPC:(k + 1) * NPC] = loc
        sc = out["scal_out"][0]
        vsum += sc[0:8].astype(np.float64)
        esum += float(sc[8])
        msum += float(sc[9])
    avg_entropy = np.float32(esum / max(msum, 1.0))
    value_preds = (vsum / MAXN).astype(np.float32)
    return alp, avg_entropy, value_preds


# revision 27
# speedup vs baseline: 1.1179x; 1.0139x over previous
"""Trainium2 Bass kernel for nn_ActorCritic (GNN message passing + actor/critic MLPs).

Sharding: nodes are partitioned across the 8 NeuronCores (2500 nodes each, all
8 batch elements on every core). The one-hop segment_sums (nb, nb_rev) are
computed as dense fp8 matmuls against host-built adjacency blocks:
    nbT[pay, dst] = sum_src X[src, pay] * A[src, dst]
with X = one-hot colors built on device (payload = batch*32 + color, 256 wide,
split into hi/lo 128-partition planes) and A the 0/1 adjacency (pure index
data). The MLPs run with float32r (layer 0) and bfloat16 (layers 1/2) matmuls;
log-softmax / entropy / value reductions run on DVE/ACT after a PE transpose
back to node-major layout. Outputs are gathered and re-assembled on host.
"""
import os
import sys

sys.path.insert(0, "/opt/trn_rl_repo")
sys.path.insert(0, os.path.dirname(os.path.abspath(__file__)))

import numpy as np
import ml_dtypes

import concourse.bass as bass
import concourse.mybir as mybir
import concourse.tile as tile_mod
from concourse.tile import TileContext
from concourse.bass_utils import run_bass_kernel_spmd
from concourse.masks import make_identity
from concourse.vector_clock import ScopedClock


# ---------------------------------------------------------------------------
# Walrus-compat patches: this build rejects >1 sem wait per instruction, and
# the stock TileContext tail drain carries one wait per live logical proc.
# ---------------------------------------------------------------------------

MAX_WAITS = 1


def _patched_drain_and_barrier(self, tick_clock, wait_clock):
    nc = self.nc
    probe = nc.sync.nop()
    wait_clock.add_sem_waits(probe.ins, ScopedClock({None: tick_clock.global_clock}))
    si = probe.ins.sync_info
    waits = list(si.on_wait) if si is not None else []
    if len(waits) > MAX_WAITS:
        si.on_wait = waits[:MAX_WAITS]
        rest = waits[MAX_WAITS:]
        for j in range(0, len(rest), MAX_WAITS):
            n = nc.sync.nop()
            nsi = n.ins.sync_info
            if nsi is None:
                n.ins.sync_info = mybir.SyncInfo(
                    on_update=[], on_wait=rest[j : j + MAX_WAITS]
                )
            else:
                nsi.on_wait = rest[j : j + MAX_WAITS]
    nc.sync.drain()
    nc.all_engine_barrier()
    assert self.sems is not None
    popped = nc._tile_sem_poison_stack.pop()
    assert popped is self._sem_poison
    nc.clear_and_free_semaphores(list(self.sems.allocated().values()))
    nc.all_engine_barrier()


_ws_counter = [0]


def fix_waits(nc, max_waits: int = 1):
    """Post-pass over the finished module: any instruction carrying more than
    ``max_waits`` sem waits gets the excess hoisted onto same-engine NoOps
    inserted immediately before it (this walrus build rejects multi-wait
    instructions at codegen)."""
    for f in nc.m.functions:
        for blk in f.blocks:
            insns = blk.instructions
            out = []
            changed = False
            for ins in insns:
                si = ins.sync_info
                if si is not None and len(si.on_wait) > max_waits:
                    waits = list(si.on_wait)
                    keep = waits[: max_waits]
                    rest = waits[max_waits:]
                    for j in range(0, len(rest), max_waits):
                        _ws_counter[0] += 1
                        nop = mybir.InstNoOp(
                            name=f"WSPLIT-{_ws_counter[0]}",
                            ins=[],
                            outs=[],
                            engine=ins.engine,
                            sync_info=mybir.SyncInfo(
                                on_update=[], on_wait=rest[j : j + max_waits]
                            ),
                        )
                        out.append(nop)
                    si.on_wait = keep
                    changed = True
                out.append(ins)
            if changed:
                blk.instructions = out


def install():
    tile_mod.TileContext._drain_and_barrier = _patched_drain_and_barrier


class _TP:
    fix_waits = staticmethod(fix_waits)


tile_patch = _TP()
install()

# ---- problem constants (hardcoded per spec) ----
N, B, NCOL, H, E = 20000, 8, 20, 1024, 320000
IN_D, OUT_D, MAXN = 42, 21, 20000
NCORES = 8
NPC = N // NCORES            # 2500 nodes per core
NPCP = 2560                  # padded: 20 x 128 = 5 x 512
NT = NPCP // 128             # 20 node tiles per core
NRANGE = NPCP // 512         # 5 psum ranges
KT = 157                     # src k-chunks (ceil(20000/128))
NP = KT * 128                # 20096 padded source nodes
NBLK = 2 * NRANGE * KT       # 1570 adjacency blocks per core

f32 = mybir.dt.float32
f32r = mybir.dt.float32r
bf16 = mybir.dt.bfloat16
fp8 = mybir.dt.float8e4
i32 = mybir.dt.int32

_FP8_LUT = np.arange(256, dtype=np.uint8).astype(np.float32).astype(
    ml_dtypes.float8_e4m3)

LAST_EXEC_NS = None
_CACHED = {}


def _build_program():
    nc = bass.Bass("TRN2")
    p = {}
    p["obx_g"] = nc.declare_dram_parameter("obx_g", [128, KT, B], i32, isOutput=False)
    p["obtT"] = nc.declare_dram_parameter("obtT", [B + 1, NPCP], f32, isOutput=False)
    p["obxN"] = nc.declare_dram_parameter("obxN", [B, 128, NT], i32, isOutput=False)
    p["actN"] = nc.declare_dram_parameter("actN", [B, 128, NT], i32, isOutput=False)
    p["a_all"] = nc.declare_dram_parameter("a_all", [NBLK, 128, 512], fp8, isOutput=False)
    p["iota32"] = nc.declare_dram_parameter("iota32", [128, 32], i32, isOutput=False)
    p["iota21"] = nc.declare_dram_parameter("iota21", [128, 21], f32, isOutput=False)
    p["w0a"] = nc.declare_dram_parameter("w0a", [IN_D, H], f32, isOutput=False)
    p["w0c"] = nc.declare_dram_parameter("w0c", [IN_D, H], f32, isOutput=False)
    p["w1a"] = nc.declare_dram_parameter("w1a", [128, 64, 128], bf16, isOutput=False)
    p["w1c"] = nc.declare_dram_parameter("w1c", [128, 64, 128], bf16, isOutput=False)
    p["w2a"] = nc.declare_dram_parameter("w2a", [128, 8, OUT_D], bf16, isOutput=False)
    p["w2c"] = nc.declare_dram_parameter("w2c", [128, 8, 1], bf16, isOutput=False)
    p["b0a"] = nc.declare_dram_parameter("b0a", [128, 8], f32, isOutput=False)
    p["b0c"] = nc.declare_dram_parameter("b0c", [128, 8], f32, isOutput=False)
    p["b1a"] = nc.declare_dram_parameter("b1a", [128, 8], f32, isOutput=False)
    p["b1c"] = nc.declare_dram_parameter("b1c", [128, 8], f32, isOutput=False)
    p["b2cat"] = nc.declare_dram_parameter("b2cat", [64, 1], f32, isOutput=False)
    alp_out = nc.declare_dram_parameter("alp_out", [B, 128, NT], f32, isOutput=True)
    scal_out = nc.declare_dram_parameter("scal_out", [1, 10], f32, isOutput=True)

    with TileContext(nc) as tc:
        with (
            tc.tile_pool(name="per", bufs=1) as per,       # persistent
            tc.tile_pool(name="nbp", bufs=1) as nbp,       # nb storage
        ):
            # ---- persistent small tiles ----
            iota32_t = per.tile([128, 32], i32)
            nc.sync.dma_start(out=iota32_t[:], in_=p["iota32"][:, :])
            iota21_t = per.tile([128, 21], f32)
            nc.sync.dma_start(out=iota21_t[:], in_=p["iota21"][:, :])
            ident_f32 = per.tile([128, 128], f32)
            make_identity(nc, ident_f32[:])
            ones_col = per.tile([128, 1], f32)
            nc.vector.memset(ones_col[:], 1.0)
            acc_all = per.tile([128, 10], f32)
            nc.vector.memset(acc_all[:], 0.0)
            b2cat_t = per.tile([64, 1], f32)
            nc.sync.dma_start(out=b2cat_t[:], in_=p["b2cat"][:, :])

            # weights
            w0a_t = per.tile([IN_D, H], f32r)
            nc.gpsimd.dma_start(out=w0a_t[:], in_=p["w0a"][:, :])
            w0c_t = per.tile([IN_D, H], f32r)
            nc.gpsimd.dma_start(out=w0c_t[:], in_=p["w0c"][:, :])
            w1a_t = per.tile([128, 64, 128], bf16)
            nc.sync.dma_start(out=w1a_t[:], in_=p["w1a"][:, :, :])
            w1c_t = per.tile([128, 64, 128], bf16)
            nc.sync.dma_start(out=w1c_t[:], in_=p["w1c"][:, :, :])
            w2a_t = per.tile([128, 8, OUT_D], bf16)
            nc.sync.dma_start(out=w2a_t[:], in_=p["w2a"][:, :, :])
            w2c_t = per.tile([128, 8, 1], bf16)
            nc.sync.dma_start(out=w2c_t[:], in_=p["w2c"][:, :, :])
            biases = {}
            for nm in ("b0a", "b0c", "b1a", "b1c"):
                t = per.tile([128, 8], f32, name=nm)
                nc.sync.dma_start(out=t[:], in_=p[nm][:, :])
                biases[nm] = t

            # nb storage: [payload(4b x 32), node] f32
            nb_hi = nbp.tile([128, NPCP], f32)
            nb_lo = nbp.tile([128, NPCP], f32)
            nbr_hi = nbp.tile([128, NPCP], f32)
            nbr_lo = nbp.tile([128, NPCP], f32)
            nbt = {(0, 0): nb_hi, (0, 1): nb_lo, (1, 0): nbr_hi, (1, 1): nbr_lo}

            # ---- phase 0: build one-hot X in SBUF ----
            with tc.tile_pool(name="xp", bufs=1) as xp:
                KS = 80
                x_lo = xp.tile([128, KS, 256], fp8)
                x_hi = xp.tile([128, KT - KS, 256], fp8)

                def x_sb_at(k):
                    return (x_lo, k) if k < KS else (x_hi, k - KS)
                obx_sb = xp.tile([128, KT, 8], i32)
                nc.sync.dma_start(
                    out=obx_sb[:, :KS, :], in_=p["obx_g"][:, :KS, :])
                nc.sync.dma_start(
                    out=obx_sb[:, KS:, :], in_=p["obx_g"][:, KS:, :])
                for t in range(KT):
                    xt, tt = x_sb_at(t)
                    nc.vector.tensor_tensor(
                        out=xt[:, tt, :],
                        in0=obx_sb[:, t, :].rearrange(
                            "p (b one) -> p b one", one=1
                        ).to_broadcast([128, 8, 32]),
                        in1=iota32_t[:, :].rearrange(
                            "p (one j) -> p one j", one=1
                        ).to_broadcast([128, 8, 32]),
                        op=mybir.AluOpType.is_equal,
                    )

                # ---- phase 1: dense scatter matmuls ----
                with (
                    tc.tile_pool(name="ab", bufs=8) as ab,
                    tc.tile_pool(name="scps", bufs=3, space="PSUM") as scps,
                ):
                    for pas in range(2):
                        for r in range(NRANGE):
                            ps_hi = scps.tile([128, 512], f32, space="PSUM", tag="hi")
                            ps_lo = scps.tile([128, 512], f32, space="PSUM", tag="lo")
                            for k0 in range(0, KT, 16):
                                kk = min(16, KT - k0)
                                blk = (pas * NRANGE + r) * KT + k0
                                a_t = ab.tile([128, 16, 512], fp8, tag="a")
                                nc.sync.dma_start(
                                    out=a_t[:, :kk, :],
                                    in_=p["a_all"][blk : blk + kk, :, :].rearrange(
                                        "k q n -> q k n"),
                                )
                                for j0 in range(0, kk, 2):
                                    k0j = k0 + j0
                                    jj = min(2, kk - j0)
                                    if jj == 2:
                                        nc.tensor.matmul(
                                            ps_hi[:],
                                            lhsT=x_sb_at(k0j)[0][:, x_sb_at(k0j)[1] : x_sb_at(k0j)[1] + 2, 0:128],
                                            rhs=a_t[:, j0 : j0 + 2, :],
                                            start=(k0j == 0),
                                            stop=(k0j + 2 == KT),
                                            perf_mode=mybir.MatmulPerfMode.DoubleRow,
                                        )
                                        nc.tensor.matmul(
                                            ps_lo[:],
                                            lhsT=x_sb_at(k0j)[0][:, x_sb_at(k0j)[1] : x_sb_at(k0j)[1] + 2, 128:256],
                                            rhs=a_t[:, j0 : j0 + 2, :],
                                            start=(k0j == 0),
                                            stop=(k0j + 2 == KT),
                                            perf_mode=mybir.MatmulPerfMode.DoubleRow,
                                        )
                                    else:
                                        nc.tensor.matmul(
                                            ps_hi[:],
                                            lhsT=x_sb_at(k0j)[0][:, x_sb_at(k0j)[1], 0:128],
                                            rhs=a_t[:, j0, :],
                                            start=(k0j == 0),
                                            stop=(k0j == KT - 1),
                                        )
                                        nc.tensor.matmul(
                                            ps_lo[:],
                                            lhsT=x_sb_at(k0j)[0][:, x_sb_at(k0j)[1], 128:256],
                                            rhs=a_t[:, j0, :],
                                            start=(k0j == 0),
                                            stop=(k0j == KT - 1),
                                        )
                            nc.vector.tensor_copy(
                                out=nbt[(pas, 0)][:, r * 512 : (r + 1) * 512],
                                in_=ps_hi[:],
                            )
                            nc.vector.tensor_copy(
                                out=nbt[(pas, 1)][:, r * 512 : (r + 1) * 512],
                                in_=ps_lo[:],
                            )

            # ---- phase 2: MLPs + post ----
            with (
                tc.tile_pool(name="mlp", bufs=1) as mlp,
                tc.tile_pool(name="post", bufs=2) as post,
                tc.tile_pool(name="l0ps", bufs=2, space="PSUM") as l0ps,
                tc.tile_pool(name="l1ps", bufs=3, space="PSUM") as l1ps,
                tc.tile_pool(name="l2ps", bufs=1, space="PSUM") as l2ps,
                tc.tile_pool(name="trps", bufs=2, space="PSUM") as trps,
            ):
                alp_sb = mlp.tile([128, B * NT], f32)
                for b in range(B):
                    bq = (b % 4) * 32
                    plane = b // 4
                    hT = mlp.tile([IN_D, NPCP], f32r, tag="hT", bufs=2)
                    nc.gpsimd.dma_start(out=hT[0:1, :], in_=p["obtT"][b : b + 1, :])
                    nc.gpsimd.dma_start(
                        out=hT[1:21, :], in_=nbt[(0, plane)][bq : bq + 20, :]
                    )
                    nc.gpsimd.dma_start(
                        out=hT[21:41, :], in_=nbt[(1, plane)][bq : bq + 20, :]
                    )
                    nc.gpsimd.dma_start(out=hT[41:42, :], in_=p["obtT"][B : B + 1, :])

                    maskf = post.tile([128, NT], f32, tag="maskf")
                    obxn = post.tile([128, NT], i32, tag="obxn")
                    nc.sync.dma_start(out=obxn[:], in_=p["obxN"][b, :, :])
                    nc.vector.tensor_scalar(
                        out=maskf[:], in0=obxn[:], scalar1=0, scalar2=None,
                        op0=mybir.AluOpType.is_equal,
                    )
                    actf = post.tile([128, NT], f32, tag="actf")
                    actn = post.tile([128, NT], i32, tag="actn")
                    nc.sync.dma_start(out=actn[:], in_=p["actN"][b, :, :])
                    nc.vector.tensor_copy(out=actf[:], in_=actn[:])
                    msum = post.tile([128, 1], f32, tag="msum")
                    nc.vector.tensor_reduce(
                        out=msum[:], in_=maskf[:], axis=mybir.AxisListType.X,
                        op=mybir.AluOpType.add,
                    )
                    nc.vector.tensor_tensor(
                        out=acc_all[:, 9:10], in0=acc_all[:, 9:10], in1=msum[:],
                        op=mybir.AluOpType.add,
                    )

                    def do_post(rt_p, catT_p):
                        for ntile in range(4):
                            tg = rt_p * 4 + ntile
                            trp = trps.tile([128, 64], f32, space="PSUM", tag="tr")
                            nc.tensor.transpose(
                                out=trp[:],
                                in_=catT_p[:, ntile * 128 : (ntile + 1) * 128],
                                identity=ident_f32[0:64, 0:64],
                            )
                            ln = post.tile([128, 33], f32, tag="ln")
                            nc.vector.tensor_copy(out=ln[:], in_=trp[:, 0:33])
                            lg = ln[:, 0:OUT_D]
                            mx = post.tile([128, 1], f32, tag="mx")
                            nc.vector.tensor_reduce(
                                out=mx[:], in_=lg, axis=mybir.AxisListType.X,
                                op=mybir.AluOpType.max,
                            )
                            nmx = post.tile([128, 1], f32, tag="nmx")
                            nc.vector.tensor_scalar_mul(nmx[:], mx[:], -1.0)
                            ex = post.tile([128, OUT_D], f32, tag="ex")
                            s = post.tile([128, 1], f32, tag="s")
                            nc.scalar.activation(
                                out=ex[:], in_=lg,
                                func=mybir.ActivationFunctionType.Exp,
                                bias=nmx[:], accum_out=s[:],
                            )
                            logs = post.tile([128, 1], f32, tag="logs")
                            nc.scalar.activation(
                                out=logs[:], in_=s[:],
                                func=mybir.ActivationFunctionType.Ln,
                            )
                            sel = post.tile([128, OUT_D], f32, tag="sel")
                            nc.vector.tensor_scalar(
                                out=sel[:], in0=iota21_t[:],
                                scalar1=actf[:, tg : tg + 1], scalar2=None,
                                op0=mybir.AluOpType.is_equal,
                            )
                            junk = post.tile([128, OUT_D], f32, tag="junk")
                            asel = post.tile([128, 1], f32, tag="asel")
                            nc.vector.tensor_tensor(
                                out=junk[:], in0=sel[:], in1=lg,
                                op=mybir.AluOpType.mult,
                            )
                            nc.vector.tensor_reduce(
                                out=asel[:], in_=junk[:],
                                axis=mybir.AxisListType.X, op=mybir.AluOpType.add,
                            )
                            junk2 = post.tile([128, OUT_D], f32, tag="junk2")
                            t3 = post.tile([128, 1], f32, tag="t3")
                            nc.vector.tensor_tensor(
                                out=junk2[:], in0=ex[:], in1=lg,
                                op=mybir.AluOpType.mult,
                            )
                            nc.vector.tensor_reduce(
                                out=t3[:], in_=junk2[:],
                                axis=mybir.AxisListType.X, op=mybir.AluOpType.add,
                            )
                            # alp = (asel - mx - logs) * mask
                            alp0 = post.tile([128, 1], f32, tag="alp0")
                            nc.vector.tensor_scalar(
                                out=alp0[:], in0=asel[:], scalar1=mx[:],
                                scalar2=None, op0=mybir.AluOpType.subtract,
                            )
                            nc.vector.tensor_tensor(
                                out=alp0[:], in0=alp0[:], in1=logs[:],
                                op=mybir.AluOpType.subtract,
                            )
                            nc.vector.tensor_tensor(
                                out=alp_sb[:, b * NT + tg : b * NT + tg + 1],
                                in0=alp0[:], in1=maskf[:, tg : tg + 1],
                                op=mybir.AluOpType.mult,
                            )
                            # ent = mx + logs - t3 / s
                            rs = post.tile([128, 1], f32, tag="rs")
                            nc.vector.reciprocal(rs[:], s[:])
                            ent0 = post.tile([128, 1], f32, tag="ent0")
                            nc.vector.tensor_tensor(
                                out=ent0[:], in0=t3[:], in1=rs[:],
                                op=mybir.AluOpType.mult,
                            )
                            nc.vector.tensor_scalar(
                                out=ent0[:], in0=ent0[:], scalar1=-1.0,
                                scalar2=mx[:], op0=mybir.AluOpType.mult,
                                op1=mybir.AluOpType.add,
                            )
                            nc.vector.tensor_tensor(
                                out=ent0[:], in0=ent0[:], in1=logs[:],
                                op=mybir.AluOpType.add,
                            )
                            nc.vector.tensor_tensor(
                                out=ent0[:], in0=ent0[:], in1=maskf[:, tg : tg + 1],
                                op=mybir.AluOpType.mult,
                            )
                            nc.vector.tensor_tensor(
                                out=acc_all[:, 8:9], in0=acc_all[:, 8:9],
                                in1=ent0[:], op=mybir.AluOpType.add,
                            )
                            # value
                            vm = post.tile([128, 1], f32, tag="vm")
                            nc.vector.tensor_tensor(
                                out=vm[:], in0=ln[:, 32:33],
                                in1=maskf[:, tg : tg + 1], op=mybir.AluOpType.mult,
                            )
                            nc.vector.tensor_tensor(
                                out=acc_all[:, b : b + 1], in0=acc_all[:, b : b + 1],
                                in1=vm[:], op=mybir.AluOpType.add,
                            )

                    prev_post = None
                    for rt in range(NRANGE):
                        cs = rt * 512
                        h0a = mlp.tile([128, 8, 512], bf16, tag="h0a", bufs=2)
                        h0c = mlp.tile([128, 8, 512], bf16, tag="h0c", bufs=2)
                        for net, w0t, h0t, b0 in (
                            ("a", w0a_t, h0a, biases["b0a"]),
                            ("c", w0c_t, h0c, biases["b0c"]),
                        ):
                            for m in range(8):
                                ps = l0ps.tile([128, 512], f32, space="PSUM", tag="l0")
                                nc.tensor.matmul(
                                    ps[:],
                                    lhsT=w0t[:, m * 128 : (m + 1) * 128],
                                    rhs=hT[:, cs : cs + 512],
                                    start=True, stop=True,
                                )
                                nc.scalar.activation(
                                    out=h0t[:, m, :], in_=ps[:],
                                    func=mybir.ActivationFunctionType.Relu,
                                    bias=b0[:, m : m + 1],
                                )
                        h1a = mlp.tile([128, 8, 512], bf16, tag="h1a", bufs=2)
                        h1c = mlp.tile([128, 8, 512], bf16, tag="h1c", bufs=2)
                        for m in range(8):
                            ps = l1ps.tile([128, 512], f32, space="PSUM", tag="l1")
                            for k in range(8):
                                nc.tensor.matmul(
                                    ps[:],
                                    lhsT=w1a_t[:, k * 8 + m, :],
                                    rhs=h0a[:, k, :],
                                    start=(k == 0), stop=(k == 7),
                                )
                            nc.scalar.activation(
                                out=h1a[:, m, :], in_=ps[:],
                                func=mybir.ActivationFunctionType.Relu,
                                bias=biases["b1a"][:, m : m + 1],
                            )
                        for m in range(8):
                            ps = l1ps.tile([128, 512], f32, space="PSUM", tag="l1")
                            for k in range(8):
                                nc.tensor.matmul(
                                    ps[:],
                                    lhsT=w1c_t[:, k * 8 + m, :],
                                    rhs=h0c[:, k, :],
                                    start=(k == 0), stop=(k == 7),
                                )
                            nc.scalar.activation(
                                out=h1c[:, m, :], in_=ps[:],
                                func=mybir.ActivationFunctionType.Relu,
                                bias=biases["b1c"][:, m : m + 1],
                            )
                        ps2 = l2ps.tile([64, 512], f32, space="PSUM", tag="l2")
                        for k in range(8):
                            nc.tensor.matmul(
                                ps2[0:OUT_D, :], lhsT=w2a_t[:, k, :], rhs=h1a[:, k, :],
                                start=(k == 0), stop=(k == 7),
                            )
                        for k in range(8):
                            nc.tensor.matmul(
                                ps2[32:33, :], lhsT=w2c_t[:, k, :], rhs=h1c[:, k, :],
                                start=(k == 0), stop=(k == 7),
                            )
                        catT = post.tile([64, 512], f32, tag="catT")
                        nc.vector.tensor_scalar(
                            out=catT[0:33, :], in0=ps2[0:33, :],
                            scalar1=b2cat_t[0:33, :], scalar2=None,
                            op0=mybir.AluOpType.add,
                        )
                        if prev_post is not None:
                            do_post(*prev_post)
                        prev_post = (rt, catT)
                    do_post(*prev_post)
                    nc.sync.dma_start(
                        out=alp_out[b, :, :], in_=alp_sb[:, b * NT : (b + 1) * NT]
                    )

                # final partition reduce via ones matmul (reuses a trps slot)
                if True:
                    red = trps.tile([128, 10], f32, space="PSUM", tag="tr")
                    nc.tensor.matmul(
                        red[0:1, :], lhsT=ones_col[:], rhs=acc_all[:],
                        start=True, stop=True,
                    )
                    scal_sb = per.tile([1, 10], f32)
                    nc.vector.tensor_copy(out=scal_sb[:], in_=red[0:1, :])
                    nc.sync.dma_start(out=scal_out[:, :], in_=scal_sb[:])

    tile_patch.fix_waits(nc)
    return nc


def _prep_inputs(ob_x, ob_t, action, src, dst, aW0, ab0, aW1, ab1, aW2, ab2,
                 cW0, cb0, cW1, cb1, cW2, cb2):
    ob_x = np.asarray(ob_x, np.int32)
    ob_t = np.asarray(ob_t, np.float32)
    action = np.asarray(action, np.int32)
    src = np.asarray(src, np.int64)
    dst = np.asarray(dst, np.int64)

    iota32 = np.tile(np.arange(1, 33, dtype=np.int32), (128, 1))
    iota21 = np.tile(np.arange(OUT_D, dtype=np.float32), (128, 1))
    w1a = np.ascontiguousarray(
        np.asarray(aW1, np.float32).reshape(8, 128, 8, 128)
        .transpose(1, 0, 2, 3).reshape(128, 64, 128)).astype(ml_dtypes.bfloat16)
    w1c = np.ascontiguousarray(
        np.asarray(cW1, np.float32).reshape(8, 128, 8, 128)
        .transpose(1, 0, 2, 3).reshape(128, 64, 128)).astype(ml_dtypes.bfloat16)
    w2a = np.ascontiguousarray(
        np.asarray(aW2, np.float32).reshape(8, 128, OUT_D).transpose(1, 0, 2)
    ).astype(ml_dtypes.bfloat16)
    w2c = np.ascontiguousarray(
        np.asarray(cW2, np.float32).reshape(8, 128, 1).transpose(1, 0, 2)
    ).astype(ml_dtypes.bfloat16)
    b0a = np.ascontiguousarray(np.asarray(ab0, np.float32).reshape(8, 128).T)
    b0c = np.ascontiguousarray(np.asarray(cb0, np.float32).reshape(8, 128).T)
    b1a = np.ascontiguousarray(np.asarray(ab1, np.float32).reshape(8, 128).T)
    b1c = np.ascontiguousarray(np.asarray(cb1, np.float32).reshape(8, 128).T)
    b2cat = np.zeros((64, 1), np.float32)
    b2cat[0:OUT_D, 0] = np.asarray(ab2, np.float32)
    b2cat[32, 0] = np.asarray(cb2, np.float32)[0]

    obx_pad = np.zeros((NP, B), np.int32)
    obx_pad[:N] = ob_x
    obx_g = np.ascontiguousarray(
        obx_pad.reshape(KT, 128, B).transpose(1, 0, 2))
    shared = {
        "obx_g": obx_g, "iota32": iota32, "iota21": iota21,
        "w0a": np.asarray(aW0, np.float32), "w0c": np.asarray(cW0, np.float32),
        "w1a": w1a, "w1c": w1c, "w2a": w2a, "w2c": w2c,
        "b0a": b0a, "b0c": b0c, "b1a": b1a, "b1c": b1c, "b2cat": b2cat,
    }

    in_maps = []
    for k in range(NCORES):
        lo, hi = k * NPC, (k + 1) * NPC
        obtT = np.zeros((B + 1, NPCP), np.float32)
        obtT[:B, :NPC] = ob_t[lo:hi, :].T
        obtT[B, :] = 1.0
        obxN = np.ones((B, 128, NT), np.int32)     # pad color 1 -> mask 0
        actN = np.zeros((B, 128, NT), np.int32)
        obx_loc = ob_x[lo:hi].T                    # [B, NPC]
        act_loc = action[lo:hi].T
        padded_x = np.ones((B, NPCP), np.int32)
        padded_x[:, :NPC] = obx_loc
        padded_a = np.zeros((B, NPCP), np.int32)
        padded_a[:, :NPC] = act_loc
        obxN[:] = padded_x.reshape(B, NT, 128).transpose(0, 2, 1)
        actN[:] = padded_a.reshape(B, NT, 128).transpose(0, 2, 1)

        a_all = np.zeros((NBLK, 128, 512), np.uint8)
        for pas, (g_arr, s_arr) in enumerate(((src, dst), (dst, src))):
            selm = (s_arr >= lo) & (s_arr < hi)
            gg = g_arr[selm]
            ss = s_arr[selm] - lo
            dense = np.zeros((NP, NPCP), np.uint8)
            np.add.at(dense, (gg, ss), 1)
            blocks = dense.reshape(KT, 128, NRANGE, 512).transpose(2, 0, 1, 3)
            a_all[pas * NRANGE * KT:(pas + 1) * NRANGE * KT] = blocks.reshape(
                NRANGE * KT, 128, 512)
            del dense
        a_fp8 = _FP8_LUT[a_all]
        del a_all

        m = dict(shared)
        m.update({"obtT": obtT, "obxN": obxN, "actN": actN, "a_all": a_fp8})
        in_maps.append(m)
    return in_maps


def kernel(**inputs):
    global LAST_EXEC_NS
    if "nc" not in _CACHED:
        _CACHED["nc"] = _build_program()
    nc = _CACHED["nc"]
    in_maps = _prep_inputs(**inputs)
    trace = bool(os.environ.get("KBENCH_TRACE"))
    res = run_bass_kernel_spmd(
        nc, in_maps, core_ids=list(range(NCORES)), trace=trace)
    LAST_EXEC_NS = res.exec_time_ns

    alp = np.zeros((N, B), np.float32)
    vsum = np.zeros(B, np.float64)
    esum = 0.0
    msum = 0.0
    for k in range(NCORES):
        out = res.results[k]
        a = out["alp_out"]                     # [B, 128, NT]
        loc = a.transpose(2, 1, 0).reshape(NPCP, B)[:NPC]
        alp[k * NPC:(k + 1) * NPC] = loc
        sc = out["scal_out"][0]
        vsum += sc[0:8].astype(np.float64)
        esum += float(sc[8])
        msum += float(sc[9])
    avg_entropy = np.float32(esum / max(msum, 1.0))
    value_preds = (vsum / MAXN).astype(np.float32)
    return alp, avg_entropy, value_preds


# revision 28
# speedup vs baseline: 1.1234x; 1.0049x over previous
"""Trainium2 Bass kernel for nn_ActorCritic (GNN message passing + actor/critic MLPs).

Sharding: nodes are partitioned across the 8 NeuronCores (2500 nodes each, all
8 batch elements on every core). The one-hop segment_sums (nb, nb_rev) are
computed as dense fp8 matmuls against host-built adjacency blocks:
    nbT[pay, dst] = sum_src X[src, pay] * A[src, dst]
with X = one-hot colors built on device (payload = batch*32 + color, 256 wide,
split into hi/lo 128-partition planes) and A the 0/1 adjacency (pure index
data). The MLPs run with float32r (layer 0) and bfloat16 (layers 1/2) matmuls;
log-softmax / entropy / value reductions run on DVE/ACT after a PE transpose
back to node-major layout. Outputs are gathered and re-assembled on host.
"""
import os
import sys

sys.path.insert(0, "/opt/trn_rl_repo")
sys.path.insert(0, os.path.dirname(os.path.abspath(__file__)))

import numpy as np
import ml_dtypes

import concourse.bass as bass
import concourse.mybir as mybir
import concourse.tile as tile_mod
from concourse.tile import TileContext
from concourse.bass_utils import run_bass_kernel_spmd
from concourse.masks import make_identity
from concourse.vector_clock import ScopedClock


# ---------------------------------------------------------------------------
# Walrus-compat patches: this build rejects >1 sem wait per instruction, and
# the stock TileContext tail drain carries one wait per live logical proc.
# ---------------------------------------------------------------------------

MAX_WAITS = 1


def _patched_drain_and_barrier(self, tick_clock, wait_clock):
    nc = self.nc
    probe = nc.sync.nop()
    wait_clock.add_sem_waits(probe.ins, ScopedClock({None: tick_clock.global_clock}))
    si = probe.ins.sync_info
    waits = list(si.on_wait) if si is not None else []
    if len(waits) > MAX_WAITS:
        si.on_wait = waits[:MAX_WAITS]
        rest = waits[MAX_WAITS:]
        for j in range(0, len(rest), MAX_WAITS):
            n = nc.sync.nop()
            nsi = n.ins.sync_info
            if nsi is None:
                n.ins.sync_info = mybir.SyncInfo(
                    on_update=[], on_wait=rest[j : j + MAX_WAITS]
                )
            else:
                nsi.on_wait = rest[j : j + MAX_WAITS]
    nc.sync.drain()
    nc.all_engine_barrier()
    assert self.sems is not None
    popped = nc._tile_sem_poison_stack.pop()
    assert popped is self._sem_poison
    nc.clear_and_free_semaphores(list(self.sems.allocated().values()))
    nc.all_engine_barrier()


_ws_counter = [0]


def fix_waits(nc, max_waits: int = 1):
    """Post-pass over the finished module: any instruction carrying more than
    ``max_waits`` sem waits gets the excess hoisted onto same-engine NoOps
    inserted immediately before it (this walrus build rejects multi-wait
    instructions at codegen)."""
    for f in nc.m.functions:
        for blk in f.blocks:
            insns = blk.instructions
            out = []
            changed = False
            for ins in insns:
                si = ins.sync_info
                if si is not None and len(si.on_wait) > max_waits:
                    waits = list(si.on_wait)
                    keep = waits[: max_waits]
                    rest = waits[max_waits:]
                    for j in range(0, len(rest), max_waits):
                        _ws_counter[0] += 1
                        nop = mybir.InstNoOp(
                            name=f"WSPLIT-{_ws_counter[0]}",
                            ins=[],
                            outs=[],
                            engine=ins.engine,
                            sync_info=mybir.SyncInfo(
                                on_update=[], on_wait=rest[j : j + max_waits]
                            ),
                        )
                        out.append(nop)
                    si.on_wait = keep
                    changed = True
                out.append(ins)
            if changed:
                blk.instructions = out


def install():
    tile_mod.TileContext._drain_and_barrier = _patched_drain_and_barrier


class _TP:
    fix_waits = staticmethod(fix_waits)


tile_patch = _TP()
install()

# ---- problem constants (hardcoded per spec) ----
N, B, NCOL, H, E = 20000, 8, 20, 1024, 320000
IN_D, OUT_D, MAXN = 42, 21, 20000
NCORES = 8
NPC = N // NCORES            # 2500 nodes per core
NPCP = 2560                  # padded: 20 x 128 = 5 x 512
NT = NPCP // 128             # 20 node tiles per core
NRANGE = NPCP // 512         # 5 psum ranges
KT = 157                     # src k-chunks (ceil(20000/128))
NP = KT * 128                # 20096 padded source nodes
NBLK = 2 * NRANGE * KT       # 1570 adjacency blocks per core

f32 = mybir.dt.float32
f32r = mybir.dt.float32r
bf16 = mybir.dt.bfloat16
fp8 = mybir.dt.float8e4
i32 = mybir.dt.int32

_FP8_LUT = np.arange(256, dtype=np.uint8).astype(np.float32).astype(
    ml_dtypes.float8_e4m3)

LAST_EXEC_NS = None
_CACHED = {}


def _build_program():
    nc = bass.Bass("TRN2")
    p = {}
    p["obx_g"] = nc.declare_dram_parameter("obx_g", [128, KT, B], i32, isOutput=False)
    p["obtT"] = nc.declare_dram_parameter("obtT", [B + 1, NPCP], f32, isOutput=False)
    p["obxN"] = nc.declare_dram_parameter("obxN", [B, 128, NT], i32, isOutput=False)
    p["actN"] = nc.declare_dram_parameter("actN", [B, 128, NT], i32, isOutput=False)
    p["a_all"] = nc.declare_dram_parameter("a_all", [NBLK, 128, 512], fp8, isOutput=False)
    p["iota32"] = nc.declare_dram_parameter("iota32", [128, 32], i32, isOutput=False)
    p["iota21"] = nc.declare_dram_parameter("iota21", [128, 21], f32, isOutput=False)
    p["w0a"] = nc.declare_dram_parameter("w0a", [IN_D, H], f32, isOutput=False)
    p["w0c"] = nc.declare_dram_parameter("w0c", [IN_D, H], f32, isOutput=False)
    p["w1a"] = nc.declare_dram_parameter("w1a", [128, 64, 128], bf16, isOutput=False)
    p["w1c"] = nc.declare_dram_parameter("w1c", [128, 64, 128], bf16, isOutput=False)
    p["w2a"] = nc.declare_dram_parameter("w2a", [128, 8, OUT_D], bf16, isOutput=False)
    p["w2c"] = nc.declare_dram_parameter("w2c", [128, 8, 1], bf16, isOutput=False)
    p["b0a"] = nc.declare_dram_parameter("b0a", [128, 8], f32, isOutput=False)
    p["b0c"] = nc.declare_dram_parameter("b0c", [128, 8], f32, isOutput=False)
    p["b1a"] = nc.declare_dram_parameter("b1a", [128, 8], f32, isOutput=False)
    p["b1c"] = nc.declare_dram_parameter("b1c", [128, 8], f32, isOutput=False)
    p["b2cat"] = nc.declare_dram_parameter("b2cat", [64, 1], f32, isOutput=False)
    alp_out = nc.declare_dram_parameter("alp_out", [B, 128, NT], f32, isOutput=True)
    scal_out = nc.declare_dram_parameter("scal_out", [1, 10], f32, isOutput=True)

    with TileContext(nc) as tc:
        with (
            tc.tile_pool(name="per", bufs=1) as per,       # persistent
            tc.tile_pool(name="nbp", bufs=1) as nbp,       # nb storage
        ):
            # ---- persistent small tiles ----
            iota32_t = per.tile([128, 32], i32)
            nc.sync.dma_start(out=iota32_t[:], in_=p["iota32"][:, :])
            iota21_t = per.tile([128, 21], f32)
            nc.sync.dma_start(out=iota21_t[:], in_=p["iota21"][:, :])
            ident_f32 = per.tile([128, 128], f32)
            make_identity(nc, ident_f32[:])
            ones_col = per.tile([128, 1], f32)
            nc.vector.memset(ones_col[:], 1.0)
            acc_all = per.tile([128, 10], f32)
            nc.vector.memset(acc_all[:], 0.0)
            b2cat_t = per.tile([64, 1], f32)
            nc.sync.dma_start(out=b2cat_t[:], in_=p["b2cat"][:, :])

            # weights
            w0a_t = per.tile([IN_D, H], f32r)
            nc.gpsimd.dma_start(out=w0a_t[:], in_=p["w0a"][:, :])
            w0c_t = per.tile([IN_D, H], f32r)
            nc.gpsimd.dma_start(out=w0c_t[:], in_=p["w0c"][:, :])
            w1a_t = per.tile([128, 64, 128], bf16)
            nc.sync.dma_start(out=w1a_t[:], in_=p["w1a"][:, :, :])
            w1c_t = per.tile([128, 64, 128], bf16)
            nc.sync.dma_start(out=w1c_t[:], in_=p["w1c"][:, :, :])
            w2a_t = per.tile([128, 8, OUT_D], bf16)
            nc.sync.dma_start(out=w2a_t[:], in_=p["w2a"][:, :, :])
            w2c_t = per.tile([128, 8, 1], bf16)
            nc.sync.dma_start(out=w2c_t[:], in_=p["w2c"][:, :, :])
            biases = {}
            for nm in ("b0a", "b0c", "b1a", "b1c"):
                t = per.tile([128, 8], f32, name=nm)
                nc.sync.dma_start(out=t[:], in_=p[nm][:, :])
                biases[nm] = t

            # nb storage: [payload(4b x 32), node] f32
            nb_hi = nbp.tile([128, NPCP], f32)
            nb_lo = nbp.tile([128, NPCP], f32)
            nbr_hi = nbp.tile([128, NPCP], f32)
            nbr_lo = nbp.tile([128, NPCP], f32)
            nbt = {(0, 0): nb_hi, (0, 1): nb_lo, (1, 0): nbr_hi, (1, 1): nbr_lo}

            # ---- phase 0: build one-hot X in SBUF ----
            with tc.tile_pool(name="xp", bufs=1) as xp:
                KS = 80
                x_lo = xp.tile([128, KS, 256], fp8)
                x_hi = xp.tile([128, KT - KS, 256], fp8)

                def x_sb_at(k):
                    return (x_lo, k) if k < KS else (x_hi, k - KS)
                obx_sb = xp.tile([128, KT, 8], i32)
                nc.sync.dma_start(
                    out=obx_sb[:, :KS, :], in_=p["obx_g"][:, :KS, :])
                nc.sync.dma_start(
                    out=obx_sb[:, KS:, :], in_=p["obx_g"][:, KS:, :])
                for t in range(KT):
                    xt, tt = x_sb_at(t)
                    nc.vector.tensor_tensor(
                        out=xt[:, tt, :],
                        in0=obx_sb[:, t, :].rearrange(
                            "p (b one) -> p b one", one=1
                        ).to_broadcast([128, 8, 32]),
                        in1=iota32_t[:, :].rearrange(
                            "p (one j) -> p one j", one=1
                        ).to_broadcast([128, 8, 32]),
                        op=mybir.AluOpType.is_equal,
                    )

                # ---- phase 1: dense scatter matmuls ----
                with (
                    tc.tile_pool(name="ab", bufs=8) as ab,
                    tc.tile_pool(name="scps", bufs=3, space="PSUM") as scps,
                ):
                    for pas in range(2):
                        for r in range(NRANGE):
                            ps_hi = scps.tile([128, 512], f32, space="PSUM", tag="hi")
                            ps_lo = scps.tile([128, 512], f32, space="PSUM", tag="lo")
                            for k0 in range(0, KT, 16):
                                kk = min(16, KT - k0)
                                blk = (pas * NRANGE + r) * KT + k0
                                a_t = ab.tile([128, 16, 512], fp8, tag="a")
                                nc.sync.dma_start(
                                    out=a_t[:, :kk, :],
                                    in_=p["a_all"][blk : blk + kk, :, :].rearrange(
                                        "k q n -> q k n"),
                                )
                                for j0 in range(0, kk, 2):
                                    k0j = k0 + j0
                                    jj = min(2, kk - j0)
                                    if jj == 2:
                                        nc.tensor.matmul(
                                            ps_hi[:],
                                            lhsT=x_sb_at(k0j)[0][:, x_sb_at(k0j)[1] : x_sb_at(k0j)[1] + 2, 0:128],
                                            rhs=a_t[:, j0 : j0 + 2, :],
                                            start=(k0j == 0),
                                            stop=(k0j + 2 == KT),
                                            perf_mode=mybir.MatmulPerfMode.DoubleRow,
                                        )
                                        nc.tensor.matmul(
                                            ps_lo[:],
                                            lhsT=x_sb_at(k0j)[0][:, x_sb_at(k0j)[1] : x_sb_at(k0j)[1] + 2, 128:256],
                                            rhs=a_t[:, j0 : j0 + 2, :],
                                            start=(k0j == 0),
                                            stop=(k0j + 2 == KT),
                                            perf_mode=mybir.MatmulPerfMode.DoubleRow,
                                        )
                                    else:
                                        nc.tensor.matmul(
                                            ps_hi[:],
                                            lhsT=x_sb_at(k0j)[0][:, x_sb_at(k0j)[1], 0:128],
                                            rhs=a_t[:, j0, :],
                                            start=(k0j == 0),
                                            stop=(k0j == KT - 1),
                                        )
                                        nc.tensor.matmul(
                                            ps_lo[:],
                                            lhsT=x_sb_at(k0j)[0][:, x_sb_at(k0j)[1], 128:256],
                                            rhs=a_t[:, j0, :],
                                            start=(k0j == 0),
                                            stop=(k0j == KT - 1),
                                        )
                            nc.vector.tensor_copy(
                                out=nbt[(pas, 0)][:, r * 512 : (r + 1) * 512],
                                in_=ps_hi[:],
                            )
                            nc.vector.tensor_copy(
                                out=nbt[(pas, 1)][:, r * 512 : (r + 1) * 512],
                                in_=ps_lo[:],
                            )

            # ---- phase 2: MLPs + post ----
            with (
                tc.tile_pool(name="mlp", bufs=1) as mlp,
                tc.tile_pool(name="post", bufs=2) as post,
                tc.tile_pool(name="l0ps", bufs=2, space="PSUM") as l0ps,
                tc.tile_pool(name="l1ps", bufs=3, space="PSUM") as l1ps,
                tc.tile_pool(name="l2ps", bufs=1, space="PSUM") as l2ps,
                tc.tile_pool(name="trps", bufs=2, space="PSUM") as trps,
            ):
                alp_sb = mlp.tile([128, B * NT], f32)
                prev_post = None
                for b in range(B):
                    bq = (b % 4) * 32
                    plane = b // 4
                    hT = mlp.tile([IN_D, NPCP], f32r, tag="hT", bufs=2)
                    nc.gpsimd.dma_start(out=hT[0:1, :], in_=p["obtT"][b : b + 1, :])
                    nc.gpsimd.dma_start(
                        out=hT[1:21, :], in_=nbt[(0, plane)][bq : bq + 20, :]
                    )
                    nc.gpsimd.dma_start(
                        out=hT[21:41, :], in_=nbt[(1, plane)][bq : bq + 20, :]
                    )
                    nc.gpsimd.dma_start(out=hT[41:42, :], in_=p["obtT"][B : B + 1, :])

                    maskf = post.tile([128, NT], f32, tag="maskf")
                    obxn = post.tile([128, NT], i32, tag="obxn")
                    nc.sync.dma_start(out=obxn[:], in_=p["obxN"][b, :, :])
                    nc.vector.tensor_scalar(
                        out=maskf[:], in0=obxn[:], scalar1=0, scalar2=None,
                        op0=mybir.AluOpType.is_equal,
                    )
                    actf = post.tile([128, NT], f32, tag="actf")
                    actn = post.tile([128, NT], i32, tag="actn")
                    nc.sync.dma_start(out=actn[:], in_=p["actN"][b, :, :])
                    nc.vector.tensor_copy(out=actf[:], in_=actn[:])
                    msum = post.tile([128, 1], f32, tag="msum")
                    nc.vector.tensor_reduce(
                        out=msum[:], in_=maskf[:], axis=mybir.AxisListType.X,
                        op=mybir.AluOpType.add,
                    )
                    nc.vector.tensor_tensor(
                        out=acc_all[:, 9:10], in0=acc_all[:, 9:10], in1=msum[:],
                        op=mybir.AluOpType.add,
                    )

                    def do_post(rt_p, catT_p, maskf_p, actf_p, b_p):
                        for ntile in range(4):
                            tg = rt_p * 4 + ntile
                            trp = trps.tile([128, 64], f32, space="PSUM", tag="tr")
                            nc.tensor.transpose(
                                out=trp[:],
                                in_=catT_p[:, ntile * 128 : (ntile + 1) * 128],
                                identity=ident_f32[0:64, 0:64],
                            )
                            ln = post.tile([128, 33], f32, tag="ln")
                            nc.vector.tensor_copy(out=ln[:], in_=trp[:, 0:33])
                            lg = ln[:, 0:OUT_D]
                            mx = post.tile([128, 1], f32, tag="mx")
                            nc.vector.tensor_reduce(
                                out=mx[:], in_=lg, axis=mybir.AxisListType.X,
                                op=mybir.AluOpType.max,
                            )
                            nmx = post.tile([128, 1], f32, tag="nmx")
                            nc.vector.tensor_scalar_mul(nmx[:], mx[:], -1.0)
                            ex = post.tile([128, OUT_D], f32, tag="ex")
                            s = post.tile([128, 1], f32, tag="s")
                            nc.scalar.activation(
                                out=ex[:], in_=lg,
                                func=mybir.ActivationFunctionType.Exp,
                                bias=nmx[:], accum_out=s[:],
                            )
                            logs = post.tile([128, 1], f32, tag="logs")
                            nc.scalar.activation(
                                out=logs[:], in_=s[:],
                                func=mybir.ActivationFunctionType.Ln,
                            )
                            sel = post.tile([128, OUT_D], f32, tag="sel")
                            nc.vector.tensor_scalar(
                                out=sel[:], in0=iota21_t[:],
                                scalar1=actf_p[:, tg : tg + 1], scalar2=None,
                                op0=mybir.AluOpType.is_equal,
                            )
                            junk = post.tile([128, OUT_D], f32, tag="junk")
                            asel = post.tile([128, 1], f32, tag="asel")
                            nc.vector.tensor_tensor(
                                out=junk[:], in0=sel[:], in1=lg,
                                op=mybir.AluOpType.mult,
                            )
                            nc.vector.tensor_reduce(
                                out=asel[:], in_=junk[:],
                                axis=mybir.AxisListType.X, op=mybir.AluOpType.add,
                            )
                            junk2 = post.tile([128, OUT_D], f32, tag="junk2")
                            t3 = post.tile([128, 1], f32, tag="t3")
                            nc.vector.tensor_tensor(
                                out=junk2[:], in0=ex[:], in1=lg,
                                op=mybir.AluOpType.mult,
                            )
                            nc.vector.tensor_reduce(
                                out=t3[:], in_=junk2[:],
                                axis=mybir.AxisListType.X, op=mybir.AluOpType.add,
                            )
                            # alp = (asel - mx - logs) * mask
                            alp0 = post.tile([128, 1], f32, tag="alp0")
                            nc.vector.tensor_scalar(
                                out=alp0[:], in0=asel[:], scalar1=mx[:],
                                scalar2=None, op0=mybir.AluOpType.subtract,
                            )
                            nc.vector.tensor_tensor(
                                out=alp0[:], in0=alp0[:], in1=logs[:],
                                op=mybir.AluOpType.subtract,
                            )
                            nc.vector.tensor_tensor(
                                out=alp_sb[:, b_p * NT + tg : b_p * NT + tg + 1],
                                in0=alp0[:], in1=maskf_p[:, tg : tg + 1],
                                op=mybir.AluOpType.mult,
                            )
                            # ent = mx + logs - t3 / s
                            rs = post.tile([128, 1], f32, tag="rs")
                            nc.vector.reciprocal(rs[:], s[:])
                            ent0 = post.tile([128, 1], f32, tag="ent0")
                            nc.vector.tensor_tensor(
                                out=ent0[:], in0=t3[:], in1=rs[:],
                                op=mybir.AluOpType.mult,
                            )
                            nc.vector.tensor_scalar(
                                out=ent0[:], in0=ent0[:], scalar1=-1.0,
                                scalar2=mx[:], op0=mybir.AluOpType.mult,
                                op1=mybir.AluOpType.add,
                            )
                            nc.vector.tensor_tensor(
                                out=ent0[:], in0=ent0[:], in1=logs[:],
                                op=mybir.AluOpType.add,
                            )
                            nc.vector.tensor_tensor(
                                out=ent0[:], in0=ent0[:], in1=maskf_p[:, tg : tg + 1],
                                op=mybir.AluOpType.mult,
                            )
                            nc.vector.tensor_tensor(
                                out=acc_all[:, 8:9], in0=acc_all[:, 8:9],
                                in1=ent0[:], op=mybir.AluOpType.add,
                            )
                            # value
                            vm = post.tile([128, 1], f32, tag="vm")
                            nc.vector.tensor_tensor(
                                out=vm[:], in0=ln[:, 32:33],
                                in1=maskf_p[:, tg : tg + 1], op=mybir.AluOpType.mult,
                            )
                            nc.vector.tensor_tensor(
                                out=acc_all[:, b_p : b_p + 1], in0=acc_all[:, b_p : b_p + 1],
                                in1=vm[:], op=mybir.AluOpType.add,
                            )

                    for rt in range(NRANGE):
                        cs = rt * 512
                        h0a = mlp.tile([128, 8, 512], bf16, tag="h0a", bufs=2)
                        h0c = mlp.tile([128, 8, 512], bf16, tag="h0c", bufs=2)
                        for net, w0t, h0t, b0 in (
                            ("a", w0a_t, h0a, biases["b0a"]),
                            ("c", w0c_t, h0c, biases["b0c"]),
                        ):
                            for m in range(8):
                                ps = l0ps.tile([128, 512], f32, space="PSUM", tag="l0")
                                nc.tensor.matmul(
                                    ps[:],
                                    lhsT=w0t[:, m * 128 : (m + 1) * 128],
                                    rhs=hT[:, cs : cs + 512],
                                    start=True, stop=True,
                                )
                                nc.scalar.activation(
                                    out=h0t[:, m, :], in_=ps[:],
                                    func=mybir.ActivationFunctionType.Relu,
                                    bias=b0[:, m : m + 1],
                                )
                        h1a = mlp.tile([128, 8, 512], bf16, tag="h1a", bufs=2)
                        h1c = mlp.tile([128, 8, 512], bf16, tag="h1c", bufs=2)
                        for m in range(8):
                            ps = l1ps.tile([128, 512], f32, space="PSUM", tag="l1")
                            for k in range(8):
                                nc.tensor.matmul(
                                    ps[:],
                                    lhsT=w1a_t[:, k * 8 + m, :],
                                    rhs=h0a[:, k, :],
                                    start=(k == 0), stop=(k == 7),
                                )
                            nc.scalar.activation(
                                out=h1a[:, m, :], in_=ps[:],
                                func=mybir.ActivationFunctionType.Relu,
                                bias=biases["b1a"][:, m : m + 1],
                            )
                        for m in range(8):
                            ps = l1ps.tile([128, 512], f32, space="PSUM", tag="l1")
                            for k in range(8):
                                nc.tensor.matmul(
                                    ps[:],
                                    lhsT=w1c_t[:, k * 8 + m, :],
                                    rhs=h0c[:, k, :],
                                    start=(k == 0), stop=(k == 7),
                                )
                            nc.scalar.activation(
                                out=h1c[:, m, :], in_=ps[:],
                                func=mybir.ActivationFunctionType.Relu,
                                bias=biases["b1c"][:, m : m + 1],
                            )
                        ps2 = l2ps.tile([64, 512], f32, space="PSUM", tag="l2")
                        for k in range(8):
                            nc.tensor.matmul(
                                ps2[0:OUT_D, :], lhsT=w2a_t[:, k, :], rhs=h1a[:, k, :],
                                start=(k == 0), stop=(k == 7),
                            )
                        for k in range(8):
                            nc.tensor.matmul(
                                ps2[32:33, :], lhsT=w2c_t[:, k, :], rhs=h1c[:, k, :],
                                start=(k == 0), stop=(k == 7),
                            )
                        catT = post.tile([64, 512], f32, tag="catT")
                        nc.vector.tensor_scalar(
                            out=catT[0:33, :], in0=ps2[0:33, :],
                            scalar1=b2cat_t[0:33, :], scalar2=None,
                            op0=mybir.AluOpType.add,
                        )
                        if prev_post is not None:
                            do_post(*prev_post)
                        prev_post = (rt, catT, maskf, actf, b)


                do_post(*prev_post)
                for b in range(B):
                    nc.sync.dma_start(
                        out=alp_out[b, :, :], in_=alp_sb[:, b * NT : (b + 1) * NT]
                    )

                # final partition reduce via ones matmul (reuses a trps slot)
                if True:
                    red = trps.tile([128, 10], f32, space="PSUM", tag="tr")
                    nc.tensor.matmul(
                        red[0:1, :], lhsT=ones_col[:], rhs=acc_all[:],
                        start=True, stop=True,
                    )
                    scal_sb = per.tile([1, 10], f32)
                    nc.vector.tensor_copy(out=scal_sb[:], in_=red[0:1, :])
                    nc.sync.dma_start(out=scal_out[:, :], in_=scal_sb[:])

    tile_patch.fix_waits(nc)
    return nc


def _prep_inputs(ob_x, ob_t, action, src, dst, aW0, ab0, aW1, ab1, aW2, ab2,
                 cW0, cb0, cW1, cb1, cW2, cb2):
    ob_x = np.asarray(ob_x, np.int32)
    ob_t = np.asarray(ob_t, np.float32)
    action = np.asarray(action, np.int32)
    src = np.asarray(src, np.int64)
    dst = np.asarray(dst, np.int64)

    iota32 = np.tile(np.arange(1, 33, dtype=np.int32), (128, 1))
    iota21 = np.tile(np.arange(OUT_D, dtype=np.float32), (128, 1))
    w1a = np.ascontiguousarray(
        np.asarray(aW1, np.float32).reshape(8, 128, 8, 128)
        .transpose(1, 0, 2, 3).reshape(128, 64, 128)).astype(ml_dtypes.bfloat16)
    w1c = np.ascontiguousarray(
        np.asarray(cW1, np.float32).reshape(8, 128, 8, 128)
        .transpose(1, 0, 2, 3).reshape(128, 64, 128)).astype(ml_dtypes.bfloat16)
    w2a = np.ascontiguousarray(
        np.asarray(aW2, np.float32).reshape(8, 128, OUT_D).transpose(1, 0, 2)
    ).astype(ml_dtypes.bfloat16)
    w2c = np.ascontiguousarray(
        np.asarray(cW2, np.float32).reshape(8, 128, 1).transpose(1, 0, 2)
    ).astype(ml_dtypes.bfloat16)
    b0a = np.ascontiguousarray(np.asarray(ab0, np.float32).reshape(8, 128).T)
    b0c = np.ascontiguousarray(np.asarray(cb0, np.float32).reshape(8, 128).T)
    b1a = np.ascontiguousarray(np.asarray(ab1, np.float32).reshape(8, 128).T)
    b1c = np.ascontiguousarray(np.asarray(cb1, np.float32).reshape(8, 128).T)
    b2cat = np.zeros((64, 1), np.float32)
    b2cat[0:OUT_D, 0] = np.asarray(ab2, np.float32)
    b2cat[32, 0] = np.asarray(cb2, np.float32)[0]

    obx_pad = np.zeros((NP, B), np.int32)
    obx_pad[:N] = ob_x
    obx_g = np.ascontiguousarray(
        obx_pad.reshape(KT, 128, B).transpose(1, 0, 2))
    shared = {
        "obx_g": obx_g, "iota32": iota32, "iota21": iota21,
        "w0a": np.asarray(aW0, np.float32), "w0c": np.asarray(cW0, np.float32),
        "w1a": w1a, "w1c": w1c, "w2a": w2a, "w2c": w2c,
        "b0a": b0a, "b0c": b0c, "b1a": b1a, "b1c": b1c, "b2cat": b2cat,
    }

    in_maps = []
    for k in range(NCORES):
        lo, hi = k * NPC, (k + 1) * NPC
        obtT = np.zeros((B + 1, NPCP), np.float32)
        obtT[:B, :NPC] = ob_t[lo:hi, :].T
        obtT[B, :] = 1.0
        obxN = np.ones((B, 128, NT), np.int32)     # pad color 1 -> mask 0
        actN = np.zeros((B, 128, NT), np.int32)
        obx_loc = ob_x[lo:hi].T                    # [B, NPC]
        act_loc = action[lo:hi].T
        padded_x = np.ones((B, NPCP), np.int32)
        padded_x[:, :NPC] = obx_loc
        padded_a = np.zeros((B, NPCP), np.int32)
        padded_a[:, :NPC] = act_loc
        obxN[:] = padded_x.reshape(B, NT, 128).transpose(0, 2, 1)
        actN[:] = padded_a.reshape(B, NT, 128).transpose(0, 2, 1)

        a_all = np.zeros((NBLK, 128, 512), np.uint8)
        for pas, (g_arr, s_arr) in enumerate(((src, dst), (dst, src))):
            selm = (s_arr >= lo) & (s_arr < hi)
            gg = g_arr[selm]
            ss = s_arr[selm] - lo
            dense = np.zeros((NP, NPCP), np.uint8)
            np.add.at(dense, (gg, ss), 1)
            blocks = dense.reshape(KT, 128, NRANGE, 512).transpose(2, 0, 1, 3)
            a_all[pas * NRANGE * KT:(pas + 1) * NRANGE * KT] = blocks.reshape(
                NRANGE * KT, 128, 512)
            del dense
        a_fp8 = _FP8_LUT[a_all]
        del a_all

        m = dict(shared)
        m.update({"obtT": obtT, "obxN": obxN, "actN": actN, "a_all": a_fp8})
        in_maps.append(m)
    return in_maps


def kernel(**inputs):
    global LAST_EXEC_NS
    if "nc" not in _CACHED:
        _CACHED["nc"] = _build_program()
    nc = _CACHED["nc"]
    in_maps = _prep_inputs(**inputs)
    trace = bool(os.environ.get("KBENCH_TRACE"))
    res = run_bass_kernel_spmd(
        nc, in_maps, core_ids=list(range(NCORES)), trace=trace)
    LAST_EXEC_NS = res.exec_time_ns

    alp = np.zeros((N, B), np.float32)
    vsum = np.zeros(B, np.float64)
    esum = 0.0
    msum = 0.0
    for k in range(NCORES):
        out = res.results[k]
        a = out["alp_out"]                     # [B, 128, NT]
        loc = a.transpose(2, 1, 0).reshape(NPCP, B)[:NPC]
        alp[k * NPC:(k + 1) * NPC] = loc
        sc = out["scal_out"][0]
        vsum += sc[0:8].astype(np.float64)
        esum += float(sc[8])
        msum += float(sc[9])
    avg_entropy = np.float32(esum / max(msum, 1.0))
    value_preds = (vsum / MAXN).astype(np.float32)
    return alp, avg_entropy, value_preds
